# revision 1
# baseline (speedup 1.0000x reference)
"""BiLSTM(2-layer) + CRF NLL Trainium2 kernel.

Sharding: data-parallel over batch. B=64 sequences are split 8 per core across
8 NeuronCores; every core runs the full model on its slice and emits a partial
loss scalar; the host sums the 8 partials.

Device layout is fully "transposed": all activations live as
[feature-partitions, batch-in-free] so the LSTM elementwise pipeline runs with
128 active lanes. The CRF forward algorithm runs in linear space
(ea_{t+1} = (exp(trans)^T @ ea_t) * exp(em_t)) with periodic per-sequence
renormalization, which needs only one tiny matmul + one vector multiply per
timestep.
"""

import os
import sys
import numpy as np
import ml_dtypes

sys.path.insert(0, "/opt/trn_rl_repo")

import concourse.bass as bass
import concourse.mybir as mybir
import concourse.tile as tile

dt = mybir.dt
AF = mybir.ActivationFunctionType
bf16 = ml_dtypes.bfloat16

# problem constants
B, T, E, H, K = 64, 256, 768, 384, 9
NC = 8          # cores
BL = B // NC    # batch per core = 8
G = 4 * H       # 1536
NE = E // 128   # 6 input chunks
NH = H // 128   # 3 hidden chunks
NG = G // 128   # 12 gate chunks
BLK = 32        # timesteps per xg block
CRF_S = 8       # CRF renorm period

# permuted gate order: [i, f, o, g] blocks (pytorch order is i, f, g, o)
GATE_PERM = np.concatenate(
    [np.arange(0, H), np.arange(H, 2 * H), np.arange(3 * H, 4 * H), np.arange(2 * H, 3 * H)]
)


def split_waits(nc):
    """Legalize sem waits: the walrus backend in this toolchain accepts at most
    one sync wait per instruction, but Tile attaches one wait per producer
    engine. Hoist all but the last wait onto same-engine NoOps just before the
    instruction (engine streams are sequential, so semantics are unchanged)."""
    import bass_rust

    n_split = 0
    for f in nc.m.functions:
        for blk in f.blocks:
            out = []
            changed = False
            for inst in blk.instructions:
                si = inst.sync_info
                if si is not None and si.on_wait and len(si.on_wait) > 1:
                    waits = list(si.on_wait)
                    for k, w in enumerate(waits[:-1]):
                        nop = mybir.InstNoOp(name=f"{inst.name}_w{k}", ins=[], outs=[])
                        nop.engine = inst.engine
                        nop.sync_info = bass_rust.SyncInfo(on_wait=[w], on_update=[])
                        out.append(nop)
                        n_split += 1
                    inst.sync_info = bass_rust.SyncInfo(
                        on_wait=[waits[-1]], on_update=list(si.on_update or [])
                    )
                    changed = True
                out.append(inst)
            if changed:
                blk.instructions = out
    return n_split


def build_nc(T_=T, legalize=True):
    """Build the single-core Bass program (SPMD: same program on all 8 cores)."""
    nc = bass.Bass(trn_type="TRN2")
    NQ = T_ * BL
    f32 = dt.float32

    xT_d = nc.declare_dram_parameter("xT", [NE, 128, NQ], dt.bfloat16, False)
    w0_d = nc.declare_dram_parameter("w0T", [NE, 128, 2 * G], dt.bfloat16, False)
    w1_d = nc.declare_dram_parameter("w1T", [NE, 128, 2 * G], dt.bfloat16, False)
    whh_d = nc.declare_dram_parameter("whhT", [4, NH, 128, G], dt.bfloat16, False)
    bias_d = nc.declare_dram_parameter("bias", [128, 4 * NG], f32, False)
    wout_d = nc.declare_dram_parameter("woutT", [NE, 128, K], dt.bfloat16, False)
    bout_d = nc.declare_dram_parameter("bout", [K, 1], f32, False)
    oh_d = nc.declare_dram_parameter("ohT", [K, NQ], f32, False)
    crf_d = nc.declare_dram_parameter("crf", [K, 32], f32, False)
    loss_d = nc.declare_dram_parameter("loss", [1, 1], f32, True)

    BLK_ = min(BLK, T_)
    NB = T_ // BLK_
    NTC = min(512, NQ)  # emission matmul free-dim tile

    with tile.TileContext(nc) as tc:
        with (
            tc.tile_pool(name="big", bufs=1) as big,
            tc.tile_pool(name="state", bufs=2) as state,
            tc.tile_pool(name="tmp", bufs=3) as tmp,
            tc.tile_pool(name="xgp", bufs=1) as xgp,
            tc.tile_pool(name="ps", bufs=2, space="PSUM") as ps,
        ):
            # ---- persistent loads ----
            xT = big.tile([128, NE, NQ], dt.bfloat16, tag="xT")
            for ch in range(NE):
                nc.sync.dma_start(xT[:, ch], xT_d[ch])
            bias = big.tile([128, 4 * NG], f32, tag="bias")
            nc.sync.dma_start(bias[:], bias_d[:])
            wout = big.tile([128, NE, K], dt.bfloat16, tag="wout")
            for ch in range(NE):
                nc.sync.dma_start(wout[:, ch], wout_d[ch])
            bout = big.tile([K, 1], f32, tag="bout")
            nc.sync.dma_start(bout[:], bout_d[:])
            # ohT/crf are staged through DVE copies: engine instructions that
            # would otherwise be the first consumer of two DMA queues trip a
            # walrus sync-wait limit, so give each DMA exactly one DVE waiter.
            ohT_raw = big.tile([K, NQ], f32, tag="ohT_raw")
            nc.sync.dma_start(ohT_raw[:], oh_d[:])
            ohT = big.tile([K, NQ], f32, tag="ohT")
            nc.vector.tensor_copy(ohT[:], ohT_raw[:])
            crf_raw = big.tile([K, 32], f32, tag="crf_raw")
            nc.sync.dma_start(crf_raw[:], crf_d[:])
            crf = big.tile([K, 32], f32, tag="crf")
            nc.vector.tensor_copy(crf[:], crf_raw[:])

            h0T = big.tile([128, 2 * NH, T_, BL], dt.bfloat16, tag="h0T")
            h1T = big.tile([128, 2 * NH, T_, BL], dt.bfloat16, tag="h1T")

            # ---- two BiLSTM layers ----
            for layer in range(2):
                wih = big.tile([128, NE, 2 * G], dt.bfloat16, tag="wih")
                w_src = w0_d if layer == 0 else w1_d
                for ch in range(NE):
                    nc.sync.dma_start(wih[:, ch], w_src[ch])
                whh = big.tile([128, 2 * NH, G], dt.bfloat16, tag="whh")
                for d in range(2):
                    for kc in range(NH):
                        nc.sync.dma_start(whh[:, d * NH + kc], whh_d[2 * layer + d, kc])

                h_out = h0T if layer == 0 else h1T

                c_st = [None, None]  # per-direction running cell state tiles
                for blk in range(NB):
                    xg_t = [None, None]
                    for d in range(2):
                        xblk = blk if d == 0 else NB - 1 - blk
                        xg = xgp.tile([128, NG, BLK_ * BL], f32, tag=f"xg{d}")
                        q0 = xblk * BLK_ * BL
                        for j in range(NG):
                            pxg = ps.tile([128, BLK_ * BL], f32, tag="xg", bufs=2)
                            for kc in range(NE):
                                if layer == 0:
                                    rhs = xT[:, kc, q0:q0 + BLK_ * BL]
                                else:
                                    # h0T free dims are (chunk, t, b); chunk kc,
                                    # t-range, all b -> contiguous BLK*BL block
                                    rhs = h0T[:, kc, xblk * BLK_:(xblk + 1) * BLK_, :]
                                nc.tensor.matmul(
                                    pxg[:],
                                    wih[:, kc, d * G + j * 128:d * G + (j + 1) * 128],
                                    rhs,
                                    start=(kc == 0),
                                    stop=(kc == NE - 1),
                                )
                            nc.scalar.add(
                                xg[:, j], pxg[:], bias[:, (2 * layer + d) * NG + j:(2 * layer + d) * NG + j + 1]
                            )
                        xg_t[d] = xg

                    for tl in range(BLK_):
                        for d in range(2):
                            tt = blk * BLK_ + tl if d == 0 else T_ - 1 - (blk * BLK_ + tl)
                            first = blk == 0 and tl == 0
                            dd = str(d)
                            # within-block offset of timestep tt in this dir's xg block
                            u = tl if d == 0 else BLK_ - 1 - tl
                            xg_sl = xg_t[d][:, :, u * BL:(u + 1) * BL]  # [128, NG, BL]

                            if not first:
                                tprev = tt - 1 if d == 0 else tt + 1
                                gp = ps.tile([128, NG, BL], f32, tag=f"g{d}", bufs=2)
                                n_mm = 0
                                for j in range(NG):
                                    for kc in range(NH):
                                        nc.tensor.matmul(
                                            gp[:, j],
                                            whh[:, d * NH + kc, j * 128:(j + 1) * 128],
                                            h_out[:, d * NH + kc, tprev, :],
                                            start=(n_mm == 0),
                                            stop=(n_mm == NG * NH - 1),
                                        )
                                        n_mm += 1
                                pre = tmp.tile([128, NG, BL], f32, tag="pre" + dd)
                                nc.vector.tensor_add(pre[:], gp[:], xg_sl)
                            else:
                                pre = xg_sl

                            s = tmp.tile([128, 3 * NH, BL], f32, tag="s" + dd)
                            nc.scalar.activation(s[:], pre[:, 0:3 * NH], AF.Sigmoid)
                            g = tmp.tile([128, NH, BL], f32, tag="gg" + dd)
                            nc.scalar.activation(g[:], pre[:, 3 * NH:4 * NH], AF.Tanh)

                            cN = state.tile([128, NH, BL], f32, tag="c" + dd)
                            if first:
                                nc.vector.tensor_mul(cN[:], s[:, 0:NH], g[:])
                            else:
                                t1 = tmp.tile([128, NH, BL], f32, tag="t1" + dd)
                                nc.vector.tensor_mul(t1[:], s[:, 0:NH], g[:])
                                t2 = tmp.tile([128, NH, BL], f32, tag="t2" + dd)
                                nc.vector.tensor_mul(t2[:], s[:, NH:2 * NH], c_st[d][:])
                                nc.vector.tensor_add(cN[:], t1[:], t2[:])
                            c_st[d] = cN

                            tc_t = tmp.tile([128, NH, BL], f32, tag="tc" + dd)
                            nc.scalar.activation(tc_t[:], cN[:], AF.Tanh)
                            nc.vector.tensor_mul(
                                h_out[:, d * NH:(d + 1) * NH, tt, :], s[:, 2 * NH:3 * NH], tc_t[:]
                            )

            # ---- emissions: em[k, q] = w_out @ h1 + b_out ----
            em = big.tile([K, NQ], f32, tag="em")
            NT = NQ // NTC
            for nt in range(NT):
                pem = ps.tile([K, NTC], f32, tag="misc", bufs=1)
                for kc in range(NE):
                    nc.tensor.matmul(
                        pem[:],
                        wout[:, kc],
                        h1T[:, kc, nt * (NTC // BL):(nt + 1) * (NTC // BL), :],
                        start=(kc == 0),
                        stop=(kc == NE - 1),
                    )
                nc.scalar.add(em[:, nt * NTC:(nt + 1) * NTC], pem[:], bout[:, 0:1])

            # ---- gold path score (numerator), accumulated per (k, b) ----
            # scr holds elementwise products in (k, b, t) memory order so a
            # free-dim reduce over the innermost t gives per-(k, b) sums.
            scr = big.tile([K, BL, T_], f32, tag="scratch")
            nkb = tmp.tile([K, BL], f32, tag="nkb")
            # em * onehot: inputs iterate (t, b); write transposed to (k, b, t)
            nc.vector.tensor_tensor(
                scr[:].rearrange("k b t -> k t b"),
                em[:], ohT[:], mybir.AluOpType.mult,
            )
            nc.vector.tensor_reduce(
                nkb[:], scr[:], mybir.AxisListType.X, mybir.AluOpType.add
            )

            # transition pairs: A[j, q] = sum_i trans[i, j] * oh[i, q], then
            # dot with oh at t+1; valid for q in [0, NQ-BL)
            NQm = NQ - BL
            scr2 = big.tile([K, BL, T_], f32, tag="scratch2")
            scr2_tb = scr2[:].rearrange("k b t -> k t b")  # [K, T, BL]
            for nt in range((NQm + NTC - 1) // NTC):
                n0 = nt * NTC
                n1 = min(n0 + NTC, NQm)
                pa = ps.tile([K, NTC], f32, tag="misc", bufs=1)
                nc.tensor.matmul(pa[:, 0:n1 - n0], crf[:, 0:K], ohT[:, n0:n1],
                                 start=True, stop=True)
                nc.vector.tensor_tensor(
                    scr2_tb[:, n0 // BL:n1 // BL, :],
                    pa[:, 0:n1 - n0], ohT[:, n0 + BL:n1 + BL],
                    mybir.AluOpType.mult,
                )
            tr_t = tmp.tile([K, BL], f32, tag="trt")
            nc.vector.tensor_reduce(
                tr_t[:], scr2[:, :, 0:T_ - 1], mybir.AxisListType.X, mybir.AluOpType.add
            )
            nc.vector.tensor_add(nkb[:], nkb[:], tr_t[:])
            # reduce over k via ones-matmul, plus start/end transition gold
            # terms folded in as two extra rank-9 contractions -> num [1, BL]
            pnum = ps.tile([1, BL], f32, tag="misc", bufs=1)
            nc.tensor.matmul(pnum[:], crf[:, 22:23], nkb[:], start=True, stop=False)
            nc.tensor.matmul(pnum[:], crf[:, 20:21], ohT[:, 0:BL], start=False, stop=False)
            nc.tensor.matmul(pnum[:], crf[:, 21:22], ohT[:, NQ - BL:NQ], start=False, stop=True)
            num = tmp.tile([1, BL], f32, tag="num")
            nc.vector.tensor_copy(num[:], pnum[:])

            # ---- CRF forward algorithm (denominator), linear space ----
            eem = big.tile([K, NQ], f32, tag="scratch")  # reuses scratch slot
            nc.scalar.activation(eem[:], em[:], AF.Exp)
            ea = state.tile([K, BL], f32, tag="ea")
            nc.vector.tensor_tensor(ea[:], eem[:, 0:BL], crf[:, 18:19].broadcast_to((K, BL)), mybir.AluOpType.mult)
            logc = None
            for t_ in range(1, T_):
                pea = ps.tile([K, BL], f32, tag="crf", bufs=1)
                nc.tensor.matmul(pea[:], crf[:, 9:9 + K], ea[:], start=True, stop=True)
                eaN = state.tile([K, BL], f32, tag="ea")
                nc.vector.tensor_tensor(
                    eaN[:], pea[:], eem[:, t_ * BL:(t_ + 1) * BL], mybir.AluOpType.mult
                )
                ea = eaN
                if t_ % CRF_S == 0:
                    r = tmp.tile([1, BL], f32, tag="crf_r")
                    nc.vector.reciprocal(r[:], ea[0:1, :])
                    # broadcast r across the 9 state partitions via matmul
                    pbc = ps.tile([K, BL], f32, tag="crf", bufs=1)
                    nc.tensor.matmul(pbc[:], crf[0:1, 23:23 + K], r[:],
                                     start=True, stop=True)
                    lg = tmp.tile([1, BL], f32, tag="crf_lg")
                    nc.scalar.activation(lg[:], ea[0:1, :], AF.Ln)
                    eaN2 = state.tile([K, BL], f32, tag="ea")
                    nc.vector.tensor_tensor(eaN2[:], ea[:], pbc[:], mybir.AluOpType.mult)
                    logcN = state.tile([1, BL], f32, tag="logc")
                    if logc is None:
                        nc.vector.tensor_copy(logcN[:], lg[:])
                    else:
                        nc.vector.tensor_add(logcN[:], logc[:], lg[:])
                    logc = logcN
                    ea = eaN2
            pden = ps.tile([1, BL], f32, tag="misc", bufs=1)
            nc.tensor.matmul(pden[:], crf[:, 19:20], ea[:], start=True, stop=True)
            den = tmp.tile([1, BL], f32, tag="den")
            nc.scalar.activation(den[:], pden[:], AF.Ln)
            if logc is not None:
                nc.vector.tensor_add(den[:], den[:], logc[:])

            # ---- loss = sum_b (den - num) ----
            diff = tmp.tile([1, BL], f32, tag="diff")
            nc.vector.tensor_sub(diff[:], den[:], num[:])
            lout = tmp.tile([1, 1], f32, tag="lout")
            nc.vector.tensor_reduce(
                lout[:], diff[:], mybir.AxisListType.X, mybir.AluOpType.add
            )
            nc.sync.dma_start(loss_d[:], lout[:])

    if legalize:
        split_waits(nc)
    nc.finalize()
    return nc


def stage_inputs(inputs, T_=T):
    """Host-side staging: slice/transpose/cast the full inputs into 8 in_maps."""
    NQ = T_ * BL
    x = np.asarray(inputs["embedding"], np.float32)[:, :T_]
    tags = np.asarray(inputs["target_tag"]).astype(np.int64)[:, :T_]

    def pget(name):
        return np.asarray(inputs[name], np.float32)

    # weights (shared across cores)
    def wihT(name):  # [4H, in] -> [in/128, 128, 1536] permuted, bf16
        w = pget(name)[GATE_PERM]  # [1536, in]
        inw = w.shape[1]
        return np.ascontiguousarray(
            w.T.reshape(inw // 128, 128, G)
        ).astype(bf16)

    w0 = np.concatenate([wihT("w_ih_0f"), wihT("w_ih_0b")], axis=2)  # [6,128,3072]
    w1 = np.concatenate([wihT("w_ih_1f"), wihT("w_ih_1b")], axis=2)

    def whhT(name):  # [1536, 384] -> [3, 128, 1536]
        w = pget(name)[GATE_PERM]
        return np.ascontiguousarray(w.T.reshape(NH, 128, G)).astype(bf16)

    whh = np.stack([whhT("w_hh_0f"), whhT("w_hh_0b"), whhT("w_hh_1f"), whhT("w_hh_1b")])

    def biasv(name):  # [1536] -> [128, 12]
        b = pget(name)[GATE_PERM]
        return b.reshape(NG, 128).T

    bias = np.concatenate(
        [biasv("b_0f"), biasv("b_0b"), biasv("b_1f"), biasv("b_1b")], axis=1
    ).astype(np.float32)  # [128, 48]

    wout = np.ascontiguousarray(
        pget("w_out").T.reshape(NE, 128, K)
    ).astype(bf16)
    bout = pget("b_out").reshape(K, 1)

    trans = pget("trans")
    crf_c = np.zeros((K, 32), np.float32)
    crf_c[:, 0:9] = trans
    crf_c[:, 9:18] = np.exp(trans)
    crf_c[:, 18] = np.exp(pget("start_trans"))
    crf_c[:, 19] = np.exp(pget("end_trans"))
    crf_c[:, 20] = pget("start_trans")
    crf_c[:, 21] = pget("end_trans")
    crf_c[:, 22] = 1.0          # ones column: [9,1] lhsT for partition reduce
    crf_c[0, 23:32] = 1.0       # ones row: [1,9] lhsT for free-axis broadcast

    in_maps = []
    for c in range(NC):
        xs = x[c * BL:(c + 1) * BL]  # [8, T, E]
        # xT: [6, 128, T*8] with columns q = t*8 + b
        xTc = np.ascontiguousarray(
            xs.transpose(2, 1, 0).reshape(NE, 128, NQ)
        ).astype(bf16)
        tg = tags[c * BL:(c + 1) * BL]  # [8, T]
        oh = np.zeros((K, T_, BL), np.float32)
        oh[tg.T.reshape(-1), np.repeat(np.arange(T_), BL), np.tile(np.arange(BL), T_)] = 1.0
        ohc = np.ascontiguousarray(oh.reshape(K, NQ))
        in_maps.append(
            dict(
                xT=xTc, w0T=w0, w1T=w1, whhT=whh, bias=bias, woutT=wout,
                bout=bout, ohT=ohc, crf=crf_c,
            )
        )
    return in_maps


_NC_CACHE = {}


def get_nc(T_=T):
    if T_ not in _NC_CACHE:
        _NC_CACHE[T_] = build_nc(T_)
    return _NC_CACHE[T_]


def kernel(**inputs):
    from concourse.bass_utils import run_bass_kernel_spmd

    nc = get_nc(T)
    in_maps = stage_inputs(inputs, T)
    res = run_bass_kernel_spmd(nc, in_maps, list(range(NC)))
    total = np.float32(0.0)
    for r in res.results:
        total += np.float32(r["loss"].reshape(-1)[0])
    return np.asarray(total, dtype=np.float32)



# revision 2
# speedup vs baseline: 1.1491x; 1.1491x over previous
"""BiLSTM(2-layer) + CRF NLL Trainium2 kernel, v2: direction-split sharding.

8 cores = 4 pairs. Pair p owns 16 sequences; core 2p runs the FORWARD
direction of both LSTM layers for those 16 sequences, core 2p+1 the BACKWARD
direction. Backward cores see time-reversed inputs, so every core runs an
identical forward-scan program; all direction asymmetry lives in host staging
(weights, reversed inputs, transposed CRF transitions, swapped start/end).

Between layers the pair exchanges hidden states with a 2-core AllGather
(bounce via DRAM, sent time-reversed so the partner receives data in its own
time order); each core reconstructs the partner's h via
(slot0 + slot1) - own, computed in fp32 so the bf16 cancellation is exact.
Emissions are per-direction partials pair-summed the same way. Each core then
runs the CRF on all 16 sequences and masks the per-sequence losses so each
sequence is counted on exactly one core.

vs v1: the serial recurrent matmul chain per core drops from 36864 LDW+MM
pairs (N=8) to 18360 (N=16), and input-projection/emission matmuls are
interleaved into the recurrence as PE filler during the per-step elementwise
tails. Gate chunks are reordered [g, i, f, o] so tanh(g)/sigmoid(i,f) start
before the step's matmuls finish and only sigmoid(o) trails them.
"""

import sys
import numpy as np
import ml_dtypes

sys.path.insert(0, "/opt/trn_rl_repo")

import concourse.bass as bass
import concourse.mybir as mybir
import concourse.tile as tile

dt = mybir.dt
AF = mybir.ActivationFunctionType
bf16 = ml_dtypes.bfloat16

# problem constants
B, T, E, H, K = 64, 256, 768, 384, 9
NC = 8
BL = 16         # sequences per core (one direction)
G = 4 * H       # 1536
NE = 6          # input contract chunks (768/128, both layers)
NH = H // 128   # 3
NG = G // 128   # 12
BLK = 32        # timesteps per xg block
NB = T // BLK   # 8
NQ = T * BL     # 4096
CRF_S = 8

# gate chunk order [g, i, f, o] (pytorch order is i, f, g, o); g first so
# tanh(g) can start after 9 of the 36 recurrent matmuls, o last so only
# sigmoid(o) + one mul trail the step's final matmul.
GATE_PERM = np.concatenate(
    [np.arange(2 * H, 3 * H), np.arange(0, H), np.arange(H, 2 * H), np.arange(3 * H, 4 * H)]
)


def split_waits(nc):
    """Legalize sem waits: walrus accepts at most one sync wait per
    instruction; hoist extra waits onto same-engine NoOps."""
    import bass_rust

    n_split = 0
    for f in nc.m.functions:
        for blk in f.blocks:
            out = []
            changed = False
            for inst in blk.instructions:
                si = inst.sync_info
                if si is not None and si.on_wait and len(si.on_wait) > 1:
                    waits = list(si.on_wait)
                    for k, w in enumerate(waits[:-1]):
                        nop = mybir.InstNoOp(name=f"{inst.name}_w{k}", ins=[], outs=[])
                        nop.engine = inst.engine
                        nop.sync_info = bass_rust.SyncInfo(on_wait=[w], on_update=[])
                        out.append(nop)
                        n_split += 1
                    inst.sync_info = bass_rust.SyncInfo(
                        on_wait=[waits[-1]], on_update=list(si.on_update or [])
                    )
                    changed = True
                out.append(inst)
            if changed:
                blk.instructions = out
    return n_split


def rev_slice(a, b):
    """slice covering [a, b) traversed in reverse order."""
    return slice(b - 1, None if a == 0 else a - 1, -1)


def build_nc(legalize=True):
    nc = bass.Bass(trn_type="TRN2", num_devices=NC)
    f32 = dt.float32
    groups = [[2 * p, 2 * p + 1] for p in range(NC // 2)]

    xT_d = nc.declare_dram_parameter("xT", [NE, 128, T, BL], dt.bfloat16, False)
    w0_d = nc.declare_dram_parameter("w0T", [NE, 128, G], dt.bfloat16, False)
    w1_d = nc.declare_dram_parameter("w1T", [NE, 128, G], dt.bfloat16, False)
    whh_d = nc.declare_dram_parameter("whhT", [2, NH, 128, G], dt.bfloat16, False)
    bias_d = nc.declare_dram_parameter("bias", [128, 2 * NG], f32, False)
    wout_d = nc.declare_dram_parameter("woutT", [NH, 128, K], dt.bfloat16, False)
    bout_d = nc.declare_dram_parameter("bout", [K, 1], f32, False)
    oh_d = nc.declare_dram_parameter("ohT", [K, T, BL], f32, False)
    crf_d = nc.declare_dram_parameter("crf", [K, 32], f32, False)
    lmask_d = nc.declare_dram_parameter("lmask", [1, BL], f32, False)
    loss_d = nc.declare_dram_parameter("loss", [1, 1], f32, True)

    RB = 2 * BLK  # h1 ring length (timesteps); emissions drain a block behind

    with tile.TileContext(nc) as tc:
        with (
            tc.tile_pool(name="big", bufs=1) as big,
            tc.tile_pool(name="state", bufs=2) as state,
            tc.tile_pool(name="tmp", bufs=3) as tmp,
            tc.tile_pool(name="xgp", bufs=1) as xgp,
            tc.tile_pool(name="ps", bufs=2, space="PSUM") as ps,
            tc.tile_pool(name="dram", bufs=1, space="DRAM") as dram,
        ):
            # ---- persistent loads ----
            xT = big.tile([128, NE, T, BL], dt.bfloat16, tag="xT")
            for ch in range(NE):
                nc.sync.dma_start(xT[:, ch], xT_d[ch])
            bias = big.tile([128, 2 * NG], f32, tag="bias")
            nc.sync.dma_start(bias[:], bias_d[:])
            wout = big.tile([128, NH, K], dt.bfloat16, tag="wout")
            for ch in range(NH):
                nc.sync.dma_start(wout[:, ch], wout_d[ch])
            bout = big.tile([K, 1], f32, tag="bout")
            nc.sync.dma_start(bout[:], bout_d[:])
            # stage via DVE copies (single-DMA-queue-consumer rule)
            ohT_raw = big.tile([K, T, BL], f32, tag="em")  # slot later: u, em
            nc.sync.dma_start(ohT_raw[:], oh_d[:])
            ohT = big.tile([K, T, BL], f32, tag="ohT")
            nc.vector.tensor_copy(ohT[:], ohT_raw[:])
            crf_raw = big.tile([K, 32], f32, tag="crf_raw")
            nc.sync.dma_start(crf_raw[:], crf_d[:])
            crf = big.tile([K, 32], f32, tag="crf")
            nc.vector.tensor_copy(crf[:], crf_raw[:])
            lmask_raw = big.tile([1, BL], f32, tag="lmask_raw")
            nc.sync.dma_start(lmask_raw[:], lmask_d[:])
            lmask = big.tile([1, BL], f32, tag="lmask")
            nc.vector.tensor_copy(lmask[:], lmask_raw[:])

            h0 = big.tile([128, NH, T, BL], dt.bfloat16, tag="h0")
            h1r = big.tile([128, NH, RB, BL], dt.bfloat16, tag="h1r")

            # DRAM bounce buffers for the pairwise exchanges
            b0_in = dram.tile([128, NH, T, BL], dt.bfloat16, tag="b0_in")
            b0_out = nc.dram_tensor("b0_out", [2, 128, NH, T, BL], dt.bfloat16)
            b1_in = dram.tile([K, T, BL], f32, tag="b1_in")
            b1_out = nc.dram_tensor("b1_out", [2, K, T, BL], f32)

            em = None  # allocated after the h0 exchange (shares slot with u)

            class XgEmitter:
                """Incrementally emits the input-projection matmuls for one
                32-step block (12 gate chunks x 6 contract chunks) so they can
                be interleaved into the recurrence as PE filler."""

                def __init__(self, layer, blk, xg_tile, wih):
                    self.layer, self.blk, self.xg, self.wih = layer, blk, xg_tile, wih
                    self.j, self.kc, self.p = 0, 0, None

                def rhs(self, kc):
                    sl = slice(self.blk * BLK, (self.blk + 1) * BLK)
                    if self.layer == 0:
                        return xT[:, kc, sl, :]
                    if kc < NH:
                        return h0[:, kc, sl, :]
                    return xT[:, kc - NH, sl, :]  # partner h0 lives in xT[:, 0:3]

                def step(self):
                    if self.j >= NG:
                        return False
                    if self.kc == 0:
                        self.p = ps.tile([128, BLK * BL], dt.float32, tag="pxg", bufs=2)
                    j = self.j
                    nc.tensor.matmul(
                        self.p[:],
                        self.wih[:, self.kc, j * 128:(j + 1) * 128],
                        self.rhs(self.kc),
                        start=(self.kc == 0),
                        stop=(self.kc == NE - 1),
                    )
                    self.kc += 1
                    if self.kc == NE:
                        bcol = self.layer * NG + j
                        nc.scalar.add(self.xg[:, j], self.p[:], bias[:, bcol:bcol + 1])
                        self.kc = 0
                        self.j += 1
                    return True

                def drain(self):
                    while self.step():
                        pass

            # ---- two LSTM layers (one direction each; SPMD over cores) ----
            for layer in range(2):
                wih = big.tile([128, NE, G], dt.bfloat16, tag="wih")
                w_src = w0_d if layer == 0 else w1_d
                for ch in range(NE):
                    nc.sync.dma_start(wih[:, ch], w_src[ch])
                whh = big.tile([128, NH, G], dt.bfloat16, tag="whh")
                for kc in range(NH):
                    nc.sync.dma_start(whh[:, kc], whh_d[layer, kc])

                if layer == 1:
                    em = big.tile([K, T, BL], f32, tag="em")

                def h_chunk(t, kc):
                    if layer == 0:
                        return h0[:, kc, t, :]
                    return h1r[:, kc, t % RB, :]

                def h_full(t):
                    if layer == 0:
                        return h0[:, :, t, :]
                    return h1r[:, :, t % RB, :]

                xg_cur = xgp.tile([128, NG, BLK * BL], dt.bfloat16, tag="xg", bufs=2)
                em0 = XgEmitter(layer, 0, xg_cur, wih)
                em0.drain()

                c_st = None
                for blk in range(NB):
                    if blk + 1 < NB:
                        xg_nxt = xgp.tile(
                            [128, NG, BLK * BL], dt.bfloat16, tag="xg", bufs=2
                        )
                        nxt = XgEmitter(layer, blk + 1, xg_nxt, wih)
                    else:
                        xg_nxt, nxt = None, None

                    for tl in range(BLK):
                        t = blk * BLK + tl
                        first = t == 0
                        u0 = tl * BL
                        xg_sl = xg_cur[:, :, u0:u0 + BL]  # [128, NG, BL]

                        if not first:
                            gp = ps.tile([128, NG, BL], f32, tag="gp", bufs=2)
                            n = 0
                            for j in range(NG):
                                for kc in range(NH):
                                    nc.tensor.matmul(
                                        gp[:, j],
                                        whh[:, kc, j * 128:(j + 1) * 128],
                                        h_chunk(t - 1, kc),
                                        start=(n == 0),
                                        stop=(n == NG * NH - 1),
                                    )
                                    n += 1
                            pre_g = tmp.tile([128, NH, BL], f32, tag="pre_g")
                            nc.vector.tensor_add(pre_g[:], gp[:, 0:NH], xg_sl[:, 0:NH])
                            pre_if = tmp.tile([128, 2 * NH, BL], f32, tag="pre_if")
                            nc.vector.tensor_add(
                                pre_if[:], gp[:, NH:3 * NH], xg_sl[:, NH:3 * NH]
                            )
                            pre_o = tmp.tile([128, NH, BL], f32, tag="pre_o")
                            nc.vector.tensor_add(
                                pre_o[:], gp[:, 3 * NH:4 * NH], xg_sl[:, 3 * NH:4 * NH]
                            )
                        else:
                            pre_g = xg_sl[:, 0:NH]
                            pre_if = xg_sl[:, NH:3 * NH]
                            pre_o = xg_sl[:, 3 * NH:4 * NH]

                        tg = tmp.tile([128, NH, BL], f32, tag="tg")
                        nc.scalar.activation(tg[:], pre_g, AF.Tanh)
                        sif = tmp.tile([128, 2 * NH, BL], f32, tag="sif")
                        nc.scalar.activation(sif[:], pre_if, AF.Sigmoid)

                        cN = state.tile([128, NH, BL], f32, tag="c")
                        if first:
                            nc.vector.tensor_mul(cN[:], sif[:, 0:NH], tg[:])
                        else:
                            t1 = tmp.tile([128, NH, BL], f32, tag="t1")
                            nc.vector.tensor_mul(t1[:], sif[:, 0:NH], tg[:])
                            t2 = tmp.tile([128, NH, BL], f32, tag="t2")
                            nc.vector.tensor_mul(t2[:], sif[:, NH:2 * NH], c_st[:])
                            nc.vector.tensor_add(cN[:], t1[:], t2[:])
                        c_st = cN

                        tc_t = tmp.tile([128, NH, BL], f32, tag="tc")
                        nc.scalar.activation(tc_t[:], cN[:], AF.Tanh)
                        so = tmp.tile([128, NH, BL], f32, tag="so")
                        nc.scalar.activation(so[:], pre_o, AF.Sigmoid)
                        nc.vector.tensor_mul(h_full(t), so[:], tc_t[:])

                        # PE filler: next block's input projections
                        if nxt is not None:
                            for _ in range(3):
                                nxt.step()

                    if nxt is not None:
                        nxt.drain()
                        xg_cur = xg_nxt

                    if layer == 0:
                        # send this h0 block time-reversed into the bounce
                        # (per chunk: DMA APs are limited to 3 dims)
                        rsl = rev_slice(T - (blk + 1) * BLK, T - blk * BLK)
                        for c in range(NH):
                            nc.sync.dma_start(
                                b0_in[:, c, rsl, :],
                                h0[:, c, blk * BLK:(blk + 1) * BLK, :],
                            )
                    else:
                        # emissions for the ring block just completed
                        r0 = (blk % 2) * BLK
                        pem = ps.tile([K, BLK, BL], f32, tag="pem", bufs=1)
                        for kc in range(NH):
                            nc.tensor.matmul(
                                pem[:],
                                wout[:, kc],
                                h1r[:, kc, r0:r0 + BLK, :],
                                start=(kc == 0),
                                stop=(kc == NH - 1),
                            )
                        nc.scalar.add(
                            em[:, blk * BLK:(blk + 1) * BLK, :], pem[:], bout[:, 0:1]
                        )

                if layer == 0:
                    # ---- pairwise h0 exchange ----
                    nc.gpsimd.collective_compute(
                        "AllGather",
                        mybir.AluOpType.bypass,
                        replica_groups=groups,
                        ins=[b0_in[:].opt()],
                        outs=[b0_out[:].opt()],
                    )
                    # partner h0 = (slot0 + slot1) - own(reversed); fp32 sum
                    # makes the bf16 cancellation exact
                    for c in range(NH):
                        s0c = big.tile([128, T, BL], dt.bfloat16, tag="XC")
                        nc.sync.dma_start(s0c[:], b0_out[0, :, c])
                        s1c = big.tile([128, T, BL], dt.bfloat16, tag="XD")
                        nc.sync.dma_start(s1c[:], b0_out[1, :, c])
                        u = big.tile([128, T, BL], f32, tag="em")
                        nc.vector.tensor_add(u[:], s0c[:], s1c[:])
                        nc.vector.tensor_sub(xT[:, c], u[:], h0[:, c, ::-1, :])

            # ---- emissions exchange: em_full = own partial + partner partial ----
            nc.sync.dma_start(b1_in[:, ::-1, :], em[:])
            nc.gpsimd.collective_compute(
                "AllGather",
                mybir.AluOpType.bypass,
                replica_groups=groups,
                ins=[b1_in[:].opt()],
                outs=[b1_out[:].opt()],
            )
            s0e = big.tile([K, T, BL], f32, tag="wih")
            nc.sync.dma_start(s0e[:], b1_out[0])
            s1e = big.tile([K, T, BL], f32, tag="XC")
            nc.sync.dma_start(s1e[:], b1_out[1])
            nc.vector.tensor_add(s0e[:], s0e[:], s1e[:])
            em_rev = big.tile([K, T, BL], f32, tag="whh")
            nc.vector.tensor_copy(em_rev[:], em[:, ::-1, :])
            nc.vector.tensor_sub(em[:], s0e[:], em_rev[:])

            em_flat = em[:].rearrange("k t b -> k (t b)")
            oh_flat = ohT[:].rearrange("k t b -> k (t b)")

            # ---- gold path score (numerator) ----
            scr = big.tile([K, BL, T], f32, tag="wih")
            nkb = tmp.tile([K, BL], f32, tag="nkb")
            nc.vector.tensor_tensor(
                scr[:].rearrange("k b t -> k t b"),
                em[:], ohT[:], mybir.AluOpType.mult,
            )
            nc.vector.tensor_reduce(
                nkb[:], scr[:], mybir.AxisListType.X, mybir.AluOpType.add
            )
            # transition pairs: A[j, q] = sum_i trans[i, j] oh[i, q], dot oh[q+BL]
            NTC = 512
            NQm = NQ - BL
            scr2 = big.tile([K, BL, T], f32, tag="whh")
            scr2_tb = scr2[:].rearrange("k b t -> k t b")  # [K, T, BL]
            for nt in range((NQm + NTC - 1) // NTC):
                n0 = nt * NTC
                n1 = min(n0 + NTC, NQm)
                pa = ps.tile([K, NTC], f32, tag="misc", bufs=1)
                nc.tensor.matmul(pa[:, 0:n1 - n0], crf[:, 0:K], oh_flat[:, n0:n1],
                                 start=True, stop=True)
                nc.vector.tensor_tensor(
                    scr2_tb[:, n0 // BL:n1 // BL, :],
                    pa[:, 0:n1 - n0], oh_flat[:, n0 + BL:n1 + BL],
                    mybir.AluOpType.mult,
                )
            tr_t = tmp.tile([K, BL], f32, tag="trt")
            nc.vector.tensor_reduce(
                tr_t[:], scr2[:, :, 0:T - 1], mybir.AxisListType.X, mybir.AluOpType.add
            )
            nc.vector.tensor_add(nkb[:], nkb[:], tr_t[:])
            pnum = ps.tile([1, BL], f32, tag="misc", bufs=1)
            nc.tensor.matmul(pnum[:], crf[:, 22:23], nkb[:], start=True, stop=False)
            nc.tensor.matmul(pnum[:], crf[:, 20:21], ohT[:, 0, :], start=False, stop=False)
            nc.tensor.matmul(pnum[:], crf[:, 21:22], ohT[:, T - 1, :], start=False, stop=True)
            num = tmp.tile([1, BL], f32, tag="num")
            nc.vector.tensor_copy(num[:], pnum[:])

            # ---- CRF forward algorithm (denominator), linear space ----
            eem = big.tile([K, T, BL], f32, tag="wih")
            nc.scalar.activation(eem[:], em[:], AF.Exp)
            ea = state.tile([K, BL], f32, tag="ea")
            nc.vector.tensor_tensor(
                ea[:], eem[:, 0, :], crf[:, 18:19].broadcast_to((K, BL)),
                mybir.AluOpType.mult,
            )
            logc = None
            for t_ in range(1, T):
                pea = ps.tile([K, BL], f32, tag="crf", bufs=1)
                nc.tensor.matmul(pea[:], crf[:, 9:9 + K], ea[:], start=True, stop=True)
                eaN = state.tile([K, BL], f32, tag="ea")
                nc.vector.tensor_tensor(
                    eaN[:], pea[:], eem[:, t_, :], mybir.AluOpType.mult
                )
                ea = eaN
                if t_ % CRF_S == 0:
                    r = tmp.tile([1, BL], f32, tag="crf_r")
                    nc.vector.reciprocal(r[:], ea[0:1, :])
                    pbc = ps.tile([K, BL], f32, tag="crf", bufs=1)
                    nc.tensor.matmul(pbc[:], crf[0:1, 23:23 + K], r[:],
                                     start=True, stop=True)
                    lg = tmp.tile([1, BL], f32, tag="crf_lg")
                    nc.scalar.activation(lg[:], ea[0:1, :], AF.Ln)
                    eaN2 = state.tile([K, BL], f32, tag="ea")
                    nc.vector.tensor_tensor(eaN2[:], ea[:], pbc[:], mybir.AluOpType.mult)
                    logcN = state.tile([1, BL], f32, tag="logc")
                    if logc is None:
                        nc.vector.tensor_copy(logcN[:], lg[:])
                    else:
                        nc.vector.tensor_add(logcN[:], logc[:], lg[:])
                    logc = logcN
                    ea = eaN2
            pden = ps.tile([1, BL], f32, tag="misc", bufs=1)
            nc.tensor.matmul(pden[:], crf[:, 19:20], ea[:], start=True, stop=True)
            den = tmp.tile([1, BL], f32, tag="den")
            nc.scalar.activation(den[:], pden[:], AF.Ln)
            if logc is not None:
                nc.vector.tensor_add(den[:], den[:], logc[:])

            # ---- loss = sum_b mask_b * (den_b - num_b) ----
            diff = tmp.tile([1, BL], f32, tag="diff")
            nc.vector.tensor_sub(diff[:], den[:], num[:])
            nc.vector.tensor_mul(diff[:], diff[:], lmask[:])
            lout = tmp.tile([1, 1], f32, tag="lout")
            nc.vector.tensor_reduce(
                lout[:], diff[:], mybir.AxisListType.X, mybir.AluOpType.add
            )
            nc.sync.dma_start(loss_d[:], lout[:])

    if legalize:
        split_waits(nc)
    nc.finalize()
    return nc


def stage_inputs(inputs):
    x = np.asarray(inputs["embedding"], np.float32)
    tags = np.asarray(inputs["target_tag"]).astype(np.int64)

    def pget(name):
        return np.asarray(inputs[name], np.float32)

    def wihT(name, row_order=None):
        w = pget(name)[GATE_PERM]            # [1536, in]
        wT = w.T                             # [in, 1536]
        if row_order is not None:
            wT = wT[row_order]
        return np.ascontiguousarray(wT).reshape(-1, 128, G).astype(bf16)

    def whhT(name):
        w = pget(name)[GATE_PERM]
        return np.ascontiguousarray(w.T).reshape(NH, 128, G).astype(bf16)

    def biasv(name):
        return pget(name)[GATE_PERM].reshape(NG, 128).T

    trans, st, et = pget("trans"), pget("start_trans"), pget("end_trans")
    w_out, b_out = pget("w_out"), pget("b_out")

    in_maps = []
    for c in range(NC):
        p, par = divmod(c, 2)
        d = "f" if par == 0 else "b"
        xs = x[16 * p:16 * p + 16]
        tg = tags[16 * p:16 * p + 16]
        if par:
            xs = xs[:, ::-1]
            tg = tg[:, ::-1]
        xT_c = np.ascontiguousarray(xs.transpose(2, 1, 0)).reshape(
            NE, 128, T, BL).astype(bf16)

        w0 = wihT(f"w_ih_0{d}")
        own = np.arange(0, H) if par == 0 else np.arange(H, 2 * H)
        oth = np.arange(H, 2 * H) if par == 0 else np.arange(0, H)
        w1 = wihT(f"w_ih_1{d}", row_order=np.concatenate([own, oth]))
        whh = np.stack([whhT(f"w_hh_0{d}"), whhT(f"w_hh_1{d}")])
        bias = np.concatenate([biasv(f"b_0{d}"), biasv(f"b_1{d}")], axis=1).astype(
            np.float32)
        wh = w_out[:, 0:H] if par == 0 else w_out[:, H:2 * H]
        woutT = np.ascontiguousarray(wh.T).reshape(NH, 128, K).astype(bf16)
        bout = (b_out if par == 0 else np.zeros(K, np.float32)).reshape(K, 1)

        oh = np.zeros((K, T, BL), np.float32)
        oh[tg.T.reshape(-1), np.repeat(np.arange(T), BL), np.tile(np.arange(BL), T)] = 1.0

        tr_eff = trans if par == 0 else np.ascontiguousarray(trans.T)
        st_eff = st if par == 0 else et
        et_eff = et if par == 0 else st
        crf_c = np.zeros((K, 32), np.float32)
        crf_c[:, 0:9] = tr_eff
        crf_c[:, 9:18] = np.exp(tr_eff)
        crf_c[:, 18] = np.exp(st_eff)
        crf_c[:, 19] = np.exp(et_eff)
        crf_c[:, 20] = st_eff
        crf_c[:, 21] = et_eff
        crf_c[:, 22] = 1.0
        crf_c[0, 23:32] = 1.0
        lm = np.zeros((1, BL), np.float32)
        if par == 0:
            lm[0, 0:8] = 1.0
        else:
            lm[0, 8:16] = 1.0

        in_maps.append(
            dict(
                xT=xT_c, w0T=w0, w1T=w1, whhT=whh, bias=bias, woutT=woutT,
                bout=bout, ohT=np.ascontiguousarray(oh), crf=crf_c, lmask=lm,
            )
        )
    return in_maps


_NC_CACHE = {}


def get_nc():
    if "nc" not in _NC_CACHE:
        _NC_CACHE["nc"] = build_nc()
    return _NC_CACHE["nc"]


def kernel(**inputs):
    from concourse.bass_utils import run_bass_kernel_spmd

    nc = get_nc()
    in_maps = stage_inputs(inputs)
    res = run_bass_kernel_spmd(nc, in_maps, list(range(NC)))
    total = np.float32(0.0)
    for r in res.results:
        total += np.float32(r["loss"].reshape(-1)[0])
    return np.asarray(total, dtype=np.float32)


# revision 3
# speedup vs baseline: 1.1726x; 1.0205x over previous
"""BiLSTM(2-layer) + CRF NLL Trainium2 kernel, v2: direction-split sharding.

8 cores = 4 pairs. Pair p owns 16 sequences; core 2p runs the FORWARD
direction of both LSTM layers for those 16 sequences, core 2p+1 the BACKWARD
direction. Backward cores see time-reversed inputs, so every core runs an
identical forward-scan program; all direction asymmetry lives in host staging
(weights, reversed inputs, transposed CRF transitions, swapped start/end).

Between layers the pair exchanges hidden states with a 2-core AllGather
(bounce via DRAM, sent time-reversed so the partner receives data in its own
time order); each core reconstructs the partner's h via
(slot0 + slot1) - own, computed in fp32 so the bf16 cancellation is exact.
Emissions are per-direction partials pair-summed the same way. Each core then
runs the CRF on all 16 sequences and masks the per-sequence losses so each
sequence is counted on exactly one core.

vs v1: the serial recurrent matmul chain per core drops from 36864 LDW+MM
pairs (N=8) to 18360 (N=16), and input-projection/emission matmuls are
interleaved into the recurrence as PE filler during the per-step elementwise
tails. Gate chunks are reordered [g, i, f, o] so tanh(g)/sigmoid(i,f) start
before the step's matmuls finish and only sigmoid(o) trails them.
"""

import sys
import numpy as np
import ml_dtypes

sys.path.insert(0, "/opt/trn_rl_repo")

import concourse.bass as bass
import concourse.mybir as mybir
import concourse.tile as tile

dt = mybir.dt
AF = mybir.ActivationFunctionType
bf16 = ml_dtypes.bfloat16

# problem constants
B, T, E, H, K = 64, 256, 768, 384, 9
NC = 8
BL = 16         # sequences per core (one direction)
G = 4 * H       # 1536
NE = 6          # input contract chunks (768/128, both layers)
NH = H // 128   # 3
NG = G // 128   # 12
BLK = 32        # timesteps per xg block
NB = T // BLK   # 8
NQ = T * BL     # 4096
CRF_S = 8

# gate chunk order [i, f, g, o] = native pytorch order. The step's matmuls
# run as three PSUM-bank groups (i+f, g, o) so each activation starts as soon
# as its bank's accumulation retires, overlapping the rest of the matmul
# stream; o last so only sigmoid(o) + one mul trail the final matmul.
GATE_PERM = np.arange(4 * H)


def split_waits(nc):
    """Legalize sem waits: walrus accepts at most one sync wait per
    instruction; hoist extra waits onto same-engine NoOps."""
    import bass_rust

    n_split = 0
    for f in nc.m.functions:
        for blk in f.blocks:
            out = []
            changed = False
            for inst in blk.instructions:
                si = inst.sync_info
                if si is not None and si.on_wait and len(si.on_wait) > 1:
                    waits = list(si.on_wait)
                    for k, w in enumerate(waits[:-1]):
                        nop = mybir.InstNoOp(name=f"{inst.name}_w{k}", ins=[], outs=[])
                        nop.engine = inst.engine
                        nop.sync_info = bass_rust.SyncInfo(on_wait=[w], on_update=[])
                        out.append(nop)
                        n_split += 1
                    inst.sync_info = bass_rust.SyncInfo(
                        on_wait=[waits[-1]], on_update=list(si.on_update or [])
                    )
                    changed = True
                out.append(inst)
            if changed:
                blk.instructions = out
    return n_split


def rev_slice(a, b):
    """slice covering [a, b) traversed in reverse order."""
    return slice(b - 1, None if a == 0 else a - 1, -1)


def build_nc(legalize=True):
    nc = bass.Bass(trn_type="TRN2", num_devices=NC)
    f32 = dt.float32
    groups = [[2 * p, 2 * p + 1] for p in range(NC // 2)]

    xT_d = nc.declare_dram_parameter("xT", [NE, 128, T, BL], dt.bfloat16, False)
    w0_d = nc.declare_dram_parameter("w0T", [NE, 128, G], dt.bfloat16, False)
    w1_d = nc.declare_dram_parameter("w1T", [NE, 128, G], dt.bfloat16, False)
    whh_d = nc.declare_dram_parameter("whhT", [2, NH, 128, G], dt.bfloat16, False)
    bias_d = nc.declare_dram_parameter("bias", [128, 2 * NG], f32, False)
    wout_d = nc.declare_dram_parameter("woutT", [NH, 128, K], dt.bfloat16, False)
    bout_d = nc.declare_dram_parameter("bout", [K, 1], f32, False)
    oh_d = nc.declare_dram_parameter("ohT", [K, T, BL], f32, False)
    id_d = nc.declare_dram_parameter("ident", [128, 128], dt.bfloat16, False)
    crf_d = nc.declare_dram_parameter("crf", [K, 32], f32, False)
    lmask_d = nc.declare_dram_parameter("lmask", [1, BL], f32, False)
    loss_d = nc.declare_dram_parameter("loss", [1, 1], f32, True)

    RB = 2 * BLK  # h1 ring length (timesteps); emissions drain a block behind

    with tile.TileContext(nc) as tc:
        with (
            tc.tile_pool(name="big", bufs=1) as big,
            tc.tile_pool(name="state", bufs=2) as state,
            tc.tile_pool(name="tmp", bufs=3) as tmp,
            tc.tile_pool(name="xgp", bufs=1) as xgp,
            tc.tile_pool(name="ps", bufs=2, space="PSUM") as ps,
            tc.tile_pool(name="dram", bufs=1, space="DRAM") as dram,
        ):
            # ---- persistent loads ----
            xT = big.tile([128, NE, T, BL], dt.bfloat16, tag="xT")
            for ch in range(NE):
                nc.sync.dma_start(xT[:, ch], xT_d[ch])
            bias = big.tile([128, 2 * NG], f32, tag="bias")
            nc.sync.dma_start(bias[:], bias_d[:])
            wout = big.tile([128, NH, K], dt.bfloat16, tag="wout")
            for ch in range(NH):
                nc.sync.dma_start(wout[:, ch], wout_d[ch])
            bout = big.tile([K, 1], f32, tag="bout")
            nc.sync.dma_start(bout[:], bout_d[:])
            ident = big.tile([128, 128], dt.bfloat16, tag="ident")
            nc.sync.dma_start(ident[:], id_d[:])
            # stage via DVE copies (single-DMA-queue-consumer rule)
            ohT_raw = big.tile([K, T, BL], f32, tag="em")  # slot later: u, em
            nc.sync.dma_start(ohT_raw[:], oh_d[:])
            ohT = big.tile([K, T, BL], f32, tag="ohT")
            nc.vector.tensor_copy(ohT[:], ohT_raw[:])
            crf_raw = big.tile([K, 32], f32, tag="crf_raw")
            nc.sync.dma_start(crf_raw[:], crf_d[:])
            crf = big.tile([K, 32], f32, tag="crf")
            nc.vector.tensor_copy(crf[:], crf_raw[:])
            lmask_raw = big.tile([1, BL], f32, tag="lmask_raw")
            nc.sync.dma_start(lmask_raw[:], lmask_d[:])
            lmask = big.tile([1, BL], f32, tag="lmask")
            nc.vector.tensor_copy(lmask[:], lmask_raw[:])

            h0 = big.tile([128, NH, T, BL], dt.bfloat16, tag="h0")
            h1r = big.tile([128, NH, RB, BL], dt.bfloat16, tag="h1r")

            # DRAM bounce buffers for the pairwise exchanges
            b0_in = dram.tile([128, NH, T, BL], dt.bfloat16, tag="b0_in")
            b0_out = nc.dram_tensor("b0_out", [2, 128, NH, T, BL], dt.bfloat16)
            b1_in = dram.tile([K, T, BL], f32, tag="b1_in")
            b1_out = nc.dram_tensor("b1_out", [2, K, T, BL], f32)

            em = None  # allocated after the h0 exchange (shares slot with u)

            class XgEmitter:
                """Incrementally emits the input-projection matmuls for one
                32-step block (12 gate chunks x 6 contract chunks) so they can
                be interleaved into the recurrence as PE filler."""

                def __init__(self, layer, blk, xg_tile, wih):
                    self.layer, self.blk, self.xg, self.wih = layer, blk, xg_tile, wih
                    self.j, self.kc, self.p = 0, 0, None

                def rhs(self, kc):
                    sl = slice(self.blk * BLK, (self.blk + 1) * BLK)
                    if self.layer == 0:
                        return xT[:, kc, sl, :]
                    if kc < NH:
                        return h0[:, kc, sl, :]
                    return xT[:, kc - NH, sl, :]  # partner h0 lives in xT[:, 0:3]

                def step(self):
                    if self.j >= NG:
                        return False
                    if self.kc == 0:
                        self.p = ps.tile([128, BLK * BL], dt.float32, tag="pxg", bufs=2)
                    j = self.j
                    nc.tensor.matmul(
                        self.p[:],
                        self.wih[:, self.kc, j * 128:(j + 1) * 128],
                        self.rhs(self.kc),
                        start=(self.kc == 0),
                        stop=(self.kc == NE - 1),
                    )
                    self.kc += 1
                    if self.kc == NE:
                        bcol = self.layer * NG + j
                        nc.scalar.add(self.xg[:, j], self.p[:], bias[:, bcol:bcol + 1])
                        self.kc = 0
                        self.j += 1
                    return True

                def drain(self):
                    while self.step():
                        pass

            # ---- two LSTM layers (one direction each; SPMD over cores) ----
            for layer in range(2):
                wih = big.tile([128, NE, G], dt.bfloat16, tag="wih")
                w_src = w0_d if layer == 0 else w1_d
                for ch in range(NE):
                    nc.sync.dma_start(wih[:, ch], w_src[ch])
                whh = big.tile([128, NH, G], dt.bfloat16, tag="whh")
                for kc in range(NH):
                    nc.sync.dma_start(whh[:, kc], whh_d[layer, kc])

                if layer == 1:
                    em = big.tile([K, T, BL], f32, tag="em")

                def h_chunk(t, kc):
                    if layer == 0:
                        return h0[:, kc, t, :]
                    return h1r[:, kc, t % RB, :]

                def h_full(t):
                    if layer == 0:
                        return h0[:, :, t, :]
                    return h1r[:, :, t % RB, :]

                xg_cur = xgp.tile([128, NG, BLK * BL], dt.bfloat16, tag="xg", bufs=2)
                em0 = XgEmitter(layer, 0, xg_cur, wih)
                em0.drain()

                c_st = None
                for blk in range(NB):
                    if blk + 1 < NB:
                        xg_nxt = xgp.tile(
                            [128, NG, BLK * BL], dt.bfloat16, tag="xg", bufs=2
                        )
                        nxt = XgEmitter(layer, blk + 1, xg_nxt, wih)
                    else:
                        xg_nxt, nxt = None, None

                    for tl in range(BLK):
                        t = blk * BLK + tl
                        first = t == 0
                        u0 = tl * BL

                        # Gate pre-activations land in three separate PSUM
                        # banks (i+f, g, o). Each bank's group: recurrent
                        # whh matmuls plus one identity-matmul per gate chunk
                        # that injects xg (incl. bias) straight into PSUM —
                        # no DVE pre-adds, and each activation reads its bank
                        # as soon as that group retires while the PE streams
                        # the next group.
                        gp_if = ps.tile([128, 2 * NH, BL], f32, tag="gp_if", bufs=1)
                        gp_g = ps.tile([128, NH, BL], f32, tag="gp_g", bufs=1)
                        gp_o = ps.tile([128, NH, BL], f32, tag="gp_o", bufs=1)

                        def emit_group(tile_, j0, nj):
                            total = nj * (1 if first else NH + 1)
                            n = 0
                            for jj in range(nj):
                                j = j0 + jj
                                if not first:
                                    for kc in range(NH):
                                        nc.tensor.matmul(
                                            tile_[:, jj],
                                            whh[:, kc, j * 128:(j + 1) * 128],
                                            h_chunk(t - 1, kc),
                                            start=(n == 0),
                                            stop=(n == total - 1),
                                        )
                                        n += 1
                                nc.tensor.matmul(
                                    tile_[:, jj],
                                    ident[:],
                                    xg_cur[:, j, u0:u0 + BL],
                                    start=(n == 0),
                                    stop=(n == total - 1),
                                )
                                n += 1

                        emit_group(gp_if, 0, 2 * NH)
                        emit_group(gp_g, 2 * NH, NH)
                        emit_group(gp_o, 3 * NH, NH)

                        sif = tmp.tile([128, 2 * NH, BL], f32, tag="sif")
                        nc.scalar.activation(sif[:], gp_if[:], AF.Sigmoid)
                        tg = tmp.tile([128, NH, BL], f32, tag="tg")
                        nc.scalar.activation(tg[:], gp_g[:], AF.Tanh)
                        so = tmp.tile([128, NH, BL], f32, tag="so")
                        nc.scalar.activation(so[:], gp_o[:], AF.Sigmoid)

                        cN = state.tile([128, NH, BL], f32, tag="c")
                        if first:
                            nc.vector.tensor_mul(cN[:], sif[:, 0:NH], tg[:])
                        else:
                            t2 = tmp.tile([128, NH, BL], f32, tag="t2")
                            nc.vector.tensor_mul(t2[:], sif[:, NH:2 * NH], c_st[:])
                            t1 = tmp.tile([128, NH, BL], f32, tag="t1")
                            nc.vector.tensor_mul(t1[:], sif[:, 0:NH], tg[:])
                            nc.vector.tensor_add(cN[:], t1[:], t2[:])
                        c_st = cN

                        tc_t = tmp.tile([128, NH, BL], f32, tag="tc")
                        nc.scalar.activation(tc_t[:], cN[:], AF.Tanh)
                        nc.vector.tensor_mul(h_full(t), so[:], tc_t[:])

                        # PE filler: next block's input projections
                        if nxt is not None:
                            for _ in range(3):
                                nxt.step()

                    if nxt is not None:
                        nxt.drain()
                        xg_cur = xg_nxt

                    if layer == 0:
                        # send this h0 block time-reversed into the bounce
                        # (per chunk: DMA APs are limited to 3 dims)
                        rsl = rev_slice(T - (blk + 1) * BLK, T - blk * BLK)
                        for c in range(NH):
                            nc.sync.dma_start(
                                b0_in[:, c, rsl, :],
                                h0[:, c, blk * BLK:(blk + 1) * BLK, :],
                            )
                    else:
                        # emissions for the ring block just completed
                        r0 = (blk % 2) * BLK
                        pem = ps.tile([K, BLK, BL], f32, tag="pem", bufs=1)
                        for kc in range(NH):
                            nc.tensor.matmul(
                                pem[:],
                                wout[:, kc],
                                h1r[:, kc, r0:r0 + BLK, :],
                                start=(kc == 0),
                                stop=(kc == NH - 1),
                            )
                        nc.scalar.add(
                            em[:, blk * BLK:(blk + 1) * BLK, :], pem[:], bout[:, 0:1]
                        )

                if layer == 0:
                    # ---- pairwise h0 exchange ----
                    nc.gpsimd.collective_compute(
                        "AllGather",
                        mybir.AluOpType.bypass,
                        replica_groups=groups,
                        ins=[b0_in[:].opt()],
                        outs=[b0_out[:].opt()],
                    )
                    # partner h0 = (slot0 + slot1) - own(reversed); fp32 sum
                    # makes the bf16 cancellation exact
                    for c in range(NH):
                        s0c = big.tile([128, T, BL], dt.bfloat16, tag="XC")
                        nc.sync.dma_start(s0c[:], b0_out[0, :, c])
                        s1c = big.tile([128, T, BL], dt.bfloat16, tag="XD")
                        nc.sync.dma_start(s1c[:], b0_out[1, :, c])
                        u = big.tile([128, T, BL], f32, tag="em")
                        nc.vector.tensor_add(u[:], s0c[:], s1c[:])
                        nc.vector.tensor_sub(xT[:, c], u[:], h0[:, c, ::-1, :])

            # ---- emissions exchange: em_full = own partial + partner partial ----
            nc.sync.dma_start(b1_in[:, ::-1, :], em[:])
            nc.gpsimd.collective_compute(
                "AllGather",
                mybir.AluOpType.bypass,
                replica_groups=groups,
                ins=[b1_in[:].opt()],
                outs=[b1_out[:].opt()],
            )
            # transition pairs (depends only on ohT/crf — overlaps the
            # collective): A[j, q] = sum_i trans[i, j] oh[i, q], dot oh[q+BL]
            oh_flat = ohT[:].rearrange("k t b -> k (t b)")
            NTC = 512
            NQm = NQ - BL
            scr2 = big.tile([K, BL, T], f32, tag="whh")
            scr2_tb = scr2[:].rearrange("k b t -> k t b")  # [K, T, BL]
            for nt in range((NQm + NTC - 1) // NTC):
                n0 = nt * NTC
                n1 = min(n0 + NTC, NQm)
                pa = ps.tile([K, NTC], f32, tag="misc", bufs=1)
                nc.tensor.matmul(pa[:, 0:n1 - n0], crf[:, 0:K], oh_flat[:, n0:n1],
                                 start=True, stop=True)
                nc.vector.tensor_tensor(
                    scr2_tb[:, n0 // BL:n1 // BL, :],
                    pa[:, 0:n1 - n0], oh_flat[:, n0 + BL:n1 + BL],
                    mybir.AluOpType.mult,
                )
            tr_t = tmp.tile([K, BL], f32, tag="trt")
            nc.vector.tensor_reduce(
                tr_t[:], scr2[:, :, 0:T - 1], mybir.AxisListType.X, mybir.AluOpType.add
            )

            s0e = big.tile([K, T, BL], f32, tag="wih")
            nc.sync.dma_start(s0e[:], b1_out[0])
            s1e = big.tile([K, T, BL], f32, tag="XC")
            nc.sync.dma_start(s1e[:], b1_out[1])
            nc.vector.tensor_add(s0e[:], s0e[:], s1e[:])
            em_rev = big.tile([K, T, BL], f32, tag="whh")
            nc.vector.tensor_copy(em_rev[:], em[:, ::-1, :])
            nc.vector.tensor_sub(em[:], s0e[:], em_rev[:])

            # ---- gold path score (numerator) ----
            scr = big.tile([K, BL, T], f32, tag="wih")
            nkb = tmp.tile([K, BL], f32, tag="nkb")
            nc.vector.tensor_tensor(
                scr[:].rearrange("k b t -> k t b"),
                em[:], ohT[:], mybir.AluOpType.mult,
            )
            nc.vector.tensor_reduce(
                nkb[:], scr[:], mybir.AxisListType.X, mybir.AluOpType.add
            )
            nc.vector.tensor_add(nkb[:], nkb[:], tr_t[:])
            pnum = ps.tile([1, BL], f32, tag="misc", bufs=1)
            nc.tensor.matmul(pnum[:], crf[:, 22:23], nkb[:], start=True, stop=False)
            nc.tensor.matmul(pnum[:], crf[:, 20:21], ohT[:, 0, :], start=False, stop=False)
            nc.tensor.matmul(pnum[:], crf[:, 21:22], ohT[:, T - 1, :], start=False, stop=True)
            num = tmp.tile([1, BL], f32, tag="num")
            nc.vector.tensor_copy(num[:], pnum[:])

            # ---- CRF forward algorithm (denominator), linear space ----
            eem = big.tile([K, T, BL], f32, tag="wih")
            nc.scalar.activation(eem[:], em[:], AF.Exp)
            ea = state.tile([K, BL], f32, tag="ea")
            nc.vector.tensor_tensor(
                ea[:], eem[:, 0, :], crf[:, 18:19].broadcast_to((K, BL)),
                mybir.AluOpType.mult,
            )
            logc = None
            for t_ in range(1, T):
                pea = ps.tile([K, BL], f32, tag="crf", bufs=1)
                nc.tensor.matmul(pea[:], crf[:, 9:9 + K], ea[:], start=True, stop=True)
                eaN = state.tile([K, BL], f32, tag="ea")
                nc.vector.tensor_tensor(
                    eaN[:], pea[:], eem[:, t_, :], mybir.AluOpType.mult
                )
                ea = eaN
                if t_ % CRF_S == 0:
                    r = tmp.tile([1, BL], f32, tag="crf_r")
                    nc.vector.reciprocal(r[:], ea[0:1, :])
                    pbc = ps.tile([K, BL], f32, tag="crf", bufs=1)
                    nc.tensor.matmul(pbc[:], crf[0:1, 23:23 + K], r[:],
                                     start=True, stop=True)
                    lg = tmp.tile([1, BL], f32, tag="crf_lg")
                    nc.scalar.activation(lg[:], ea[0:1, :], AF.Ln)
                    eaN2 = state.tile([K, BL], f32, tag="ea")
                    nc.vector.tensor_tensor(eaN2[:], ea[:], pbc[:], mybir.AluOpType.mult)
                    logcN = state.tile([1, BL], f32, tag="logc")
                    if logc is None:
                        nc.vector.tensor_copy(logcN[:], lg[:])
                    else:
                        nc.vector.tensor_add(logcN[:], logc[:], lg[:])
                    logc = logcN
                    ea = eaN2
            pden = ps.tile([1, BL], f32, tag="misc", bufs=1)
            nc.tensor.matmul(pden[:], crf[:, 19:20], ea[:], start=True, stop=True)
            den = tmp.tile([1, BL], f32, tag="den")
            nc.scalar.activation(den[:], pden[:], AF.Ln)
            if logc is not None:
                nc.vector.tensor_add(den[:], den[:], logc[:])

            # ---- loss = sum_b mask_b * (den_b - num_b) ----
            diff = tmp.tile([1, BL], f32, tag="diff")
            nc.vector.tensor_sub(diff[:], den[:], num[:])
            nc.vector.tensor_mul(diff[:], diff[:], lmask[:])
            lout = tmp.tile([1, 1], f32, tag="lout")
            nc.vector.tensor_reduce(
                lout[:], diff[:], mybir.AxisListType.X, mybir.AluOpType.add
            )
            nc.sync.dma_start(loss_d[:], lout[:])

    if legalize:
        split_waits(nc)
    nc.finalize()
    return nc


def stage_inputs(inputs):
    x = np.asarray(inputs["embedding"], np.float32)
    tags = np.asarray(inputs["target_tag"]).astype(np.int64)

    def pget(name):
        return np.asarray(inputs[name], np.float32)

    def wihT(name, row_order=None):
        w = pget(name)[GATE_PERM]            # [1536, in]
        wT = w.T                             # [in, 1536]
        if row_order is not None:
            wT = wT[row_order]
        return np.ascontiguousarray(wT).reshape(-1, 128, G).astype(bf16)

    def whhT(name):
        w = pget(name)[GATE_PERM]
        return np.ascontiguousarray(w.T).reshape(NH, 128, G).astype(bf16)

    def biasv(name):
        return pget(name)[GATE_PERM].reshape(NG, 128).T

    trans, st, et = pget("trans"), pget("start_trans"), pget("end_trans")
    w_out, b_out = pget("w_out"), pget("b_out")

    in_maps = []
    for c in range(NC):
        p, par = divmod(c, 2)
        d = "f" if par == 0 else "b"
        xs = x[16 * p:16 * p + 16]
        tg = tags[16 * p:16 * p + 16]
        if par:
            xs = xs[:, ::-1]
            tg = tg[:, ::-1]
        xT_c = np.ascontiguousarray(xs.transpose(2, 1, 0)).reshape(
            NE, 128, T, BL).astype(bf16)

        w0 = wihT(f"w_ih_0{d}")
        own = np.arange(0, H) if par == 0 else np.arange(H, 2 * H)
        oth = np.arange(H, 2 * H) if par == 0 else np.arange(0, H)
        w1 = wihT(f"w_ih_1{d}", row_order=np.concatenate([own, oth]))
        whh = np.stack([whhT(f"w_hh_0{d}"), whhT(f"w_hh_1{d}")])
        bias = np.concatenate([biasv(f"b_0{d}"), biasv(f"b_1{d}")], axis=1).astype(
            np.float32)
        wh = w_out[:, 0:H] if par == 0 else w_out[:, H:2 * H]
        woutT = np.ascontiguousarray(wh.T).reshape(NH, 128, K).astype(bf16)
        bout = (b_out if par == 0 else np.zeros(K, np.float32)).reshape(K, 1)

        oh = np.zeros((K, T, BL), np.float32)
        oh[tg.T.reshape(-1), np.repeat(np.arange(T), BL), np.tile(np.arange(BL), T)] = 1.0

        tr_eff = trans if par == 0 else np.ascontiguousarray(trans.T)
        st_eff = st if par == 0 else et
        et_eff = et if par == 0 else st
        crf_c = np.zeros((K, 32), np.float32)
        crf_c[:, 0:9] = tr_eff
        crf_c[:, 9:18] = np.exp(tr_eff)
        crf_c[:, 18] = np.exp(st_eff)
        crf_c[:, 19] = np.exp(et_eff)
        crf_c[:, 20] = st_eff
        crf_c[:, 21] = et_eff
        crf_c[:, 22] = 1.0
        crf_c[0, 23:32] = 1.0
        lm = np.zeros((1, BL), np.float32)
        if par == 0:
            lm[0, 0:8] = 1.0
        else:
            lm[0, 8:16] = 1.0

        in_maps.append(
            dict(
                xT=xT_c, w0T=w0, w1T=w1, whhT=whh, bias=bias, woutT=woutT,
                bout=bout, ohT=np.ascontiguousarray(oh), crf=crf_c, lmask=lm,
                ident=np.eye(128, dtype=bf16),
            )
        )
    return in_maps


_NC_CACHE = {}


def get_nc():
    if "nc" not in _NC_CACHE:
        _NC_CACHE["nc"] = build_nc()
    return _NC_CACHE["nc"]


def kernel(**inputs):
    from concourse.bass_utils import run_bass_kernel_spmd

    nc = get_nc()
    in_maps = stage_inputs(inputs)
    res = run_bass_kernel_spmd(nc, in_maps, list(range(NC)))
    total = np.float32(0.0)
    for r in res.results:
        total += np.float32(r["loss"].reshape(-1)[0])
    return np.asarray(total, dtype=np.float32)


# revision 4
# speedup vs baseline: 1.2837x; 1.0947x over previous
"""BiLSTM(2-layer) + CRF NLL Trainium2 kernel, v2: direction-split sharding.

8 cores = 4 pairs. Pair p owns 16 sequences; core 2p runs the FORWARD
direction of both LSTM layers for those 16 sequences, core 2p+1 the BACKWARD
direction. Backward cores see time-reversed inputs, so every core runs an
identical forward-scan program; all direction asymmetry lives in host staging
(weights, reversed inputs, transposed CRF transitions, swapped start/end).

Between layers the pair exchanges hidden states with a 2-core AllGather
(bounce via DRAM, sent time-reversed so the partner receives data in its own
time order); each core reconstructs the partner's h via
(slot0 + slot1) - own, computed in fp32 so the bf16 cancellation is exact.
Emissions are per-direction partials pair-summed the same way. Each core then
runs the CRF on all 16 sequences and masks the per-sequence losses so each
sequence is counted on exactly one core.

vs v1: the serial recurrent matmul chain per core drops from 36864 LDW+MM
pairs (N=8) to 18360 (N=16), and input-projection/emission matmuls are
interleaved into the recurrence as PE filler during the per-step elementwise
tails. Gate chunks are reordered [g, i, f, o] so tanh(g)/sigmoid(i,f) start
before the step's matmuls finish and only sigmoid(o) trails them.
"""

import sys
import numpy as np
import ml_dtypes

sys.path.insert(0, "/opt/trn_rl_repo")

import concourse.bass as bass
import concourse.mybir as mybir
import concourse.tile as tile

dt = mybir.dt
AF = mybir.ActivationFunctionType
bf16 = ml_dtypes.bfloat16

# problem constants
B, T, E, H, K = 64, 256, 768, 384, 9
NC = 8
BL = 16         # sequences per core (one direction)
G = 4 * H       # 1536
NE = 6          # input contract chunks (768/128, both layers)
NH = H // 128   # 3
NG = G // 128   # 12
BLK = 32        # timesteps per xg block
NB = T // BLK   # 8
NQ = T * BL     # 4096
CRF_S = 8

# gate chunk order [i, f, g, o] = native pytorch order. The step's matmuls
# run as three PSUM-bank groups (i+f, g, o) so each activation starts as soon
# as its bank's accumulation retires, overlapping the rest of the matmul
# stream; o last so only sigmoid(o) + one mul trail the final matmul.
GATE_PERM = np.arange(4 * H)


def split_waits(nc):
    """Legalize sem waits: walrus accepts at most one sync wait per
    instruction; hoist extra waits onto same-engine NoOps."""
    import bass_rust

    n_split = 0
    for f in nc.m.functions:
        for blk in f.blocks:
            out = []
            changed = False
            for inst in blk.instructions:
                si = inst.sync_info
                if si is not None and si.on_wait and len(si.on_wait) > 1:
                    waits = list(si.on_wait)
                    for k, w in enumerate(waits[:-1]):
                        nop = mybir.InstNoOp(name=f"{inst.name}_w{k}", ins=[], outs=[])
                        nop.engine = inst.engine
                        nop.sync_info = bass_rust.SyncInfo(on_wait=[w], on_update=[])
                        out.append(nop)
                        n_split += 1
                    inst.sync_info = bass_rust.SyncInfo(
                        on_wait=[waits[-1]], on_update=list(si.on_update or [])
                    )
                    changed = True
                out.append(inst)
            if changed:
                blk.instructions = out
    return n_split


def rev_slice(a, b):
    """slice covering [a, b) traversed in reverse order."""
    return slice(b - 1, None if a == 0 else a - 1, -1)


def build_nc(legalize=True):
    nc = bass.Bass(trn_type="TRN2", num_devices=NC)
    f32 = dt.float32
    groups = [[2 * p, 2 * p + 1] for p in range(NC // 2)]

    xT_d = nc.declare_dram_parameter("xT", [NE, 128, T, BL], dt.bfloat16, False)
    w0_d = nc.declare_dram_parameter("w0T", [NE, 128, G], dt.bfloat16, False)
    w1_d = nc.declare_dram_parameter("w1T", [NE, 128, G], dt.bfloat16, False)
    whh_d = nc.declare_dram_parameter("whhT", [2, NH, 128, G], dt.float8e4, False)
    bias_d = nc.declare_dram_parameter("bias", [128, 2 * NG], f32, False)
    wout_d = nc.declare_dram_parameter("woutT", [NH, 128, K], dt.bfloat16, False)
    bout_d = nc.declare_dram_parameter("bout", [K, 1], f32, False)
    oh_d = nc.declare_dram_parameter("ohT", [K, T, BL], f32, False)
    id_d = nc.declare_dram_parameter("ident", [128, 128], dt.bfloat16, False)
    crf_d = nc.declare_dram_parameter("crf", [K, 32], f32, False)
    lmask_d = nc.declare_dram_parameter("lmask", [1, BL], f32, False)
    loss_d = nc.declare_dram_parameter("loss", [1, 1], f32, True)

    RB = 2 * BLK  # h1 ring length (timesteps); emissions drain a block behind

    with tile.TileContext(nc) as tc:
        with (
            tc.tile_pool(name="big", bufs=1) as big,
            tc.tile_pool(name="state", bufs=2) as state,
            tc.tile_pool(name="tmp", bufs=3) as tmp,
            tc.tile_pool(name="xgp", bufs=1) as xgp,
            tc.tile_pool(name="ps", bufs=2, space="PSUM") as ps,
            tc.tile_pool(name="dram", bufs=1, space="DRAM") as dram,
        ):
            # ---- persistent loads ----
            xT = big.tile([128, NE, T, BL], dt.bfloat16, tag="xT")
            for ch in range(NE):
                nc.sync.dma_start(xT[:, ch], xT_d[ch])
            bias = big.tile([128, 2 * NG], f32, tag="bias")
            nc.sync.dma_start(bias[:], bias_d[:])
            wout = big.tile([128, NH, K], dt.bfloat16, tag="wout")
            for ch in range(NH):
                nc.sync.dma_start(wout[:, ch], wout_d[ch])
            bout = big.tile([K, 1], f32, tag="bout")
            nc.sync.dma_start(bout[:], bout_d[:])
            ident = big.tile([128, 128], dt.bfloat16, tag="ident")
            nc.sync.dma_start(ident[:], id_d[:])
            # stage via DVE copies (single-DMA-queue-consumer rule)
            ohT_raw = big.tile([K, T, BL], f32, tag="em")  # slot later: u, em
            nc.sync.dma_start(ohT_raw[:], oh_d[:])
            ohT = big.tile([K, T, BL], f32, tag="ohT")
            nc.vector.tensor_copy(ohT[:], ohT_raw[:])
            crf_raw = big.tile([K, 32], f32, tag="crf_raw")
            nc.sync.dma_start(crf_raw[:], crf_d[:])
            crf = big.tile([K, 32], f32, tag="crf")
            nc.vector.tensor_copy(crf[:], crf_raw[:])
            lmask_raw = big.tile([1, BL], f32, tag="lmask_raw")
            nc.sync.dma_start(lmask_raw[:], lmask_d[:])
            lmask = big.tile([1, BL], f32, tag="lmask")
            nc.vector.tensor_copy(lmask[:], lmask_raw[:])

            h0 = big.tile([128, NH, T, BL], dt.bfloat16, tag="h0")
            h1r = big.tile([128, NH, RB, BL], dt.bfloat16, tag="h1r")
            # fp8 copy of the two most recent h vectors: recurrent matmul
            # operands (whh is fp8; contraction over 384 averages out the
            # wider fp8 quantization noise)
            h8 = big.tile([128, NH, 2, BL], dt.float8e4, tag="h8")

            # DRAM bounce buffers for the pairwise exchanges. h0 is exchanged
            # in two halves: the half the partner needs LAST is sent at our
            # midpoint (its transfer hides under our remaining compute); only
            # the second half's latency is exposed at the layer boundary.
            b0A_in = dram.tile([128, NH, T // 2, BL], dt.bfloat16, tag="b0A_in")
            b0A_out = nc.dram_tensor("b0A_out", [2, 128, NH, T // 2, BL], dt.bfloat16)
            b0B_in = dram.tile([128, NH, T // 2, BL], dt.bfloat16, tag="b0B_in")
            b0B_out = nc.dram_tensor("b0B_out", [2, 128, NH, T // 2, BL], dt.bfloat16)
            b1_in = dram.tile([K, T, BL], f32, tag="b1_in")
            b1_out = nc.dram_tensor("b1_out", [2, K, T, BL], f32)

            em = None  # allocated after the h0 exchange (shares slot with u)

            class XgEmitter:
                """Incrementally emits the input-projection matmuls for one
                32-step block (12 gate chunks x 6 contract chunks) so they can
                be interleaved into the recurrence as PE filler."""

                def __init__(self, layer, blk, xg_tile, wih):
                    self.layer, self.blk, self.xg, self.wih = layer, blk, xg_tile, wih
                    self.j, self.kc, self.p = 0, 0, None

                def rhs(self, kc):
                    sl = slice(self.blk * BLK, (self.blk + 1) * BLK)
                    if self.layer == 0:
                        return xT[:, kc, sl, :]
                    if kc < NH:
                        return h0[:, kc, sl, :]
                    return xT[:, kc - NH, sl, :]  # partner h0 lives in xT[:, 0:3]

                def step(self):
                    if self.j >= NG:
                        return False
                    if self.kc == 0:
                        self.p = ps.tile([128, BLK * BL], dt.float32, tag="pxg", bufs=2)
                    j = self.j
                    nc.tensor.matmul(
                        self.p[:],
                        self.wih[:, self.kc, j * 128:(j + 1) * 128],
                        self.rhs(self.kc),
                        start=(self.kc == 0),
                        stop=(self.kc == NE - 1),
                    )
                    self.kc += 1
                    if self.kc == NE:
                        bcol = self.layer * NG + j
                        nc.scalar.add(self.xg[:, j], self.p[:], bias[:, bcol:bcol + 1])
                        self.kc = 0
                        self.j += 1
                    return True

                def drain(self):
                    while self.step():
                        pass

            # ---- two LSTM layers (one direction each; SPMD over cores) ----
            for layer in range(2):
                wih = big.tile([128, NE, G], dt.bfloat16, tag="wih")
                w_src = w0_d if layer == 0 else w1_d
                for ch in range(NE):
                    nc.sync.dma_start(wih[:, ch], w_src[ch])
                whh = big.tile([128, NH, G], dt.float8e4, tag="whh")
                for kc in range(NH):
                    nc.sync.dma_start(whh[:, kc], whh_d[layer, kc])

                if layer == 1:
                    em = big.tile([K, T, BL], f32, tag="em")

                def h_chunk(t, kc):
                    return h8[:, kc, t % 2, :]

                def h_full(t):
                    if layer == 0:
                        return h0[:, :, t, :]
                    return h1r[:, :, t % RB, :]

                xg_cur = xgp.tile([128, NG, BLK * BL], dt.bfloat16, tag="xg", bufs=2)
                em0 = XgEmitter(layer, 0, xg_cur, wih)
                em0.drain()

                c_st = None
                for blk in range(NB):
                    if blk + 1 < NB:
                        xg_nxt = xgp.tile(
                            [128, NG, BLK * BL], dt.bfloat16, tag="xg", bufs=2
                        )
                        nxt = XgEmitter(layer, blk + 1, xg_nxt, wih)
                    else:
                        xg_nxt, nxt = None, None

                    for tl in range(BLK):
                        t = blk * BLK + tl
                        first = t == 0
                        u0 = tl * BL

                        # Gate pre-activations land in three separate PSUM
                        # banks (i+f, g, o). Each bank's group: recurrent
                        # whh matmuls plus one identity-matmul per gate chunk
                        # that injects xg (incl. bias) straight into PSUM —
                        # no DVE pre-adds, and each activation reads its bank
                        # as soon as that group retires while the PE streams
                        # the next group.
                        gp_if = ps.tile([128, 2 * NH, BL], f32, tag="gp_if", bufs=1)
                        gp_g = ps.tile([128, NH, BL], f32, tag="gp_g", bufs=1)
                        gp_o = ps.tile([128, NH, BL], f32, tag="gp_o", bufs=1)

                        def emit_group(tile_, j0, nj):
                            total = nj * (1 if first else NH + 1)
                            n = 0
                            for jj in range(nj):
                                j = j0 + jj
                                if not first:
                                    for kc in range(NH):
                                        nc.tensor.matmul(
                                            tile_[:, jj],
                                            whh[:, kc, j * 128:(j + 1) * 128],
                                            h_chunk(t - 1, kc),
                                            start=(n == 0),
                                            stop=(n == total - 1),
                                        )
                                        n += 1
                                nc.tensor.matmul(
                                    tile_[:, jj],
                                    ident[:],
                                    xg_cur[:, j, u0:u0 + BL],
                                    start=(n == 0),
                                    stop=(n == total - 1),
                                )
                                n += 1

                        emit_group(gp_if, 0, 2 * NH)
                        emit_group(gp_g, 2 * NH, NH)
                        emit_group(gp_o, 3 * NH, NH)

                        sif = tmp.tile([128, 2 * NH, BL], f32, tag="sif")
                        nc.scalar.activation(sif[:], gp_if[:], AF.Sigmoid)
                        tg = tmp.tile([128, NH, BL], f32, tag="tg")
                        nc.scalar.activation(tg[:], gp_g[:], AF.Tanh)
                        so = tmp.tile([128, NH, BL], f32, tag="so")
                        nc.scalar.activation(so[:], gp_o[:], AF.Sigmoid)

                        cN = state.tile([128, NH, BL], f32, tag="c")
                        if first:
                            nc.vector.tensor_mul(cN[:], sif[:, 0:NH], tg[:])
                        else:
                            t2 = tmp.tile([128, NH, BL], f32, tag="t2")
                            nc.vector.tensor_mul(t2[:], sif[:, NH:2 * NH], c_st[:])
                            t1 = tmp.tile([128, NH, BL], f32, tag="t1")
                            nc.vector.tensor_mul(t1[:], sif[:, 0:NH], tg[:])
                            nc.vector.tensor_add(cN[:], t1[:], t2[:])
                        c_st = cN

                        tc_t = tmp.tile([128, NH, BL], f32, tag="tc")
                        nc.scalar.activation(tc_t[:], cN[:], AF.Tanh)
                        # fp8 h first (next step's matmuls wait on it), then
                        # the bf16 copy for xg/emissions/exchange
                        nc.vector.tensor_mul(h8[:, :, t % 2, :], so[:], tc_t[:])
                        nc.vector.tensor_mul(h_full(t), so[:], tc_t[:])

                        # PE filler: next block's input projections
                        if nxt is not None:
                            for _ in range(3):
                                nxt.step()

                    if nxt is not None:
                        nxt.drain()
                        xg_cur = xg_nxt

                    if layer == 0:
                        # send this h0 block time-reversed into the bounce
                        # (per chunk: DMA APs are limited to 3 dims). Blocks
                        # 0-3 land in bounce A (partner's t 128-255), blocks
                        # 4-7 in bounce B (partner's t 0-127).
                        th = T // 2
                        dstb = b0A_in if blk < NB // 2 else b0B_in
                        t1r = T - blk * BLK - (th if blk < NB // 2 else 0)
                        rsl = rev_slice(t1r - BLK, t1r)
                        for c in range(NH):
                            nc.sync.dma_start(
                                dstb[:, c, rsl, :],
                                h0[:, c, blk * BLK:(blk + 1) * BLK, :],
                            )
                        if blk == NB // 2 - 1:
                            # first-half collective fires at our midpoint; its
                            # transfer hides under our remaining 4 blocks
                            nc.gpsimd.collective_compute(
                                "AllGather",
                                mybir.AluOpType.bypass,
                                replica_groups=groups,
                                ins=[b0A_in[:].opt()],
                                outs=[b0A_out[:].opt()],
                            )
                    else:
                        # emissions for the ring block just completed
                        r0 = (blk % 2) * BLK
                        pem = ps.tile([K, BLK, BL], f32, tag="pem", bufs=1)
                        for kc in range(NH):
                            nc.tensor.matmul(
                                pem[:],
                                wout[:, kc],
                                h1r[:, kc, r0:r0 + BLK, :],
                                start=(kc == 0),
                                stop=(kc == NH - 1),
                            )
                        nc.scalar.add(
                            em[:, blk * BLK:(blk + 1) * BLK, :], pem[:], bout[:, 0:1]
                        )

                if layer == 0:
                    # ---- second-half h0 exchange (only this one's latency
                    # is exposed) ----
                    th = T // 2
                    nc.gpsimd.collective_compute(
                        "AllGather",
                        mybir.AluOpType.bypass,
                        replica_groups=groups,
                        ins=[b0B_in[:].opt()],
                        outs=[b0B_out[:].opt()],
                    )
                    # partner h0 = (slot0 + slot1) - own(reversed); fp32 sum
                    # makes the bf16 cancellation exact. Half B (partner t
                    # 0-127) first — layer 1 consumes it first.
                    for half, bout_t in ((0, b0B_out), (1, b0A_out)):
                        lo = half * th
                        for c in range(NH):
                            s0c = big.tile([128, th, BL], dt.bfloat16, tag="XC")
                            nc.sync.dma_start(s0c[:], bout_t[0, :, c])
                            s1c = big.tile([128, th, BL], dt.bfloat16, tag="XD")
                            nc.sync.dma_start(s1c[:], bout_t[1, :, c])
                            u = big.tile([128, th, BL], f32, tag="em")
                            nc.vector.tensor_add(u[:], s0c[:], s1c[:])
                            nc.vector.tensor_sub(
                                xT[:, c, lo:lo + th, :], u[:],
                                h0[:, c, rev_slice(T - lo - th, T - lo), :],
                            )

            # ---- emissions exchange: em_full = own partial + partner partial ----
            nc.sync.dma_start(b1_in[:, ::-1, :], em[:])
            nc.gpsimd.collective_compute(
                "AllGather",
                mybir.AluOpType.bypass,
                replica_groups=groups,
                ins=[b1_in[:].opt()],
                outs=[b1_out[:].opt()],
            )
            # transition pairs (depends only on ohT/crf — overlaps the
            # collective): A[j, q] = sum_i trans[i, j] oh[i, q], dot oh[q+BL]
            oh_flat = ohT[:].rearrange("k t b -> k (t b)")
            NTC = 512
            NQm = NQ - BL
            scr2 = big.tile([K, BL, T], f32, tag="whh")
            scr2_tb = scr2[:].rearrange("k b t -> k t b")  # [K, T, BL]
            for nt in range((NQm + NTC - 1) // NTC):
                n0 = nt * NTC
                n1 = min(n0 + NTC, NQm)
                pa = ps.tile([K, NTC], f32, tag="misc", bufs=1)
                nc.tensor.matmul(pa[:, 0:n1 - n0], crf[:, 0:K], oh_flat[:, n0:n1],
                                 start=True, stop=True)
                nc.vector.tensor_tensor(
                    scr2_tb[:, n0 // BL:n1 // BL, :],
                    pa[:, 0:n1 - n0], oh_flat[:, n0 + BL:n1 + BL],
                    mybir.AluOpType.mult,
                )
            tr_t = tmp.tile([K, BL], f32, tag="trt")
            nc.vector.tensor_reduce(
                tr_t[:], scr2[:, :, 0:T - 1], mybir.AxisListType.X, mybir.AluOpType.add
            )

            s0e = big.tile([K, T, BL], f32, tag="wih")
            nc.sync.dma_start(s0e[:], b1_out[0])
            s1e = big.tile([K, T, BL], f32, tag="XC")
            nc.sync.dma_start(s1e[:], b1_out[1])
            nc.vector.tensor_add(s0e[:], s0e[:], s1e[:])
            em_rev = big.tile([K, T, BL], f32, tag="whh")
            nc.vector.tensor_copy(em_rev[:], em[:, ::-1, :])
            nc.vector.tensor_sub(em[:], s0e[:], em_rev[:])

            # ---- gold path score (numerator) ----
            scr = big.tile([K, BL, T], f32, tag="wih")
            nkb = tmp.tile([K, BL], f32, tag="nkb")
            nc.vector.tensor_tensor(
                scr[:].rearrange("k b t -> k t b"),
                em[:], ohT[:], mybir.AluOpType.mult,
            )
            nc.vector.tensor_reduce(
                nkb[:], scr[:], mybir.AxisListType.X, mybir.AluOpType.add
            )
            nc.vector.tensor_add(nkb[:], nkb[:], tr_t[:])
            pnum = ps.tile([1, BL], f32, tag="misc", bufs=1)
            nc.tensor.matmul(pnum[:], crf[:, 22:23], nkb[:], start=True, stop=False)
            nc.tensor.matmul(pnum[:], crf[:, 20:21], ohT[:, 0, :], start=False, stop=False)
            nc.tensor.matmul(pnum[:], crf[:, 21:22], ohT[:, T - 1, :], start=False, stop=True)
            num = tmp.tile([1, BL], f32, tag="num")
            nc.vector.tensor_copy(num[:], pnum[:])

            # ---- CRF forward algorithm (denominator), linear space ----
            eem = big.tile([K, T, BL], f32, tag="wih")
            nc.scalar.activation(eem[:], em[:], AF.Exp)
            ea = state.tile([K, BL], f32, tag="ea")
            nc.vector.tensor_tensor(
                ea[:], eem[:, 0, :], crf[:, 18:19].broadcast_to((K, BL)),
                mybir.AluOpType.mult,
            )
            logc = None
            for t_ in range(1, T):
                pea = ps.tile([K, BL], f32, tag="crf", bufs=1)
                nc.tensor.matmul(pea[:], crf[:, 9:9 + K], ea[:], start=True, stop=True)
                eaN = state.tile([K, BL], f32, tag="ea")
                nc.vector.tensor_tensor(
                    eaN[:], pea[:], eem[:, t_, :], mybir.AluOpType.mult
                )
                ea = eaN
                if t_ % CRF_S == 0:
                    r = tmp.tile([1, BL], f32, tag="crf_r")
                    nc.vector.reciprocal(r[:], ea[0:1, :])
                    pbc = ps.tile([K, BL], f32, tag="crf", bufs=1)
                    nc.tensor.matmul(pbc[:], crf[0:1, 23:23 + K], r[:],
                                     start=True, stop=True)
                    lg = tmp.tile([1, BL], f32, tag="crf_lg")
                    nc.scalar.activation(lg[:], ea[0:1, :], AF.Ln)
                    eaN2 = state.tile([K, BL], f32, tag="ea")
                    nc.vector.tensor_tensor(eaN2[:], ea[:], pbc[:], mybir.AluOpType.mult)
                    logcN = state.tile([1, BL], f32, tag="logc")
                    if logc is None:
                        nc.vector.tensor_copy(logcN[:], lg[:])
                    else:
                        nc.vector.tensor_add(logcN[:], logc[:], lg[:])
                    logc = logcN
                    ea = eaN2
            pden = ps.tile([1, BL], f32, tag="misc", bufs=1)
            nc.tensor.matmul(pden[:], crf[:, 19:20], ea[:], start=True, stop=True)
            den = tmp.tile([1, BL], f32, tag="den")
            nc.scalar.activation(den[:], pden[:], AF.Ln)
            if logc is not None:
                nc.vector.tensor_add(den[:], den[:], logc[:])

            # ---- loss = sum_b mask_b * (den_b - num_b) ----
            diff = tmp.tile([1, BL], f32, tag="diff")
            nc.vector.tensor_sub(diff[:], den[:], num[:])
            nc.vector.tensor_mul(diff[:], diff[:], lmask[:])
            lout = tmp.tile([1, 1], f32, tag="lout")
            nc.vector.tensor_reduce(
                lout[:], diff[:], mybir.AxisListType.X, mybir.AluOpType.add
            )
            nc.sync.dma_start(loss_d[:], lout[:])

    if legalize:
        split_waits(nc)
    nc.finalize()
    return nc


def stage_inputs(inputs):
    x = np.asarray(inputs["embedding"], np.float32)
    tags = np.asarray(inputs["target_tag"]).astype(np.int64)

    def pget(name):
        return np.asarray(inputs[name], np.float32)

    def wihT(name, row_order=None):
        w = pget(name)[GATE_PERM]            # [1536, in]
        wT = w.T                             # [in, 1536]
        if row_order is not None:
            wT = wT[row_order]
        return np.ascontiguousarray(wT).reshape(-1, 128, G).astype(bf16)

    def whhT(name):
        w = pget(name)[GATE_PERM]
        return np.ascontiguousarray(w.T).reshape(NH, 128, G).astype(
            ml_dtypes.float8_e4m3)

    def biasv(name):
        return pget(name)[GATE_PERM].reshape(NG, 128).T

    trans, st, et = pget("trans"), pget("start_trans"), pget("end_trans")
    w_out, b_out = pget("w_out"), pget("b_out")

    in_maps = []
    for c in range(NC):
        p, par = divmod(c, 2)
        d = "f" if par == 0 else "b"
        xs = x[16 * p:16 * p + 16]
        tg = tags[16 * p:16 * p + 16]
        if par:
            xs = xs[:, ::-1]
            tg = tg[:, ::-1]
        xT_c = np.ascontiguousarray(xs.transpose(2, 1, 0)).reshape(
            NE, 128, T, BL).astype(bf16)

        w0 = wihT(f"w_ih_0{d}")
        own = np.arange(0, H) if par == 0 else np.arange(H, 2 * H)
        oth = np.arange(H, 2 * H) if par == 0 else np.arange(0, H)
        w1 = wihT(f"w_ih_1{d}", row_order=np.concatenate([own, oth]))
        whh = np.stack([whhT(f"w_hh_0{d}"), whhT(f"w_hh_1{d}")])
        bias = np.concatenate([biasv(f"b_0{d}"), biasv(f"b_1{d}")], axis=1).astype(
            np.float32)
        wh = w_out[:, 0:H] if par == 0 else w_out[:, H:2 * H]
        woutT = np.ascontiguousarray(wh.T).reshape(NH, 128, K).astype(bf16)
        bout = (b_out if par == 0 else np.zeros(K, np.float32)).reshape(K, 1)

        oh = np.zeros((K, T, BL), np.float32)
        oh[tg.T.reshape(-1), np.repeat(np.arange(T), BL), np.tile(np.arange(BL), T)] = 1.0

        tr_eff = trans if par == 0 else np.ascontiguousarray(trans.T)
        st_eff = st if par == 0 else et
        et_eff = et if par == 0 else st
        crf_c = np.zeros((K, 32), np.float32)
        crf_c[:, 0:9] = tr_eff
        crf_c[:, 9:18] = np.exp(tr_eff)
        crf_c[:, 18] = np.exp(st_eff)
        crf_c[:, 19] = np.exp(et_eff)
        crf_c[:, 20] = st_eff
        crf_c[:, 21] = et_eff
        crf_c[:, 22] = 1.0
        crf_c[0, 23:32] = 1.0
        lm = np.zeros((1, BL), np.float32)
        if par == 0:
            lm[0, 0:8] = 1.0
        else:
            lm[0, 8:16] = 1.0

        in_maps.append(
            dict(
                xT=xT_c, w0T=w0, w1T=w1, whhT=whh, bias=bias, woutT=woutT,
                bout=bout, ohT=np.ascontiguousarray(oh), crf=crf_c, lmask=lm,
                ident=np.eye(128, dtype=bf16),
            )
        )
    return in_maps


_NC_CACHE = {}


def get_nc():
    if "nc" not in _NC_CACHE:
        _NC_CACHE["nc"] = build_nc()
    return _NC_CACHE["nc"]


def kernel(**inputs):
    from concourse.bass_utils import run_bass_kernel_spmd

    nc = get_nc()
    in_maps = stage_inputs(inputs)
    res = run_bass_kernel_spmd(nc, in_maps, list(range(NC)))
    total = np.float32(0.0)
    for r in res.results:
        total += np.float32(r["loss"].reshape(-1)[0])
    return np.asarray(total, dtype=np.float32)


# revision 5
# speedup vs baseline: 1.3022x; 1.0144x over previous
"""BiLSTM(2-layer) + CRF NLL Trainium2 kernel, v2: direction-split sharding.

8 cores = 4 pairs. Pair p owns 16 sequences; core 2p runs the FORWARD
direction of both LSTM layers for those 16 sequences, core 2p+1 the BACKWARD
direction. Backward cores see time-reversed inputs, so every core runs an
identical forward-scan program; all direction asymmetry lives in host staging
(weights, reversed inputs, transposed CRF transitions, swapped start/end).

Between layers the pair exchanges hidden states with a 2-core AllGather
(bounce via DRAM, sent time-reversed so the partner receives data in its own
time order); each core reconstructs the partner's h via
(slot0 + slot1) - own, computed in fp32 so the bf16 cancellation is exact.
Emissions are per-direction partials pair-summed the same way. Each core then
runs the CRF on all 16 sequences and masks the per-sequence losses so each
sequence is counted on exactly one core.

vs v1: the serial recurrent matmul chain per core drops from 36864 LDW+MM
pairs (N=8) to 18360 (N=16), and input-projection/emission matmuls are
interleaved into the recurrence as PE filler during the per-step elementwise
tails. Gate chunks are reordered [g, i, f, o] so tanh(g)/sigmoid(i,f) start
before the step's matmuls finish and only sigmoid(o) trails them.
"""

import sys
import numpy as np
import ml_dtypes

sys.path.insert(0, "/opt/trn_rl_repo")

import concourse.bass as bass
import concourse.mybir as mybir
import concourse.tile as tile

dt = mybir.dt
AF = mybir.ActivationFunctionType
bf16 = ml_dtypes.bfloat16

# problem constants
B, T, E, H, K = 64, 256, 768, 384, 9
NC = 8
BL = 16         # sequences per core (one direction)
G = 4 * H       # 1536
NE = 6          # input contract chunks (768/128, both layers)
NH = H // 128   # 3
NG = G // 128   # 12
BLK = 32        # timesteps per xg block
NB = T // BLK   # 8
NQ = T * BL     # 4096
CRF_S = 8

# gate chunk order [i, f, g, o] = native pytorch order. The step's matmuls
# run as three PSUM-bank groups (i+f, g, o) so each activation starts as soon
# as its bank's accumulation retires, overlapping the rest of the matmul
# stream; o last so only sigmoid(o) + one mul trail the final matmul.
GATE_PERM = np.arange(4 * H)


def split_waits(nc):
    """Legalize sem waits: walrus accepts at most one sync wait per
    instruction; hoist extra waits onto same-engine NoOps."""
    import bass_rust

    n_split = 0
    for f in nc.m.functions:
        for blk in f.blocks:
            out = []
            changed = False
            for inst in blk.instructions:
                si = inst.sync_info
                if si is not None and si.on_wait and len(si.on_wait) > 1:
                    waits = list(si.on_wait)
                    for k, w in enumerate(waits[:-1]):
                        nop = mybir.InstNoOp(name=f"{inst.name}_w{k}", ins=[], outs=[])
                        nop.engine = inst.engine
                        nop.sync_info = bass_rust.SyncInfo(on_wait=[w], on_update=[])
                        out.append(nop)
                        n_split += 1
                    inst.sync_info = bass_rust.SyncInfo(
                        on_wait=[waits[-1]], on_update=list(si.on_update or [])
                    )
                    changed = True
                out.append(inst)
            if changed:
                blk.instructions = out
    return n_split


def rev_slice(a, b):
    """slice covering [a, b) traversed in reverse order."""
    return slice(b - 1, None if a == 0 else a - 1, -1)


def build_nc(legalize=True):
    nc = bass.Bass(trn_type="TRN2", num_devices=NC)
    f32 = dt.float32
    groups = [[2 * p, 2 * p + 1] for p in range(NC // 2)]

    xT_d = nc.declare_dram_parameter("xT", [NE, 128, T, BL], dt.bfloat16, False)
    w0_d = nc.declare_dram_parameter("w0T", [NE, 128, G], dt.bfloat16, False)
    w1_d = nc.declare_dram_parameter("w1T", [NE, 128, G], dt.bfloat16, False)
    whh_d = nc.declare_dram_parameter("whhT", [2, NH, 128, G], dt.bfloat16, False)
    bias_d = nc.declare_dram_parameter("bias", [128, 2 * NG], f32, False)
    wout_d = nc.declare_dram_parameter("woutT", [NH, 128, K], dt.bfloat16, False)
    bout_d = nc.declare_dram_parameter("bout", [K, 1], f32, False)
    oh_d = nc.declare_dram_parameter("ohT", [K, T, BL], f32, False)
    id_d = nc.declare_dram_parameter("ident", [128, 128], dt.bfloat16, False)
    crf_d = nc.declare_dram_parameter("crf", [K, 32], f32, False)
    lmask_d = nc.declare_dram_parameter("lmask", [1, BL], f32, False)
    loss_d = nc.declare_dram_parameter("loss", [1, 1], f32, True)

    RB = 2 * BLK  # h1 ring length (timesteps); emissions drain a block behind

    with tile.TileContext(nc) as tc:
        with (
            tc.tile_pool(name="big", bufs=1) as big,
            tc.tile_pool(name="state", bufs=2) as state,
            tc.tile_pool(name="tmp", bufs=3) as tmp,
            tc.tile_pool(name="xgp", bufs=1) as xgp,
            tc.tile_pool(name="ps", bufs=2, space="PSUM") as ps,
            tc.tile_pool(name="dram", bufs=1, space="DRAM") as dram,
        ):
            # ---- persistent loads ----
            xT = big.tile([128, NE, T, BL], dt.bfloat16, tag="xT")
            for ch in range(NE):
                nc.sync.dma_start(xT[:, ch], xT_d[ch])
            bias = big.tile([128, 2 * NG], f32, tag="bias")
            nc.sync.dma_start(bias[:], bias_d[:])
            wout = big.tile([128, NH, K], dt.bfloat16, tag="wout")
            for ch in range(NH):
                nc.sync.dma_start(wout[:, ch], wout_d[ch])
            bout = big.tile([K, 1], f32, tag="bout")
            nc.sync.dma_start(bout[:], bout_d[:])
            ident = big.tile([128, 128], dt.bfloat16, tag="ident")
            nc.sync.dma_start(ident[:], id_d[:])
            # stage via DVE copies (single-DMA-queue-consumer rule)
            ohT_raw = big.tile([K, T, BL], f32, tag="em")  # slot later: u, em
            nc.sync.dma_start(ohT_raw[:], oh_d[:])
            ohT = big.tile([K, T, BL], f32, tag="ohT")
            nc.vector.tensor_copy(ohT[:], ohT_raw[:])
            crf_raw = big.tile([K, 32], f32, tag="crf_raw")
            nc.sync.dma_start(crf_raw[:], crf_d[:])
            crf = big.tile([K, 32], f32, tag="crf")
            nc.vector.tensor_copy(crf[:], crf_raw[:])
            lmask_raw = big.tile([1, BL], f32, tag="lmask_raw")
            nc.sync.dma_start(lmask_raw[:], lmask_d[:])
            lmask = big.tile([1, BL], f32, tag="lmask")
            nc.vector.tensor_copy(lmask[:], lmask_raw[:])

            h0 = big.tile([128, NH, T, BL], dt.bfloat16, tag="h0")
            h1r = big.tile([128, NH, RB, BL], dt.bfloat16, tag="h1r")

            # DRAM bounce buffers for the pairwise exchanges. h0 is exchanged
            # in two halves: the half the partner needs LAST is sent at our
            # midpoint (its transfer hides under our remaining compute); only
            # the second half's latency is exposed at the layer boundary.
            b0A_in = dram.tile([128, NH, T // 2, BL], dt.bfloat16, tag="b0A_in")
            b0A_out = nc.dram_tensor("b0A_out", [2, 128, NH, T // 2, BL], dt.bfloat16)
            b0B_in = dram.tile([128, NH, T // 2, BL], dt.bfloat16, tag="b0B_in")
            b0B_out = nc.dram_tensor("b0B_out", [2, 128, NH, T // 2, BL], dt.bfloat16)
            b1_in = dram.tile([K, T, BL], f32, tag="b1_in")
            b1_out = nc.dram_tensor("b1_out", [2, K, T, BL], f32)

            em = None  # allocated after the h0 exchange (shares slot with u)

            class XgEmitter:
                """Incrementally emits the input-projection matmuls for one
                32-step block (12 gate chunks x 6 contract chunks) so they can
                be interleaved into the recurrence as PE filler."""

                def __init__(self, layer, blk, xg_tile, wih):
                    self.layer, self.blk, self.xg, self.wih = layer, blk, xg_tile, wih
                    self.j, self.kc, self.p = 0, 0, None

                def rhs(self, kc):
                    sl = slice(self.blk * BLK, (self.blk + 1) * BLK)
                    if self.layer == 0:
                        return xT[:, kc, sl, :]
                    if kc < NH:
                        return h0[:, kc, sl, :]
                    return xT[:, kc - NH, sl, :]  # partner h0 lives in xT[:, 0:3]

                def step(self):
                    if self.j >= NG:
                        return False
                    if self.kc == 0:
                        self.p = ps.tile([128, BLK * BL], dt.float32, tag="pxg", bufs=2)
                    j = self.j
                    nc.tensor.matmul(
                        self.p[:],
                        self.wih[:, self.kc, j * 128:(j + 1) * 128],
                        self.rhs(self.kc),
                        start=(self.kc == 0),
                        stop=(self.kc == NE - 1),
                    )
                    self.kc += 1
                    if self.kc == NE:
                        bcol = self.layer * NG + j
                        nc.scalar.add(self.xg[:, j], self.p[:], bias[:, bcol:bcol + 1])
                        self.kc = 0
                        self.j += 1
                    return True

                def drain(self):
                    while self.step():
                        pass

            # ---- two LSTM layers (one direction each; SPMD over cores) ----
            for layer in range(2):
                wih = big.tile([128, NE, G], dt.bfloat16, tag="wih")
                w_src = w0_d if layer == 0 else w1_d
                for ch in range(NE):
                    nc.sync.dma_start(wih[:, ch], w_src[ch])
                whh = big.tile([128, NH, G], dt.bfloat16, tag="whh")
                for kc in range(NH):
                    nc.sync.dma_start(whh[:, kc], whh_d[layer, kc])

                if layer == 1:
                    em = big.tile([K, T, BL], f32, tag="em")

                def h_chunk(t, kc):
                    if layer == 0:
                        return h0[:, kc, t, :]
                    return h1r[:, kc, t % RB, :]

                def h_full(t):
                    if layer == 0:
                        return h0[:, :, t, :]
                    return h1r[:, :, t % RB, :]

                xg_cur = xgp.tile([128, NG, BLK * BL], dt.bfloat16, tag="xg", bufs=2)
                em0 = XgEmitter(layer, 0, xg_cur, wih)
                em0.drain()

                c_st = None
                for blk in range(NB):
                    if blk + 1 < NB:
                        xg_nxt = xgp.tile(
                            [128, NG, BLK * BL], dt.bfloat16, tag="xg", bufs=2
                        )
                        nxt = XgEmitter(layer, blk + 1, xg_nxt, wih)
                    else:
                        xg_nxt, nxt = None, None

                    for tl in range(BLK):
                        t = blk * BLK + tl
                        first = t == 0
                        u0 = tl * BL

                        # Gate pre-activations land in three separate PSUM
                        # banks (i+f, g, o). Each bank's group: recurrent
                        # whh matmuls plus one identity-matmul per gate chunk
                        # that injects xg (incl. bias) straight into PSUM —
                        # no DVE pre-adds, and each activation reads its bank
                        # as soon as that group retires while the PE streams
                        # the next group.
                        gp_if = ps.tile([128, 2 * NH, BL], f32, tag="gp_if", bufs=1)
                        gp_g = ps.tile([128, NH, BL], f32, tag="gp_g", bufs=1)
                        gp_o = ps.tile([128, NH, BL], f32, tag="gp_o", bufs=1)

                        def emit_group(tile_, j0, nj):
                            # nj*NH recurrent matmuls plus ONE identity
                            # matmul whose free dim spans (gate-chunk, batch),
                            # injecting xg for the whole group at once
                            total = (0 if first else nj * NH) + 1
                            n = 0
                            if not first:
                                for jj in range(nj):
                                    j = j0 + jj
                                    for kc in range(NH):
                                        nc.tensor.matmul(
                                            tile_[:, jj],
                                            whh[:, kc, j * 128:(j + 1) * 128],
                                            h_chunk(t - 1, kc),
                                            start=(n == 0),
                                            stop=(n == total - 1),
                                        )
                                        n += 1
                            nc.tensor.matmul(
                                tile_[:],
                                ident[:],
                                xg_cur[:, j0:j0 + nj, u0:u0 + BL],
                                start=(n == 0),
                                stop=(n == total - 1),
                            )

                        emit_group(gp_if, 0, 2 * NH)
                        emit_group(gp_g, 2 * NH, NH)
                        emit_group(gp_o, 3 * NH, NH)

                        sif = tmp.tile([128, 2 * NH, BL], f32, tag="sif")
                        nc.scalar.activation(sif[:], gp_if[:], AF.Sigmoid)
                        tg = tmp.tile([128, NH, BL], f32, tag="tg")
                        nc.scalar.activation(tg[:], gp_g[:], AF.Tanh)
                        so = tmp.tile([128, NH, BL], f32, tag="so")
                        nc.scalar.activation(so[:], gp_o[:], AF.Sigmoid)

                        cN = state.tile([128, NH, BL], f32, tag="c")
                        if first:
                            nc.vector.tensor_mul(cN[:], sif[:, 0:NH], tg[:])
                        else:
                            t2 = tmp.tile([128, NH, BL], f32, tag="t2")
                            nc.vector.tensor_mul(t2[:], sif[:, NH:2 * NH], c_st[:])
                            t1 = tmp.tile([128, NH, BL], f32, tag="t1")
                            nc.vector.tensor_mul(t1[:], sif[:, 0:NH], tg[:])
                            nc.vector.tensor_add(cN[:], t1[:], t2[:])
                        c_st = cN

                        tc_t = tmp.tile([128, NH, BL], f32, tag="tc")
                        nc.scalar.activation(tc_t[:], cN[:], AF.Tanh)
                        nc.vector.tensor_mul(h_full(t), so[:], tc_t[:])

                        # PE filler: next block's input projections
                        if nxt is not None:
                            for _ in range(3):
                                nxt.step()

                    if nxt is not None:
                        nxt.drain()
                        xg_cur = xg_nxt

                    if layer == 0:
                        # send this h0 block time-reversed into the bounce
                        # (per chunk: DMA APs are limited to 3 dims). Blocks
                        # 0-3 land in bounce A (partner's t 128-255), blocks
                        # 4-7 in bounce B (partner's t 0-127).
                        th = T // 2
                        dstb = b0A_in if blk < NB // 2 else b0B_in
                        t1r = T - blk * BLK - (th if blk < NB // 2 else 0)
                        rsl = rev_slice(t1r - BLK, t1r)
                        for c in range(NH):
                            nc.sync.dma_start(
                                dstb[:, c, rsl, :],
                                h0[:, c, blk * BLK:(blk + 1) * BLK, :],
                            )
                        if blk == NB // 2 - 1:
                            # first-half collective fires at our midpoint; its
                            # transfer hides under our remaining 4 blocks
                            nc.gpsimd.collective_compute(
                                "AllGather",
                                mybir.AluOpType.bypass,
                                replica_groups=groups,
                                ins=[b0A_in[:].opt()],
                                outs=[b0A_out[:].opt()],
                            )
                    else:
                        # emissions for the ring block just completed
                        r0 = (blk % 2) * BLK
                        pem = ps.tile([K, BLK, BL], f32, tag="misc", bufs=1)
                        for kc in range(NH):
                            nc.tensor.matmul(
                                pem[:],
                                wout[:, kc],
                                h1r[:, kc, r0:r0 + BLK, :],
                                start=(kc == 0),
                                stop=(kc == NH - 1),
                            )
                        nc.scalar.add(
                            em[:, blk * BLK:(blk + 1) * BLK, :], pem[:], bout[:, 0:1]
                        )

                if layer == 0:
                    # ---- second-half h0 exchange (only this one's latency
                    # is exposed) ----
                    th = T // 2
                    nc.gpsimd.collective_compute(
                        "AllGather",
                        mybir.AluOpType.bypass,
                        replica_groups=groups,
                        ins=[b0B_in[:].opt()],
                        outs=[b0B_out[:].opt()],
                    )
                    # partner h0 = (slot0 + slot1) - own(reversed); fp32 sum
                    # makes the bf16 cancellation exact. Half B (partner t
                    # 0-127) first — layer 1 consumes it first.
                    for half, bout_t in ((0, b0B_out), (1, b0A_out)):
                        lo = half * th
                        for c in range(NH):
                            s0c = big.tile([128, th, BL], dt.bfloat16, tag="XC")
                            nc.sync.dma_start(s0c[:], bout_t[0, :, c])
                            s1c = big.tile([128, th, BL], dt.bfloat16, tag="XD")
                            nc.sync.dma_start(s1c[:], bout_t[1, :, c])
                            u = big.tile([128, th, BL], f32, tag="em")
                            nc.vector.tensor_add(u[:], s0c[:], s1c[:])
                            nc.vector.tensor_sub(
                                xT[:, c, lo:lo + th, :], u[:],
                                h0[:, c, rev_slice(T - lo - th, T - lo), :],
                            )

            # ---- emissions exchange: em_full = own partial + partner partial ----
            nc.sync.dma_start(b1_in[:, ::-1, :], em[:])
            nc.gpsimd.collective_compute(
                "AllGather",
                mybir.AluOpType.bypass,
                replica_groups=groups,
                ins=[b1_in[:].opt()],
                outs=[b1_out[:].opt()],
            )
            # transition pairs (depends only on ohT/crf — overlaps the
            # collective): A[j, q] = sum_i trans[i, j] oh[i, q], dot oh[q+BL]
            oh_flat = ohT[:].rearrange("k t b -> k (t b)")
            NTC = 512
            NQm = NQ - BL
            scr2 = big.tile([K, BL, T], f32, tag="whh")
            scr2_tb = scr2[:].rearrange("k b t -> k t b")  # [K, T, BL]
            for nt in range((NQm + NTC - 1) // NTC):
                n0 = nt * NTC
                n1 = min(n0 + NTC, NQm)
                pa = ps.tile([K, NTC], f32, tag="misc", bufs=1)
                nc.tensor.matmul(pa[:, 0:n1 - n0], crf[:, 0:K], oh_flat[:, n0:n1],
                                 start=True, stop=True)
                nc.vector.tensor_tensor(
                    scr2_tb[:, n0 // BL:n1 // BL, :],
                    pa[:, 0:n1 - n0], oh_flat[:, n0 + BL:n1 + BL],
                    mybir.AluOpType.mult,
                )
            tr_t = tmp.tile([K, BL], f32, tag="trt")
            nc.vector.tensor_reduce(
                tr_t[:], scr2[:, :, 0:T - 1], mybir.AxisListType.X, mybir.AluOpType.add
            )

            s0e = big.tile([K, T, BL], f32, tag="wih")
            nc.sync.dma_start(s0e[:], b1_out[0])
            s1e = big.tile([K, T, BL], f32, tag="XC")
            nc.sync.dma_start(s1e[:], b1_out[1])
            nc.vector.tensor_add(s0e[:], s0e[:], s1e[:])
            em_rev = big.tile([K, T, BL], f32, tag="whh")
            nc.vector.tensor_copy(em_rev[:], em[:, ::-1, :])
            nc.vector.tensor_sub(em[:], s0e[:], em_rev[:])

            # ---- gold path score (numerator) ----
            scr = big.tile([K, BL, T], f32, tag="wih")
            nkb = tmp.tile([K, BL], f32, tag="nkb")
            nc.vector.tensor_tensor(
                scr[:].rearrange("k b t -> k t b"),
                em[:], ohT[:], mybir.AluOpType.mult,
            )
            nc.vector.tensor_reduce(
                nkb[:], scr[:], mybir.AxisListType.X, mybir.AluOpType.add
            )
            nc.vector.tensor_add(nkb[:], nkb[:], tr_t[:])
            pnum = ps.tile([1, BL], f32, tag="misc", bufs=1)
            nc.tensor.matmul(pnum[:], crf[:, 22:23], nkb[:], start=True, stop=False)
            nc.tensor.matmul(pnum[:], crf[:, 20:21], ohT[:, 0, :], start=False, stop=False)
            nc.tensor.matmul(pnum[:], crf[:, 21:22], ohT[:, T - 1, :], start=False, stop=True)
            num = tmp.tile([1, BL], f32, tag="num")
            nc.vector.tensor_copy(num[:], pnum[:])

            # ---- CRF forward algorithm (denominator), linear space ----
            # two independent 8-sequence chains interleaved so each chain's
            # PE->DVE latency hides under the other's ops
            eem = big.tile([K, T, BL], f32, tag="wih")
            nc.scalar.activation(eem[:], em[:], AF.Exp)
            HB = BL // 2
            ea = [None, None]
            logc = [None, None]
            for ch in range(2):
                eac = state.tile([K, HB], f32, tag=f"ea{ch}")
                nc.vector.tensor_tensor(
                    eac[:], eem[:, 0, ch * HB:(ch + 1) * HB],
                    crf[:, 18:19].broadcast_to((K, HB)),
                    mybir.AluOpType.mult,
                )
                ea[ch] = eac
            for t_ in range(1, T):
                pea = [None, None]
                for ch in range(2):
                    pea[ch] = ps.tile([K, HB], f32, tag=f"crf{ch}", bufs=1, name=f"pea{ch}")
                    nc.tensor.matmul(pea[ch][:], crf[:, 9:9 + K], ea[ch][:],
                                     start=True, stop=True)
                for ch in range(2):
                    eaN = state.tile([K, HB], f32, tag=f"ea{ch}")
                    nc.vector.tensor_tensor(
                        eaN[:], pea[ch][:], eem[:, t_, ch * HB:(ch + 1) * HB],
                        mybir.AluOpType.mult,
                    )
                    ea[ch] = eaN
                if t_ % CRF_S == 0:
                    r = [None, None]
                    for ch in range(2):
                        r[ch] = tmp.tile([1, HB], f32, tag=f"crf_r{ch}", name=f"r{ch}")
                        nc.vector.reciprocal(r[ch][:], ea[ch][0:1, :])
                    pbc = [None, None]
                    for ch in range(2):
                        pbc[ch] = ps.tile([K, HB], f32, tag=f"crf{ch}", bufs=1, name=f"pbc{ch}")
                        nc.tensor.matmul(pbc[ch][:], crf[0:1, 23:23 + K], r[ch][:],
                                         start=True, stop=True)
                    for ch in range(2):
                        lg = tmp.tile([1, HB], f32, tag=f"crf_lg{ch}")
                        nc.scalar.activation(lg[:], ea[ch][0:1, :], AF.Ln)
                        eaN2 = state.tile([K, HB], f32, tag=f"ea{ch}")
                        nc.vector.tensor_tensor(
                            eaN2[:], ea[ch][:], pbc[ch][:], mybir.AluOpType.mult
                        )
                        logcN = state.tile([1, HB], f32, tag=f"logc{ch}")
                        if logc[ch] is None:
                            nc.vector.tensor_copy(logcN[:], lg[:])
                        else:
                            nc.vector.tensor_add(logcN[:], logc[ch][:], lg[:])
                        logc[ch] = logcN
                        ea[ch] = eaN2
            eacat = tmp.tile([K, BL], f32, tag="eacat")
            for ch in range(2):
                nc.vector.tensor_copy(eacat[:, ch * HB:(ch + 1) * HB], ea[ch][:])
            pden = ps.tile([1, BL], f32, tag="misc", bufs=1)
            nc.tensor.matmul(pden[:], crf[:, 19:20], eacat[:], start=True, stop=True)
            den = tmp.tile([1, BL], f32, tag="den")
            nc.scalar.activation(den[:], pden[:], AF.Ln)
            lcat = tmp.tile([1, BL], f32, tag="lcat")
            for ch in range(2):
                nc.vector.tensor_copy(lcat[:, ch * HB:(ch + 1) * HB], logc[ch][:])
            nc.vector.tensor_add(den[:], den[:], lcat[:])

            # ---- loss = sum_b mask_b * (den_b - num_b) ----
            diff = tmp.tile([1, BL], f32, tag="diff")
            nc.vector.tensor_sub(diff[:], den[:], num[:])
            nc.vector.tensor_mul(diff[:], diff[:], lmask[:])
            lout = tmp.tile([1, 1], f32, tag="lout")
            nc.vector.tensor_reduce(
                lout[:], diff[:], mybir.AxisListType.X, mybir.AluOpType.add
            )
            nc.sync.dma_start(loss_d[:], lout[:])

    if legalize:
        split_waits(nc)
    nc.finalize()
    return nc


def stage_inputs(inputs):
    x = np.asarray(inputs["embedding"], np.float32)
    tags = np.asarray(inputs["target_tag"]).astype(np.int64)

    def pget(name):
        return np.asarray(inputs[name], np.float32)

    def wihT(name, row_order=None):
        w = pget(name)[GATE_PERM]            # [1536, in]
        wT = w.T                             # [in, 1536]
        if row_order is not None:
            wT = wT[row_order]
        return np.ascontiguousarray(wT).reshape(-1, 128, G).astype(bf16)

    def whhT(name):
        w = pget(name)[GATE_PERM]
        return np.ascontiguousarray(w.T).reshape(NH, 128, G).astype(bf16)

    def biasv(name):
        return pget(name)[GATE_PERM].reshape(NG, 128).T

    trans, st, et = pget("trans"), pget("start_trans"), pget("end_trans")
    w_out, b_out = pget("w_out"), pget("b_out")

    in_maps = []
    for c in range(NC):
        p, par = divmod(c, 2)
        d = "f" if par == 0 else "b"
        xs = x[16 * p:16 * p + 16]
        tg = tags[16 * p:16 * p + 16]
        if par:
            xs = xs[:, ::-1]
            tg = tg[:, ::-1]
        xT_c = np.ascontiguousarray(xs.transpose(2, 1, 0)).reshape(
            NE, 128, T, BL).astype(bf16)

        w0 = wihT(f"w_ih_0{d}")
        own = np.arange(0, H) if par == 0 else np.arange(H, 2 * H)
        oth = np.arange(H, 2 * H) if par == 0 else np.arange(0, H)
        w1 = wihT(f"w_ih_1{d}", row_order=np.concatenate([own, oth]))
        whh = np.stack([whhT(f"w_hh_0{d}"), whhT(f"w_hh_1{d}")])
        bias = np.concatenate([biasv(f"b_0{d}"), biasv(f"b_1{d}")], axis=1).astype(
            np.float32)
        wh = w_out[:, 0:H] if par == 0 else w_out[:, H:2 * H]
        woutT = np.ascontiguousarray(wh.T).reshape(NH, 128, K).astype(bf16)
        bout = (b_out if par == 0 else np.zeros(K, np.float32)).reshape(K, 1)

        oh = np.zeros((K, T, BL), np.float32)
        oh[tg.T.reshape(-1), np.repeat(np.arange(T), BL), np.tile(np.arange(BL), T)] = 1.0

        tr_eff = trans if par == 0 else np.ascontiguousarray(trans.T)
        st_eff = st if par == 0 else et
        et_eff = et if par == 0 else st
        crf_c = np.zeros((K, 32), np.float32)
        crf_c[:, 0:9] = tr_eff
        crf_c[:, 9:18] = np.exp(tr_eff)
        crf_c[:, 18] = np.exp(st_eff)
        crf_c[:, 19] = np.exp(et_eff)
        crf_c[:, 20] = st_eff
        crf_c[:, 21] = et_eff
        crf_c[:, 22] = 1.0
        crf_c[0, 23:32] = 1.0
        lm = np.zeros((1, BL), np.float32)
        if par == 0:
            lm[0, 0:8] = 1.0
        else:
            lm[0, 8:16] = 1.0

        in_maps.append(
            dict(
                xT=xT_c, w0T=w0, w1T=w1, whhT=whh, bias=bias, woutT=woutT,
                bout=bout, ohT=np.ascontiguousarray(oh), crf=crf_c, lmask=lm,
                ident=np.eye(128, dtype=bf16),
            )
        )
    return in_maps


_NC_CACHE = {}


def get_nc():
    if "nc" not in _NC_CACHE:
        _NC_CACHE["nc"] = build_nc()
    return _NC_CACHE["nc"]


def kernel(**inputs):
    from concourse.bass_utils import run_bass_kernel_spmd

    nc = get_nc()
    in_maps = stage_inputs(inputs)
    res = run_bass_kernel_spmd(nc, in_maps, list(range(NC)))
    total = np.float32(0.0)
    for r in res.results:
        total += np.float32(r["loss"].reshape(-1)[0])
    return np.asarray(total, dtype=np.float32)


# revision 6
# speedup vs baseline: 1.3032x; 1.0008x over previous
"""BiLSTM(2-layer) + CRF NLL Trainium2 kernel, v2: direction-split sharding.

8 cores = 4 pairs. Pair p owns 16 sequences; core 2p runs the FORWARD
direction of both LSTM layers for those 16 sequences, core 2p+1 the BACKWARD
direction. Backward cores see time-reversed inputs, so every core runs an
identical forward-scan program; all direction asymmetry lives in host staging
(weights, reversed inputs, transposed CRF transitions, swapped start/end).

Between layers the pair exchanges hidden states with a 2-core AllGather
(bounce via DRAM, sent time-reversed so the partner receives data in its own
time order); each core reconstructs the partner's h via
(slot0 + slot1) - own, computed in fp32 so the bf16 cancellation is exact.
Emissions are per-direction partials pair-summed the same way. Each core then
runs the CRF on all 16 sequences and masks the per-sequence losses so each
sequence is counted on exactly one core.

vs v1: the serial recurrent matmul chain per core drops from 36864 LDW+MM
pairs (N=8) to 18360 (N=16), and input-projection/emission matmuls are
interleaved into the recurrence as PE filler during the per-step elementwise
tails. Gate chunks are reordered [g, i, f, o] so tanh(g)/sigmoid(i,f) start
before the step's matmuls finish and only sigmoid(o) trails them.
"""

import sys
import numpy as np
import ml_dtypes

sys.path.insert(0, "/opt/trn_rl_repo")

import concourse.bass as bass
import concourse.mybir as mybir
import concourse.tile as tile

dt = mybir.dt
AF = mybir.ActivationFunctionType
bf16 = ml_dtypes.bfloat16

# problem constants
B, T, E, H, K = 64, 256, 768, 384, 9
NC = 8
BL = 16         # sequences per core (one direction)
G = 4 * H       # 1536
NE = 6          # input contract chunks (768/128, both layers)
NH = H // 128   # 3
NG = G // 128   # 12
BLK = 32        # timesteps per xg block
NB = T // BLK   # 8
NQ = T * BL     # 4096
CRF_S = 8

# gate chunk order [i, f, g, o] = native pytorch order. The step's matmuls
# run as three PSUM-bank groups (i+f, g, o) so each activation starts as soon
# as its bank's accumulation retires, overlapping the rest of the matmul
# stream; o last so only sigmoid(o) + one mul trail the final matmul.
GATE_PERM = np.arange(4 * H)


def split_waits(nc):
    """Legalize sem waits: walrus accepts at most one sync wait per
    instruction; hoist extra waits onto same-engine NoOps."""
    import bass_rust

    n_split = 0
    for f in nc.m.functions:
        for blk in f.blocks:
            out = []
            changed = False
            for inst in blk.instructions:
                si = inst.sync_info
                if si is not None and si.on_wait and len(si.on_wait) > 1:
                    waits = list(si.on_wait)
                    for k, w in enumerate(waits[:-1]):
                        nop = mybir.InstNoOp(name=f"{inst.name}_w{k}", ins=[], outs=[])
                        nop.engine = inst.engine
                        nop.sync_info = bass_rust.SyncInfo(on_wait=[w], on_update=[])
                        out.append(nop)
                        n_split += 1
                    inst.sync_info = bass_rust.SyncInfo(
                        on_wait=[waits[-1]], on_update=list(si.on_update or [])
                    )
                    changed = True
                out.append(inst)
            if changed:
                blk.instructions = out
    return n_split


def rev_slice(a, b):
    """slice covering [a, b) traversed in reverse order."""
    return slice(b - 1, None if a == 0 else a - 1, -1)


def build_nc(legalize=True):
    nc = bass.Bass(trn_type="TRN2", num_devices=NC)
    f32 = dt.float32
    groups = [[2 * p, 2 * p + 1] for p in range(NC // 2)]

    xT_d = nc.declare_dram_parameter("xT", [NE, 128, T, BL], dt.bfloat16, False)
    w0_d = nc.declare_dram_parameter("w0T", [NE, 128, G], dt.bfloat16, False)
    w1_d = nc.declare_dram_parameter("w1T", [NE, 128, G], dt.bfloat16, False)
    whh_d = nc.declare_dram_parameter("whhT", [2, NH, 128, G], dt.bfloat16, False)
    bias_d = nc.declare_dram_parameter("bias", [128, 2 * NG], f32, False)
    wout_d = nc.declare_dram_parameter("woutT", [NH, 128, K], dt.bfloat16, False)
    bout_d = nc.declare_dram_parameter("bout", [K, 1], f32, False)
    oh_d = nc.declare_dram_parameter("ohT", [K, T, BL], f32, False)
    id_d = nc.declare_dram_parameter("ident", [128, 128], dt.bfloat16, False)
    crf_d = nc.declare_dram_parameter("crf", [K, 32], f32, False)
    lmask_d = nc.declare_dram_parameter("lmask", [1, BL], f32, False)
    loss_d = nc.declare_dram_parameter("loss", [1, 1], f32, True)

    RB = 2 * BLK  # h1 ring length (timesteps); emissions drain a block behind

    with tile.TileContext(nc) as tc:
        with (
            tc.tile_pool(name="big", bufs=1) as big,
            tc.tile_pool(name="state", bufs=2) as state,
            tc.tile_pool(name="tmp", bufs=3) as tmp,
            tc.tile_pool(name="xgp", bufs=1) as xgp,
            tc.tile_pool(name="ps", bufs=2, space="PSUM") as ps,
            tc.tile_pool(name="dram", bufs=1, space="DRAM") as dram,
        ):
            # ---- persistent loads ----
            xT = big.tile([128, NE, T, BL], dt.bfloat16, tag="xT")
            for ch in range(NE):
                nc.sync.dma_start(xT[:, ch], xT_d[ch])
            bias = big.tile([128, 2 * NG], f32, tag="bias")
            nc.sync.dma_start(bias[:], bias_d[:])
            wout = big.tile([128, NH, K], dt.bfloat16, tag="wout")
            for ch in range(NH):
                nc.sync.dma_start(wout[:, ch], wout_d[ch])
            bout = big.tile([K, 1], f32, tag="bout")
            nc.sync.dma_start(bout[:], bout_d[:])
            ident = big.tile([128, 128], dt.bfloat16, tag="ident")
            nc.sync.dma_start(ident[:], id_d[:])
            # stage via DVE copies (single-DMA-queue-consumer rule)
            ohT_raw = big.tile([K, T, BL], f32, tag="em")  # slot later: u, em
            nc.sync.dma_start(ohT_raw[:], oh_d[:])
            ohT = big.tile([K, T, BL], f32, tag="ohT")
            nc.vector.tensor_copy(ohT[:], ohT_raw[:])
            crf_raw = big.tile([K, 32], f32, tag="crf_raw")
            nc.sync.dma_start(crf_raw[:], crf_d[:])
            crf = big.tile([K, 32], f32, tag="crf")
            nc.vector.tensor_copy(crf[:], crf_raw[:])
            lmask_raw = big.tile([1, BL], f32, tag="lmask_raw")
            nc.sync.dma_start(lmask_raw[:], lmask_d[:])
            lmask = big.tile([1, BL], f32, tag="lmask")
            nc.vector.tensor_copy(lmask[:], lmask_raw[:])

            h0 = big.tile([128, NH, T, BL], dt.bfloat16, tag="h0")
            h1r = big.tile([128, NH, RB, BL], dt.bfloat16, tag="h1r")

            # DRAM bounce buffers for the pairwise exchanges. h0 is exchanged
            # in two halves: the half the partner needs LAST is sent at our
            # midpoint (its transfer hides under our remaining compute); only
            # the second half's latency is exposed at the layer boundary.
            b0A_in = dram.tile([128, NH, T // 2, BL], dt.bfloat16, tag="b0A_in")
            b0A_out = nc.dram_tensor("b0A_out", [2, 128, NH, T // 2, BL], dt.bfloat16)
            b0B_in = dram.tile([128, NH, T // 2, BL], dt.bfloat16, tag="b0B_in")
            b0B_out = nc.dram_tensor("b0B_out", [2, 128, NH, T // 2, BL], dt.bfloat16)
            b1_in = dram.tile([K, T, BL], f32, tag="b1_in")
            b1_out = nc.dram_tensor("b1_out", [2, K, T, BL], f32)

            em = None  # allocated after the h0 exchange (shares slot with u)

            # numerator transition-pairs scratch: filled one tile per layer-0
            # block (depends only on ohT/crf), reduced at layer-0 end
            oh_flat = ohT[:].rearrange("k t b -> k (t b)")
            NTC = 512
            NQm = NQ - BL
            scr2 = big.tile([K, BL, T], f32, tag="XC")
            scr2_tb = scr2[:].rearrange("k b t -> k t b")  # [K, T, BL]
            tr_t = tmp.tile([K, BL], f32, tag="trt")

            def pairs_task(nt):
                n0 = nt * NTC
                n1 = min(n0 + NTC, NQm)
                pa = ps.tile([K, NTC], f32, tag="misc", bufs=1, name=f"pa{nt}")
                nc.tensor.matmul(pa[:, 0:n1 - n0], crf[:, 0:K], oh_flat[:, n0:n1],
                                 start=True, stop=True)
                nc.vector.tensor_tensor(
                    scr2_tb[:, n0 // BL:n1 // BL, :],
                    pa[:, 0:n1 - n0], oh_flat[:, n0 + BL:n1 + BL],
                    mybir.AluOpType.mult,
                )
                if nt == NB - 1:
                    nc.vector.tensor_reduce(
                        tr_t[:], scr2[:, :, 0:T - 1], mybir.AxisListType.X,
                        mybir.AluOpType.add,
                    )

            class XgEmitter:
                """Incrementally emits the input-projection matmuls for one
                32-step block (12 gate chunks x 6 contract chunks) so they can
                be interleaved into the recurrence as PE filler."""

                def __init__(self, layer, blk, xg_tile, wih):
                    self.layer, self.blk, self.xg, self.wih = layer, blk, xg_tile, wih
                    self.j, self.kc, self.p = 0, 0, None

                def rhs(self, kc):
                    sl = slice(self.blk * BLK, (self.blk + 1) * BLK)
                    if self.layer == 0:
                        return xT[:, kc, sl, :]
                    if kc < NH:
                        return h0[:, kc, sl, :]
                    return xT[:, kc - NH, sl, :]  # partner h0 lives in xT[:, 0:3]

                def step(self):
                    if self.j >= NG:
                        return False
                    if self.kc == 0:
                        self.p = ps.tile([128, BLK * BL], dt.float32, tag="pxg", bufs=2)
                    j = self.j
                    nc.tensor.matmul(
                        self.p[:],
                        self.wih[:, self.kc, j * 128:(j + 1) * 128],
                        self.rhs(self.kc),
                        start=(self.kc == 0),
                        stop=(self.kc == NE - 1),
                    )
                    self.kc += 1
                    if self.kc == NE:
                        bcol = self.layer * NG + j
                        nc.scalar.add(self.xg[:, j], self.p[:], bias[:, bcol:bcol + 1])
                        self.kc = 0
                        self.j += 1
                    return True

                def drain(self):
                    while self.step():
                        pass

            # ---- two LSTM layers (one direction each; SPMD over cores) ----
            for layer in range(2):
                wih = big.tile([128, NE, G], dt.bfloat16, tag="wih")
                w_src = w0_d if layer == 0 else w1_d
                for ch in range(NE):
                    nc.sync.dma_start(wih[:, ch], w_src[ch])
                whh = big.tile([128, NH, G], dt.bfloat16, tag="whh")
                for kc in range(NH):
                    nc.sync.dma_start(whh[:, kc], whh_d[layer, kc])

                if layer == 1:
                    em = big.tile([K, T, BL], f32, tag="em")

                def h_chunk(t, kc):
                    if layer == 0:
                        return h0[:, kc, t, :]
                    return h1r[:, kc, t % RB, :]

                def h_full(t):
                    if layer == 0:
                        return h0[:, :, t, :]
                    return h1r[:, :, t % RB, :]

                xg_cur = xgp.tile([128, NG, BLK * BL], dt.bfloat16, tag="xg", bufs=2)
                em0 = XgEmitter(layer, 0, xg_cur, wih)
                em0.drain()

                c_st = None
                for blk in range(NB):
                    if blk + 1 < NB:
                        xg_nxt = xgp.tile(
                            [128, NG, BLK * BL], dt.bfloat16, tag="xg", bufs=2
                        )
                        nxt = XgEmitter(layer, blk + 1, xg_nxt, wih)
                    else:
                        xg_nxt, nxt = None, None

                    for tl in range(BLK):
                        t = blk * BLK + tl
                        first = t == 0
                        u0 = tl * BL

                        # Gate pre-activations land in three separate PSUM
                        # banks (i+f, g, o). Each bank's group: recurrent
                        # whh matmuls plus one identity-matmul per gate chunk
                        # that injects xg (incl. bias) straight into PSUM —
                        # no DVE pre-adds, and each activation reads its bank
                        # as soon as that group retires while the PE streams
                        # the next group.
                        gp_if = ps.tile([128, 2 * NH, BL], f32, tag="gp_if", bufs=1)
                        gp_g = ps.tile([128, NH, BL], f32, tag="gp_g", bufs=1)
                        gp_o = ps.tile([128, NH, BL], f32, tag="gp_o", bufs=1)

                        def emit_group(tile_, j0, nj):
                            # ONE identity matmul FIRST (start=True): it has
                            # no dependency on h(t-1), so it issues during the
                            # previous step's elementwise tail and injects xg
                            # for the whole group; the nj*NH recurrent matmuls
                            # then accumulate on top.
                            total = (0 if first else nj * NH) + 1
                            nc.tensor.matmul(
                                tile_[:],
                                ident[:],
                                xg_cur[:, j0:j0 + nj, u0:u0 + BL],
                                start=True,
                                stop=(total == 1),
                            )
                            n = 1
                            if not first:
                                for jj in range(nj):
                                    j = j0 + jj
                                    for kc in range(NH):
                                        nc.tensor.matmul(
                                            tile_[:, jj],
                                            whh[:, kc, j * 128:(j + 1) * 128],
                                            h_chunk(t - 1, kc),
                                            start=False,
                                            stop=(n == total - 1),
                                        )
                                        n += 1

                        emit_group(gp_if, 0, 2 * NH)
                        emit_group(gp_g, 2 * NH, NH)
                        emit_group(gp_o, 3 * NH, NH)

                        sif = tmp.tile([128, 2 * NH, BL], f32, tag="sif")
                        nc.scalar.activation(sif[:], gp_if[:], AF.Sigmoid)
                        tg = tmp.tile([128, NH, BL], f32, tag="tg")
                        nc.scalar.activation(tg[:], gp_g[:], AF.Tanh)
                        so = tmp.tile([128, NH, BL], f32, tag="so")
                        nc.scalar.activation(so[:], gp_o[:], AF.Sigmoid)

                        cN = state.tile([128, NH, BL], f32, tag="c")
                        if first:
                            nc.vector.tensor_mul(cN[:], sif[:, 0:NH], tg[:])
                        else:
                            t2 = tmp.tile([128, NH, BL], f32, tag="t2")
                            nc.vector.tensor_mul(t2[:], sif[:, NH:2 * NH], c_st[:])
                            t1 = tmp.tile([128, NH, BL], f32, tag="t1")
                            nc.vector.tensor_mul(t1[:], sif[:, 0:NH], tg[:])
                            nc.vector.tensor_add(cN[:], t1[:], t2[:])
                        c_st = cN

                        tc_t = tmp.tile([128, NH, BL], f32, tag="tc")
                        nc.scalar.activation(tc_t[:], cN[:], AF.Tanh)
                        nc.vector.tensor_mul(h_full(t), so[:], tc_t[:])

                        # PE filler: next block's input projections
                        if nxt is not None:
                            for _ in range(3):
                                nxt.step()

                    if nxt is not None:
                        nxt.drain()
                        xg_cur = xg_nxt

                    if layer == 0:
                        # send this h0 block time-reversed into the bounce
                        # (per chunk: DMA APs are limited to 3 dims). Blocks
                        # 0-3 land in bounce A (partner's t 128-255), blocks
                        # 4-7 in bounce B (partner's t 0-127).
                        th = T // 2
                        dstb = b0A_in if blk < NB // 2 else b0B_in
                        t1r = T - blk * BLK - (th if blk < NB // 2 else 0)
                        rsl = rev_slice(t1r - BLK, t1r)
                        for c in range(NH):
                            nc.sync.dma_start(
                                dstb[:, c, rsl, :],
                                h0[:, c, blk * BLK:(blk + 1) * BLK, :],
                            )
                        pairs_task(blk)
                        if blk == NB // 2 - 1:
                            # first-half collective fires at our midpoint; its
                            # transfer hides under our remaining 4 blocks
                            nc.gpsimd.collective_compute(
                                "AllGather",
                                mybir.AluOpType.bypass,
                                replica_groups=groups,
                                ins=[b0A_in[:].opt()],
                                outs=[b0A_out[:].opt()],
                            )
                    else:
                        # emissions for the ring block just completed
                        r0 = (blk % 2) * BLK
                        pem = ps.tile([K, BLK, BL], f32, tag="misc", bufs=1)
                        for kc in range(NH):
                            nc.tensor.matmul(
                                pem[:],
                                wout[:, kc],
                                h1r[:, kc, r0:r0 + BLK, :],
                                start=(kc == 0),
                                stop=(kc == NH - 1),
                            )
                        nc.scalar.add(
                            em[:, blk * BLK:(blk + 1) * BLK, :], pem[:], bout[:, 0:1]
                        )

                if layer == 0:
                    # ---- second-half h0 exchange (only this one's latency
                    # is exposed) ----
                    th = T // 2
                    nc.gpsimd.collective_compute(
                        "AllGather",
                        mybir.AluOpType.bypass,
                        replica_groups=groups,
                        ins=[b0B_in[:].opt()],
                        outs=[b0B_out[:].opt()],
                    )
                    # partner h0 = (slot0 + slot1) - own(reversed); fp32 sum
                    # makes the bf16 cancellation exact. Half B (partner t
                    # 0-127) first — layer 1 consumes it first.
                    for half, bout_t in ((0, b0B_out), (1, b0A_out)):
                        lo = half * th
                        for c in range(NH):
                            s0c = big.tile([128, th, BL], dt.bfloat16, tag="XC")
                            nc.sync.dma_start(s0c[:], bout_t[0, :, c])
                            s1c = big.tile([128, th, BL], dt.bfloat16, tag="XD")
                            nc.sync.dma_start(s1c[:], bout_t[1, :, c])
                            u = big.tile([128, th, BL], f32, tag="em")
                            nc.vector.tensor_add(u[:], s0c[:], s1c[:])
                            nc.vector.tensor_sub(
                                xT[:, c, lo:lo + th, :], u[:],
                                h0[:, c, rev_slice(T - lo - th, T - lo), :],
                            )

            # ---- emissions exchange: em_full = own partial + partner partial ----
            nc.sync.dma_start(b1_in[:, ::-1, :], em[:])
            nc.gpsimd.collective_compute(
                "AllGather",
                mybir.AluOpType.bypass,
                replica_groups=groups,
                ins=[b1_in[:].opt()],
                outs=[b1_out[:].opt()],
            )
            s0e = big.tile([K, T, BL], f32, tag="wih")
            nc.sync.dma_start(s0e[:], b1_out[0])
            s1e = big.tile([K, T, BL], f32, tag="XC")
            nc.sync.dma_start(s1e[:], b1_out[1])
            nc.vector.tensor_add(s0e[:], s0e[:], s1e[:])
            em_rev = big.tile([K, T, BL], f32, tag="whh")
            nc.vector.tensor_copy(em_rev[:], em[:, ::-1, :])
            nc.vector.tensor_sub(em[:], s0e[:], em_rev[:])

            # ---- gold path score (numerator) ----
            scr = big.tile([K, BL, T], f32, tag="wih")
            nkb = tmp.tile([K, BL], f32, tag="nkb")
            nc.vector.tensor_tensor(
                scr[:].rearrange("k b t -> k t b"),
                em[:], ohT[:], mybir.AluOpType.mult,
            )
            nc.vector.tensor_reduce(
                nkb[:], scr[:], mybir.AxisListType.X, mybir.AluOpType.add
            )
            nc.vector.tensor_add(nkb[:], nkb[:], tr_t[:])
            pnum = ps.tile([1, BL], f32, tag="misc", bufs=1)
            nc.tensor.matmul(pnum[:], crf[:, 22:23], nkb[:], start=True, stop=False)
            nc.tensor.matmul(pnum[:], crf[:, 20:21], ohT[:, 0, :], start=False, stop=False)
            nc.tensor.matmul(pnum[:], crf[:, 21:22], ohT[:, T - 1, :], start=False, stop=True)
            num = tmp.tile([1, BL], f32, tag="num")
            nc.vector.tensor_copy(num[:], pnum[:])

            # ---- CRF forward algorithm (denominator), linear space ----
            # two independent 8-sequence chains interleaved so each chain's
            # PE->DVE latency hides under the other's ops
            eem = big.tile([K, T, BL], f32, tag="wih")
            nc.scalar.activation(eem[:], em[:], AF.Exp)
            HB = BL // 2
            ea = [None, None]
            logc = [None, None]
            for ch in range(2):
                eac = state.tile([K, HB], f32, tag=f"ea{ch}")
                nc.vector.tensor_tensor(
                    eac[:], eem[:, 0, ch * HB:(ch + 1) * HB],
                    crf[:, 18:19].broadcast_to((K, HB)),
                    mybir.AluOpType.mult,
                )
                ea[ch] = eac
            for t_ in range(1, T):
                pea = [None, None]
                for ch in range(2):
                    pea[ch] = ps.tile([K, HB], f32, tag=f"crf{ch}", bufs=1, name=f"pea{ch}")
                    nc.tensor.matmul(pea[ch][:], crf[:, 9:9 + K], ea[ch][:],
                                     start=True, stop=True)
                for ch in range(2):
                    eaN = state.tile([K, HB], f32, tag=f"ea{ch}")
                    nc.vector.tensor_tensor(
                        eaN[:], pea[ch][:], eem[:, t_, ch * HB:(ch + 1) * HB],
                        mybir.AluOpType.mult,
                    )
                    ea[ch] = eaN
                if t_ % CRF_S == 0:
                    r = [None, None]
                    for ch in range(2):
                        r[ch] = tmp.tile([1, HB], f32, tag=f"crf_r{ch}", name=f"r{ch}")
                        nc.vector.reciprocal(r[ch][:], ea[ch][0:1, :])
                    pbc = [None, None]
                    for ch in range(2):
                        pbc[ch] = ps.tile([K, HB], f32, tag=f"crf{ch}", bufs=1, name=f"pbc{ch}")
                        nc.tensor.matmul(pbc[ch][:], crf[0:1, 23:23 + K], r[ch][:],
                                         start=True, stop=True)
                    for ch in range(2):
                        lg = tmp.tile([1, HB], f32, tag=f"crf_lg{ch}")
                        nc.scalar.activation(lg[:], ea[ch][0:1, :], AF.Ln)
                        eaN2 = state.tile([K, HB], f32, tag=f"ea{ch}")
                        nc.vector.tensor_tensor(
                            eaN2[:], ea[ch][:], pbc[ch][:], mybir.AluOpType.mult
                        )
                        logcN = state.tile([1, HB], f32, tag=f"logc{ch}")
                        if logc[ch] is None:
                            nc.vector.tensor_copy(logcN[:], lg[:])
                        else:
                            nc.vector.tensor_add(logcN[:], logc[ch][:], lg[:])
                        logc[ch] = logcN
                        ea[ch] = eaN2
            eacat = tmp.tile([K, BL], f32, tag="eacat")
            for ch in range(2):
                nc.vector.tensor_copy(eacat[:, ch * HB:(ch + 1) * HB], ea[ch][:])
            pden = ps.tile([1, BL], f32, tag="misc", bufs=1)
            nc.tensor.matmul(pden[:], crf[:, 19:20], eacat[:], start=True, stop=True)
            den = tmp.tile([1, BL], f32, tag="den")
            nc.scalar.activation(den[:], pden[:], AF.Ln)
            lcat = tmp.tile([1, BL], f32, tag="lcat")
            for ch in range(2):
                nc.vector.tensor_copy(lcat[:, ch * HB:(ch + 1) * HB], logc[ch][:])
            nc.vector.tensor_add(den[:], den[:], lcat[:])

            # ---- loss = sum_b mask_b * (den_b - num_b) ----
            diff = tmp.tile([1, BL], f32, tag="diff")
            nc.vector.tensor_sub(diff[:], den[:], num[:])
            nc.vector.tensor_mul(diff[:], diff[:], lmask[:])
            lout = tmp.tile([1, 1], f32, tag="lout")
            nc.vector.tensor_reduce(
                lout[:], diff[:], mybir.AxisListType.X, mybir.AluOpType.add
            )
            nc.sync.dma_start(loss_d[:], lout[:])

    if legalize:
        split_waits(nc)
    nc.finalize()
    return nc


def stage_inputs(inputs):
    x = np.asarray(inputs["embedding"], np.float32)
    tags = np.asarray(inputs["target_tag"]).astype(np.int64)

    def pget(name):
        return np.asarray(inputs[name], np.float32)

    def wihT(name, row_order=None):
        w = pget(name)[GATE_PERM]            # [1536, in]
        wT = w.T                             # [in, 1536]
        if row_order is not None:
            wT = wT[row_order]
        return np.ascontiguousarray(wT).reshape(-1, 128, G).astype(bf16)

    def whhT(name):
        w = pget(name)[GATE_PERM]
        return np.ascontiguousarray(w.T).reshape(NH, 128, G).astype(bf16)

    def biasv(name):
        return pget(name)[GATE_PERM].reshape(NG, 128).T

    trans, st, et = pget("trans"), pget("start_trans"), pget("end_trans")
    w_out, b_out = pget("w_out"), pget("b_out")

    in_maps = []
    for c in range(NC):
        p, par = divmod(c, 2)
        d = "f" if par == 0 else "b"
        xs = x[16 * p:16 * p + 16]
        tg = tags[16 * p:16 * p + 16]
        if par:
            xs = xs[:, ::-1]
            tg = tg[:, ::-1]
        xT_c = np.ascontiguousarray(xs.transpose(2, 1, 0)).reshape(
            NE, 128, T, BL).astype(bf16)

        w0 = wihT(f"w_ih_0{d}")
        own = np.arange(0, H) if par == 0 else np.arange(H, 2 * H)
        oth = np.arange(H, 2 * H) if par == 0 else np.arange(0, H)
        w1 = wihT(f"w_ih_1{d}", row_order=np.concatenate([own, oth]))
        whh = np.stack([whhT(f"w_hh_0{d}"), whhT(f"w_hh_1{d}")])
        bias = np.concatenate([biasv(f"b_0{d}"), biasv(f"b_1{d}")], axis=1).astype(
            np.float32)
        wh = w_out[:, 0:H] if par == 0 else w_out[:, H:2 * H]
        woutT = np.ascontiguousarray(wh.T).reshape(NH, 128, K).astype(bf16)
        bout = (b_out if par == 0 else np.zeros(K, np.float32)).reshape(K, 1)

        oh = np.zeros((K, T, BL), np.float32)
        oh[tg.T.reshape(-1), np.repeat(np.arange(T), BL), np.tile(np.arange(BL), T)] = 1.0

        tr_eff = trans if par == 0 else np.ascontiguousarray(trans.T)
        st_eff = st if par == 0 else et
        et_eff = et if par == 0 else st
        crf_c = np.zeros((K, 32), np.float32)
        crf_c[:, 0:9] = tr_eff
        crf_c[:, 9:18] = np.exp(tr_eff)
        crf_c[:, 18] = np.exp(st_eff)
        crf_c[:, 19] = np.exp(et_eff)
        crf_c[:, 20] = st_eff
        crf_c[:, 21] = et_eff
        crf_c[:, 22] = 1.0
        crf_c[0, 23:32] = 1.0
        lm = np.zeros((1, BL), np.float32)
        if par == 0:
            lm[0, 0:8] = 1.0
        else:
            lm[0, 8:16] = 1.0

        in_maps.append(
            dict(
                xT=xT_c, w0T=w0, w1T=w1, whhT=whh, bias=bias, woutT=woutT,
                bout=bout, ohT=np.ascontiguousarray(oh), crf=crf_c, lmask=lm,
                ident=np.eye(128, dtype=bf16),
            )
        )
    return in_maps


_NC_CACHE = {}


def get_nc():
    if "nc" not in _NC_CACHE:
        _NC_CACHE["nc"] = build_nc()
    return _NC_CACHE["nc"]


def kernel(**inputs):
    from concourse.bass_utils import run_bass_kernel_spmd

    nc = get_nc()
    in_maps = stage_inputs(inputs)
    res = run_bass_kernel_spmd(nc, in_maps, list(range(NC)))
    total = np.float32(0.0)
    for r in res.results:
        total += np.float32(r["loss"].reshape(-1)[0])
    return np.asarray(total, dtype=np.float32)


# revision 7
# speedup vs baseline: 1.3050x; 1.0014x over previous
"""BiLSTM(2-layer) + CRF NLL Trainium2 kernel, v2: direction-split sharding.

8 cores = 4 pairs. Pair p owns 16 sequences; core 2p runs the FORWARD
direction of both LSTM layers for those 16 sequences, core 2p+1 the BACKWARD
direction. Backward cores see time-reversed inputs, so every core runs an
identical forward-scan program; all direction asymmetry lives in host staging
(weights, reversed inputs, transposed CRF transitions, swapped start/end).

Between layers the pair exchanges hidden states with a 2-core AllGather
(bounce via DRAM, sent time-reversed so the partner receives data in its own
time order); each core reconstructs the partner's h via
(slot0 + slot1) - own, computed in fp32 so the bf16 cancellation is exact.
Emissions are per-direction partials pair-summed the same way. Each core then
runs the CRF on all 16 sequences and masks the per-sequence losses so each
sequence is counted on exactly one core.

vs v1: the serial recurrent matmul chain per core drops from 36864 LDW+MM
pairs (N=8) to 18360 (N=16), and input-projection/emission matmuls are
interleaved into the recurrence as PE filler during the per-step elementwise
tails. Gate chunks are reordered [g, i, f, o] so tanh(g)/sigmoid(i,f) start
before the step's matmuls finish and only sigmoid(o) trails them.
"""

import sys
import numpy as np
import ml_dtypes

sys.path.insert(0, "/opt/trn_rl_repo")

import concourse.bass as bass
import concourse.mybir as mybir
import concourse.tile as tile

dt = mybir.dt
AF = mybir.ActivationFunctionType
bf16 = ml_dtypes.bfloat16

# problem constants
B, T, E, H, K = 64, 256, 768, 384, 9
NC = 8
BL = 16         # sequences per core (one direction)
G = 4 * H       # 1536
NE = 6          # input contract chunks (768/128, both layers)
NH = H // 128   # 3
NG = G // 128   # 12
BLK = 32        # timesteps per xg block
NB = T // BLK   # 8
NQ = T * BL     # 4096
CRF_S = 8

# gate chunk order [i, f, g, o] = native pytorch order. The step's matmuls
# run as three PSUM-bank groups (i+f, g, o) so each activation starts as soon
# as its bank's accumulation retires, overlapping the rest of the matmul
# stream; o last so only sigmoid(o) + one mul trail the final matmul.
GATE_PERM = np.arange(4 * H)


def split_waits(nc):
    """Legalize sem waits: walrus accepts at most one sync wait per
    instruction; hoist extra waits onto same-engine NoOps."""
    import bass_rust

    n_split = 0
    for f in nc.m.functions:
        for blk in f.blocks:
            out = []
            changed = False
            for inst in blk.instructions:
                si = inst.sync_info
                if si is not None and si.on_wait and len(si.on_wait) > 1:
                    waits = list(si.on_wait)
                    for k, w in enumerate(waits[:-1]):
                        nop = mybir.InstNoOp(name=f"{inst.name}_w{k}", ins=[], outs=[])
                        nop.engine = inst.engine
                        nop.sync_info = bass_rust.SyncInfo(on_wait=[w], on_update=[])
                        out.append(nop)
                        n_split += 1
                    inst.sync_info = bass_rust.SyncInfo(
                        on_wait=[waits[-1]], on_update=list(si.on_update or [])
                    )
                    changed = True
                out.append(inst)
            if changed:
                blk.instructions = out
    return n_split


def rev_slice(a, b):
    """slice covering [a, b) traversed in reverse order."""
    return slice(b - 1, None if a == 0 else a - 1, -1)


def build_nc(legalize=True):
    nc = bass.Bass(trn_type="TRN2", num_devices=NC)
    f32 = dt.float32
    groups = [[2 * p, 2 * p + 1] for p in range(NC // 2)]

    xT_d = nc.declare_dram_parameter("xT", [NE, 128, T, BL], dt.bfloat16, False)
    w0_d = nc.declare_dram_parameter("w0T", [NE, 128, G], dt.bfloat16, False)
    w1_d = nc.declare_dram_parameter("w1T", [NE, 128, G], dt.bfloat16, False)
    whh_d = nc.declare_dram_parameter("whhT", [2, NH, 128, G], dt.bfloat16, False)
    bias_d = nc.declare_dram_parameter("bias", [128, 2 * NG], f32, False)
    wout_d = nc.declare_dram_parameter("woutT", [NH, 128, K], dt.bfloat16, False)
    bout_d = nc.declare_dram_parameter("bout", [K, 1], f32, False)
    oh_d = nc.declare_dram_parameter("ohT", [K, T, BL], f32, False)
    id_d = nc.declare_dram_parameter("ident", [128, 128], dt.bfloat16, False)
    crf_d = nc.declare_dram_parameter("crf", [K, 32], f32, False)
    lmask_d = nc.declare_dram_parameter("lmask", [1, BL], f32, False)
    loss_d = nc.declare_dram_parameter("loss", [1, 1], f32, True)

    RB = 2 * BLK  # h1 ring length (timesteps); emissions drain a block behind

    with tile.TileContext(nc) as tc:
        with (
            tc.tile_pool(name="big", bufs=1) as big,
            tc.tile_pool(name="state", bufs=2) as state,
            tc.tile_pool(name="tmp", bufs=3) as tmp,
            tc.tile_pool(name="xgp", bufs=1) as xgp,
            tc.tile_pool(name="ps", bufs=2, space="PSUM") as ps,
            tc.tile_pool(name="dram", bufs=1, space="DRAM") as dram,
        ):
            # ---- persistent loads ----
            xT = big.tile([128, NE, T, BL], dt.bfloat16, tag="xT")
            for ch in range(NE):
                nc.sync.dma_start(xT[:, ch], xT_d[ch])
            bias = big.tile([128, 2 * NG], f32, tag="bias")
            nc.sync.dma_start(bias[:], bias_d[:])
            wout = big.tile([128, NH, K], dt.bfloat16, tag="wout")
            for ch in range(NH):
                nc.sync.dma_start(wout[:, ch], wout_d[ch])
            bout = big.tile([K, 1], f32, tag="bout")
            nc.sync.dma_start(bout[:], bout_d[:])
            ident = big.tile([128, 128], dt.bfloat16, tag="ident")
            nc.sync.dma_start(ident[:], id_d[:])
            # stage via DVE copies (single-DMA-queue-consumer rule)
            ohT_raw = big.tile([K, T, BL], f32, tag="em")  # slot later: u, em
            nc.sync.dma_start(ohT_raw[:], oh_d[:])
            ohT = big.tile([K, T, BL], f32, tag="ohT")
            nc.vector.tensor_copy(ohT[:], ohT_raw[:])
            crf_raw = big.tile([K, 32], f32, tag="crf_raw")
            nc.sync.dma_start(crf_raw[:], crf_d[:])
            crf = big.tile([K, 32], f32, tag="crf")
            nc.vector.tensor_copy(crf[:], crf_raw[:])
            lmask_raw = big.tile([1, BL], f32, tag="lmask_raw")
            nc.sync.dma_start(lmask_raw[:], lmask_d[:])
            lmask = big.tile([1, BL], f32, tag="lmask")
            nc.vector.tensor_copy(lmask[:], lmask_raw[:])

            h0 = big.tile([128, NH, T, BL], dt.bfloat16, tag="h0")
            h1r = big.tile([128, NH, RB, BL], dt.bfloat16, tag="h1r")

            # DRAM bounce buffers for the pairwise exchanges. h0 is exchanged
            # in NS pipelined segments, each fired as soon as its blocks are
            # computed; only the last segment's latency is exposed at the
            # layer boundary.
            NS = 4
            SEG = T // NS
            b0_in = [
                dram.tile([128, NH, SEG, BL], dt.bfloat16, tag=f"b0_in{s}",
                          name=f"b0_in{s}")
                for s in range(NS)
            ]
            b0_out = [
                nc.dram_tensor(f"b0_out{s}", [2, 128, NH, SEG, BL], dt.bfloat16)
                for s in range(NS)
            ]
            b1_in = dram.tile([K, T, BL], f32, tag="b1_in")
            b1_out = nc.dram_tensor("b1_out", [2, K, T, BL], f32)

            em = None  # allocated after the h0 exchange (shares slot with u)

            # numerator transition-pairs scratch: filled one tile per layer-0
            # block (depends only on ohT/crf), reduced at layer-0 end
            oh_flat = ohT[:].rearrange("k t b -> k (t b)")
            NTC = 512
            NQm = NQ - BL
            scr2 = big.tile([K, BL, T], f32, tag="XC")
            scr2_tb = scr2[:].rearrange("k b t -> k t b")  # [K, T, BL]
            tr_t = tmp.tile([K, BL], f32, tag="trt")

            def pairs_task(nt):
                n0 = nt * NTC
                n1 = min(n0 + NTC, NQm)
                pa = ps.tile([K, NTC], f32, tag="misc", bufs=1, name=f"pa{nt}")
                nc.tensor.matmul(pa[:, 0:n1 - n0], crf[:, 0:K], oh_flat[:, n0:n1],
                                 start=True, stop=True)
                nc.vector.tensor_tensor(
                    scr2_tb[:, n0 // BL:n1 // BL, :],
                    pa[:, 0:n1 - n0], oh_flat[:, n0 + BL:n1 + BL],
                    mybir.AluOpType.mult,
                )
                if nt == NB - 1:
                    nc.vector.tensor_reduce(
                        tr_t[:], scr2[:, :, 0:T - 1], mybir.AxisListType.X,
                        mybir.AluOpType.add,
                    )

            class XgEmitter:
                """Incrementally emits the input-projection matmuls for one
                32-step block (12 gate chunks x 6 contract chunks) so they can
                be interleaved into the recurrence as PE filler."""

                def __init__(self, layer, blk, xg_tile, wih):
                    self.layer, self.blk, self.xg, self.wih = layer, blk, xg_tile, wih
                    self.j, self.kc, self.p = 0, 0, None

                def rhs(self, kc):
                    sl = slice(self.blk * BLK, (self.blk + 1) * BLK)
                    if self.layer == 0:
                        return xT[:, kc, sl, :]
                    if kc < NH:
                        return h0[:, kc, sl, :]
                    return xT[:, kc - NH, sl, :]  # partner h0 lives in xT[:, 0:3]

                def step(self):
                    if self.j >= NG:
                        return False
                    if self.kc == 0:
                        self.p = ps.tile([128, BLK * BL], dt.float32, tag="pxg", bufs=2)
                    j = self.j
                    nc.tensor.matmul(
                        self.p[:],
                        self.wih[:, self.kc, j * 128:(j + 1) * 128],
                        self.rhs(self.kc),
                        start=(self.kc == 0),
                        stop=(self.kc == NE - 1),
                    )
                    self.kc += 1
                    if self.kc == NE:
                        bcol = self.layer * NG + j
                        nc.scalar.add(self.xg[:, j], self.p[:], bias[:, bcol:bcol + 1])
                        self.kc = 0
                        self.j += 1
                    return True

                def drain(self):
                    while self.step():
                        pass

            # ---- two LSTM layers (one direction each; SPMD over cores) ----
            for layer in range(2):
                wih = big.tile([128, NE, G], dt.bfloat16, tag="wih")
                w_src = w0_d if layer == 0 else w1_d
                for ch in range(NE):
                    nc.sync.dma_start(wih[:, ch], w_src[ch])
                whh = big.tile([128, NH, G], dt.bfloat16, tag="whh")
                for kc in range(NH):
                    nc.sync.dma_start(whh[:, kc], whh_d[layer, kc])

                if layer == 1:
                    em = big.tile([K, T, BL], f32, tag="em")

                def h_chunk(t, kc):
                    if layer == 0:
                        return h0[:, kc, t, :]
                    return h1r[:, kc, t % RB, :]

                def h_full(t):
                    if layer == 0:
                        return h0[:, :, t, :]
                    return h1r[:, :, t % RB, :]

                xg_cur = xgp.tile([128, NG, BLK * BL], dt.bfloat16, tag="xg", bufs=2)
                em0 = XgEmitter(layer, 0, xg_cur, wih)
                em0.drain()

                c_st = None
                for blk in range(NB):
                    if blk + 1 < NB:
                        xg_nxt = xgp.tile(
                            [128, NG, BLK * BL], dt.bfloat16, tag="xg", bufs=2
                        )
                        nxt = XgEmitter(layer, blk + 1, xg_nxt, wih)
                    else:
                        xg_nxt, nxt = None, None

                    for tl in range(BLK):
                        t = blk * BLK + tl
                        first = t == 0
                        u0 = tl * BL

                        # Gate pre-activations land in three separate PSUM
                        # banks (i+f, g, o). Each bank's group: recurrent
                        # whh matmuls plus one identity-matmul per gate chunk
                        # that injects xg (incl. bias) straight into PSUM —
                        # no DVE pre-adds, and each activation reads its bank
                        # as soon as that group retires while the PE streams
                        # the next group.
                        gp_if = ps.tile([128, 2 * NH, BL], f32, tag="gp_if", bufs=1)
                        gp_g = ps.tile([128, NH, BL], f32, tag="gp_g", bufs=1)
                        gp_o = ps.tile([128, NH, BL], f32, tag="gp_o", bufs=1)

                        def emit_group(tile_, j0, nj):
                            # ONE identity matmul FIRST (start=True): it has
                            # no dependency on h(t-1), so it issues during the
                            # previous step's elementwise tail and injects xg
                            # for the whole group; the nj*NH recurrent matmuls
                            # then accumulate on top. kc-outer order: the
                            # first matmuls need only h chunk 0, which the
                            # chunk-split hmul below writes first.
                            total = (0 if first else nj * NH) + 1
                            nc.tensor.matmul(
                                tile_[:],
                                ident[:],
                                xg_cur[:, j0:j0 + nj, u0:u0 + BL],
                                start=True,
                                stop=(total == 1),
                            )
                            n = 1
                            if not first:
                                for kc in range(NH):
                                    for jj in range(nj):
                                        j = j0 + jj
                                        nc.tensor.matmul(
                                            tile_[:, jj],
                                            whh[:, kc, j * 128:(j + 1) * 128],
                                            h_chunk(t - 1, kc),
                                            start=False,
                                            stop=(n == total - 1),
                                        )
                                        n += 1

                        emit_group(gp_if, 0, 2 * NH)
                        emit_group(gp_g, 2 * NH, NH)
                        emit_group(gp_o, 3 * NH, NH)

                        sif = tmp.tile([128, 2 * NH, BL], f32, tag="sif")
                        nc.scalar.activation(sif[:], gp_if[:], AF.Sigmoid)
                        tg = tmp.tile([128, NH, BL], f32, tag="tg")
                        nc.scalar.activation(tg[:], gp_g[:], AF.Tanh)
                        so = tmp.tile([128, NH, BL], f32, tag="so")
                        nc.scalar.activation(so[:], gp_o[:], AF.Sigmoid)

                        cN = state.tile([128, NH, BL], f32, tag="c")
                        if first:
                            nc.vector.tensor_mul(cN[:], sif[:, 0:NH], tg[:])
                        else:
                            t2 = tmp.tile([128, NH, BL], f32, tag="t2")
                            nc.vector.tensor_mul(t2[:], sif[:, NH:2 * NH], c_st[:])
                            t1 = tmp.tile([128, NH, BL], f32, tag="t1")
                            nc.vector.tensor_mul(t1[:], sif[:, 0:NH], tg[:])
                            nc.vector.tensor_add(cN[:], t1[:], t2[:])
                        c_st = cN

                        tc_t = tmp.tile([128, NH, BL], f32, tag="tc")
                        nc.scalar.activation(tc_t[:], cN[:], AF.Tanh)
                        nc.vector.tensor_mul(h_full(t), so[:], tc_t[:])

                        # PE filler: next block's input projections
                        if nxt is not None:
                            for _ in range(3):
                                nxt.step()

                    if nxt is not None:
                        nxt.drain()
                        xg_cur = xg_nxt

                    if layer == 0:
                        # send this h0 block time-reversed into its segment
                        # bounce (per chunk: DMA APs are limited to 3 dims)
                        seg = blk // (BLK_PER_SEG := NB // NS)
                        t1r = SEG * (seg + 1) - blk * BLK
                        rsl = rev_slice(t1r - BLK, t1r)
                        for c in range(NH):
                            nc.sync.dma_start(
                                b0_in[seg][:, c, rsl, :],
                                h0[:, c, blk * BLK:(blk + 1) * BLK, :],
                            )
                        pairs_task(blk)
                        if blk % BLK_PER_SEG == BLK_PER_SEG - 1 and seg < NS - 1:
                            # segment collective fires as soon as its blocks
                            # are sent; transfer hides under remaining compute
                            nc.gpsimd.collective_compute(
                                "AllGather",
                                mybir.AluOpType.bypass,
                                replica_groups=groups,
                                ins=[b0_in[seg][:].opt()],
                                outs=[b0_out[seg][:].opt()],
                            )
                    else:
                        # emissions for the ring block just completed
                        r0 = (blk % 2) * BLK
                        pem = ps.tile([K, BLK, BL], f32, tag="misc", bufs=1)
                        for kc in range(NH):
                            nc.tensor.matmul(
                                pem[:],
                                wout[:, kc],
                                h1r[:, kc, r0:r0 + BLK, :],
                                start=(kc == 0),
                                stop=(kc == NH - 1),
                            )
                        nc.scalar.add(
                            em[:, blk * BLK:(blk + 1) * BLK, :], pem[:], bout[:, 0:1]
                        )

                if layer == 0:
                    # ---- last h0 segment exchange (only this one's latency
                    # is exposed) ----
                    nc.gpsimd.collective_compute(
                        "AllGather",
                        mybir.AluOpType.bypass,
                        replica_groups=groups,
                        ins=[b0_in[NS - 1][:].opt()],
                        outs=[b0_out[NS - 1][:].opt()],
                    )
                    # partner h0 = (slot0 + slot1) - own(reversed); fp32 sum
                    # makes the bf16 cancellation exact. Descending segments:
                    # the last segment holds partner t 0.., consumed first.
                    for s in range(NS - 1, -1, -1):
                        lo = T - SEG * (s + 1)
                        for c in range(NH):
                            s0c = big.tile([128, SEG, BL], dt.bfloat16, tag="XC")
                            nc.sync.dma_start(s0c[:], b0_out[s][0, :, c])
                            s1c = big.tile([128, SEG, BL], dt.bfloat16, tag="XD")
                            nc.sync.dma_start(s1c[:], b0_out[s][1, :, c])
                            u = big.tile([128, SEG, BL], f32, tag="em")
                            nc.vector.tensor_add(u[:], s0c[:], s1c[:])
                            nc.vector.tensor_sub(
                                xT[:, c, lo:lo + SEG, :], u[:],
                                h0[:, c, rev_slice(T - lo - SEG, T - lo), :],
                            )

            # ---- emissions exchange: em_full = own partial + partner partial ----
            nc.sync.dma_start(b1_in[:, ::-1, :], em[:])
            nc.gpsimd.collective_compute(
                "AllGather",
                mybir.AluOpType.bypass,
                replica_groups=groups,
                ins=[b1_in[:].opt()],
                outs=[b1_out[:].opt()],
            )
            s0e = big.tile([K, T, BL], f32, tag="wih")
            nc.sync.dma_start(s0e[:], b1_out[0])
            s1e = big.tile([K, T, BL], f32, tag="XC")
            nc.sync.dma_start(s1e[:], b1_out[1])
            nc.vector.tensor_add(s0e[:], s0e[:], s1e[:])
            em_rev = big.tile([K, T, BL], f32, tag="whh")
            nc.vector.tensor_copy(em_rev[:], em[:, ::-1, :])
            nc.vector.tensor_sub(em[:], s0e[:], em_rev[:])

            # ---- gold path score (numerator) ----
            scr = big.tile([K, BL, T], f32, tag="wih")
            nkb = tmp.tile([K, BL], f32, tag="nkb")
            nc.vector.tensor_tensor(
                scr[:].rearrange("k b t -> k t b"),
                em[:], ohT[:], mybir.AluOpType.mult,
            )
            nc.vector.tensor_reduce(
                nkb[:], scr[:], mybir.AxisListType.X, mybir.AluOpType.add
            )
            nc.vector.tensor_add(nkb[:], nkb[:], tr_t[:])
            pnum = ps.tile([1, BL], f32, tag="misc", bufs=1)
            nc.tensor.matmul(pnum[:], crf[:, 22:23], nkb[:], start=True, stop=False)
            nc.tensor.matmul(pnum[:], crf[:, 20:21], ohT[:, 0, :], start=False, stop=False)
            nc.tensor.matmul(pnum[:], crf[:, 21:22], ohT[:, T - 1, :], start=False, stop=True)
            num = tmp.tile([1, BL], f32, tag="num")
            nc.vector.tensor_copy(num[:], pnum[:])

            # ---- CRF forward algorithm (denominator), linear space ----
            # two independent 8-sequence chains interleaved so each chain's
            # PE->DVE latency hides under the other's ops
            eem = big.tile([K, T, BL], f32, tag="wih")
            nc.scalar.activation(eem[:], em[:], AF.Exp)
            HB = BL // 2
            ea = [None, None]
            logc = [None, None]
            for ch in range(2):
                eac = state.tile([K, HB], f32, tag=f"ea{ch}")
                nc.vector.tensor_tensor(
                    eac[:], eem[:, 0, ch * HB:(ch + 1) * HB],
                    crf[:, 18:19].broadcast_to((K, HB)),
                    mybir.AluOpType.mult,
                )
                ea[ch] = eac
            for t_ in range(1, T):
                pea = [None, None]
                for ch in range(2):
                    pea[ch] = ps.tile([K, HB], f32, tag=f"crf{ch}", bufs=1, name=f"pea{ch}")
                    nc.tensor.matmul(pea[ch][:], crf[:, 9:9 + K], ea[ch][:],
                                     start=True, stop=True)
                for ch in range(2):
                    eaN = state.tile([K, HB], f32, tag=f"ea{ch}")
                    nc.vector.tensor_tensor(
                        eaN[:], pea[ch][:], eem[:, t_, ch * HB:(ch + 1) * HB],
                        mybir.AluOpType.mult,
                    )
                    ea[ch] = eaN
                if t_ % CRF_S == 0:
                    r = [None, None]
                    for ch in range(2):
                        r[ch] = tmp.tile([1, HB], f32, tag=f"crf_r{ch}", name=f"r{ch}")
                        nc.vector.reciprocal(r[ch][:], ea[ch][0:1, :])
                    pbc = [None, None]
                    for ch in range(2):
                        pbc[ch] = ps.tile([K, HB], f32, tag=f"crf{ch}", bufs=1, name=f"pbc{ch}")
                        nc.tensor.matmul(pbc[ch][:], crf[0:1, 23:23 + K], r[ch][:],
                                         start=True, stop=True)
                    for ch in range(2):
                        lg = tmp.tile([1, HB], f32, tag=f"crf_lg{ch}")
                        nc.scalar.activation(lg[:], ea[ch][0:1, :], AF.Ln)
                        eaN2 = state.tile([K, HB], f32, tag=f"ea{ch}")
                        nc.vector.tensor_tensor(
                            eaN2[:], ea[ch][:], pbc[ch][:], mybir.AluOpType.mult
                        )
                        logcN = state.tile([1, HB], f32, tag=f"logc{ch}")
                        if logc[ch] is None:
                            nc.vector.tensor_copy(logcN[:], lg[:])
                        else:
                            nc.vector.tensor_add(logcN[:], logc[ch][:], lg[:])
                        logc[ch] = logcN
                        ea[ch] = eaN2
            eacat = tmp.tile([K, BL], f32, tag="eacat")
            for ch in range(2):
                nc.vector.tensor_copy(eacat[:, ch * HB:(ch + 1) * HB], ea[ch][:])
            pden = ps.tile([1, BL], f32, tag="misc", bufs=1)
            nc.tensor.matmul(pden[:], crf[:, 19:20], eacat[:], start=True, stop=True)
            den = tmp.tile([1, BL], f32, tag="den")
            nc.scalar.activation(den[:], pden[:], AF.Ln)
            lcat = tmp.tile([1, BL], f32, tag="lcat")
            for ch in range(2):
                nc.vector.tensor_copy(lcat[:, ch * HB:(ch + 1) * HB], logc[ch][:])
            nc.vector.tensor_add(den[:], den[:], lcat[:])

            # ---- loss = sum_b mask_b * (den_b - num_b) ----
            diff = tmp.tile([1, BL], f32, tag="diff")
            nc.vector.tensor_sub(diff[:], den[:], num[:])
            nc.vector.tensor_mul(diff[:], diff[:], lmask[:])
            lout = tmp.tile([1, 1], f32, tag="lout")
            nc.vector.tensor_reduce(
                lout[:], diff[:], mybir.AxisListType.X, mybir.AluOpType.add
            )
            nc.sync.dma_start(loss_d[:], lout[:])

    if legalize:
        split_waits(nc)
    nc.finalize()
    return nc


def stage_inputs(inputs):
    x = np.asarray(inputs["embedding"], np.float32)
    tags = np.asarray(inputs["target_tag"]).astype(np.int64)

    def pget(name):
        return np.asarray(inputs[name], np.float32)

    def wihT(name, row_order=None):
        w = pget(name)[GATE_PERM]            # [1536, in]
        wT = w.T                             # [in, 1536]
        if row_order is not None:
            wT = wT[row_order]
        return np.ascontiguousarray(wT).reshape(-1, 128, G).astype(bf16)

    def whhT(name):
        w = pget(name)[GATE_PERM]
        return np.ascontiguousarray(w.T).reshape(NH, 128, G).astype(bf16)

    def biasv(name):
        return pget(name)[GATE_PERM].reshape(NG, 128).T

    trans, st, et = pget("trans"), pget("start_trans"), pget("end_trans")
    w_out, b_out = pget("w_out"), pget("b_out")

    in_maps = []
    for c in range(NC):
        p, par = divmod(c, 2)
        d = "f" if par == 0 else "b"
        xs = x[16 * p:16 * p + 16]
        tg = tags[16 * p:16 * p + 16]
        if par:
            xs = xs[:, ::-1]
            tg = tg[:, ::-1]
        xT_c = np.ascontiguousarray(xs.transpose(2, 1, 0)).reshape(
            NE, 128, T, BL).astype(bf16)

        w0 = wihT(f"w_ih_0{d}")
        own = np.arange(0, H) if par == 0 else np.arange(H, 2 * H)
        oth = np.arange(H, 2 * H) if par == 0 else np.arange(0, H)
        w1 = wihT(f"w_ih_1{d}", row_order=np.concatenate([own, oth]))
        whh = np.stack([whhT(f"w_hh_0{d}"), whhT(f"w_hh_1{d}")])
        bias = np.concatenate([biasv(f"b_0{d}"), biasv(f"b_1{d}")], axis=1).astype(
            np.float32)
        wh = w_out[:, 0:H] if par == 0 else w_out[:, H:2 * H]
        woutT = np.ascontiguousarray(wh.T).reshape(NH, 128, K).astype(bf16)
        bout = (b_out if par == 0 else np.zeros(K, np.float32)).reshape(K, 1)

        oh = np.zeros((K, T, BL), np.float32)
        oh[tg.T.reshape(-1), np.repeat(np.arange(T), BL), np.tile(np.arange(BL), T)] = 1.0

        tr_eff = trans if par == 0 else np.ascontiguousarray(trans.T)
        st_eff = st if par == 0 else et
        et_eff = et if par == 0 else st
        crf_c = np.zeros((K, 32), np.float32)
        crf_c[:, 0:9] = tr_eff
        crf_c[:, 9:18] = np.exp(tr_eff)
        crf_c[:, 18] = np.exp(st_eff)
        crf_c[:, 19] = np.exp(et_eff)
        crf_c[:, 20] = st_eff
        crf_c[:, 21] = et_eff
        crf_c[:, 22] = 1.0
        crf_c[0, 23:32] = 1.0
        lm = np.zeros((1, BL), np.float32)
        if par == 0:
            lm[0, 0:8] = 1.0
        else:
            lm[0, 8:16] = 1.0

        in_maps.append(
            dict(
                xT=xT_c, w0T=w0, w1T=w1, whhT=whh, bias=bias, woutT=woutT,
                bout=bout, ohT=np.ascontiguousarray(oh), crf=crf_c, lmask=lm,
                ident=np.eye(128, dtype=bf16),
            )
        )
    return in_maps


_NC_CACHE = {}


def get_nc():
    if "nc" not in _NC_CACHE:
        _NC_CACHE["nc"] = build_nc()
    return _NC_CACHE["nc"]


def kernel(**inputs):
    from concourse.bass_utils import run_bass_kernel_spmd

    nc = get_nc()
    in_maps = stage_inputs(inputs)
    res = run_bass_kernel_spmd(nc, in_maps, list(range(NC)))
    total = np.float32(0.0)
    for r in res.results:
        total += np.float32(r["loss"].reshape(-1)[0])
    return np.asarray(total, dtype=np.float32)


# revision 8
# speedup vs baseline: 1.3053x; 1.0002x over previous
"""BiLSTM(2-layer) + CRF NLL Trainium2 kernel, v2: direction-split sharding.

8 cores = 4 pairs. Pair p owns 16 sequences; core 2p runs the FORWARD
direction of both LSTM layers for those 16 sequences, core 2p+1 the BACKWARD
direction. Backward cores see time-reversed inputs, so every core runs an
identical forward-scan program; all direction asymmetry lives in host staging
(weights, reversed inputs, transposed CRF transitions, swapped start/end).

Between layers the pair exchanges hidden states with a 2-core AllGather
(bounce via DRAM, sent time-reversed so the partner receives data in its own
time order); each core reconstructs the partner's h via
(slot0 + slot1) - own, computed in fp32 so the bf16 cancellation is exact.
Emissions are per-direction partials pair-summed the same way. Each core then
runs the CRF on all 16 sequences and masks the per-sequence losses so each
sequence is counted on exactly one core.

vs v1: the serial recurrent matmul chain per core drops from 36864 LDW+MM
pairs (N=8) to 18360 (N=16), and input-projection/emission matmuls are
interleaved into the recurrence as PE filler during the per-step elementwise
tails. Gate chunks are reordered [g, i, f, o] so tanh(g)/sigmoid(i,f) start
before the step's matmuls finish and only sigmoid(o) trails them.
"""

import sys
import numpy as np
import ml_dtypes

sys.path.insert(0, "/opt/trn_rl_repo")

import concourse.bass as bass
import concourse.mybir as mybir
import concourse.tile as tile

dt = mybir.dt
AF = mybir.ActivationFunctionType
bf16 = ml_dtypes.bfloat16

# problem constants
B, T, E, H, K = 64, 256, 768, 384, 9
NC = 8
BL = 16         # sequences per core (one direction)
G = 4 * H       # 1536
NE = 6          # input contract chunks (768/128, both layers)
NH = H // 128   # 3
NG = G // 128   # 12
BLK = 32        # timesteps per xg block
NB = T // BLK   # 8
NQ = T * BL     # 4096
CRF_S = 8

# gate chunk order [i, f, g, o] = native pytorch order. The step's matmuls
# run as three PSUM-bank groups (i+f, g, o) so each activation starts as soon
# as its bank's accumulation retires, overlapping the rest of the matmul
# stream; o last so only sigmoid(o) + one mul trail the final matmul.
GATE_PERM = np.arange(4 * H)


def split_waits(nc):
    """Legalize sem waits: walrus accepts at most one sync wait per
    instruction; hoist extra waits onto same-engine NoOps."""
    import bass_rust

    n_split = 0
    for f in nc.m.functions:
        for blk in f.blocks:
            out = []
            changed = False
            for inst in blk.instructions:
                si = inst.sync_info
                if si is not None and si.on_wait and len(si.on_wait) > 1:
                    waits = list(si.on_wait)
                    for k, w in enumerate(waits[:-1]):
                        nop = mybir.InstNoOp(name=f"{inst.name}_w{k}", ins=[], outs=[])
                        nop.engine = inst.engine
                        nop.sync_info = bass_rust.SyncInfo(on_wait=[w], on_update=[])
                        out.append(nop)
                        n_split += 1
                    inst.sync_info = bass_rust.SyncInfo(
                        on_wait=[waits[-1]], on_update=list(si.on_update or [])
                    )
                    changed = True
                out.append(inst)
            if changed:
                blk.instructions = out
    return n_split


def rev_slice(a, b):
    """slice covering [a, b) traversed in reverse order."""
    return slice(b - 1, None if a == 0 else a - 1, -1)


def build_nc(legalize=True):
    nc = bass.Bass(trn_type="TRN2", num_devices=NC)
    f32 = dt.float32
    groups = [[2 * p, 2 * p + 1] for p in range(NC // 2)]

    xT_d = nc.declare_dram_parameter("xT", [NE, 128, T, BL], dt.bfloat16, False)
    w0_d = nc.declare_dram_parameter("w0T", [NE, 128, G], dt.bfloat16, False)
    w1_d = nc.declare_dram_parameter("w1T", [NE, 128, G], dt.bfloat16, False)
    whh_d = nc.declare_dram_parameter("whhT", [2, NH, 128, G], dt.bfloat16, False)
    bias_d = nc.declare_dram_parameter("bias", [128, 2 * NG], f32, False)
    wout_d = nc.declare_dram_parameter("woutT", [NH, 128, K], dt.bfloat16, False)
    bout_d = nc.declare_dram_parameter("bout", [K, 1], f32, False)
    oh_d = nc.declare_dram_parameter("ohT", [K, T, BL], f32, False)
    id_d = nc.declare_dram_parameter("ident", [128, 128], dt.bfloat16, False)
    crf_d = nc.declare_dram_parameter("crf", [K, 32], f32, False)
    lmask_d = nc.declare_dram_parameter("lmask", [1, BL], f32, False)
    loss_d = nc.declare_dram_parameter("loss", [1, 1], f32, True)

    RB = 2 * BLK  # h1 ring length (timesteps); emissions drain a block behind

    with tile.TileContext(nc) as tc:
        with (
            tc.tile_pool(name="big", bufs=1) as big,
            tc.tile_pool(name="state", bufs=2) as state,
            tc.tile_pool(name="tmp", bufs=3) as tmp,
            tc.tile_pool(name="xgp", bufs=1) as xgp,
            tc.tile_pool(name="ps", bufs=2, space="PSUM") as ps,
            tc.tile_pool(name="dram", bufs=1, space="DRAM") as dram,
        ):
            # ---- persistent loads ----
            xT = big.tile([128, NE, T, BL], dt.bfloat16, tag="xT")
            for ch in range(NE):
                nc.sync.dma_start(xT[:, ch], xT_d[ch])
            bias = big.tile([128, 2 * NG], f32, tag="bias")
            nc.sync.dma_start(bias[:], bias_d[:])
            wout = big.tile([128, NH, K], dt.bfloat16, tag="wout")
            for ch in range(NH):
                nc.sync.dma_start(wout[:, ch], wout_d[ch])
            bout = big.tile([K, 1], f32, tag="bout")
            nc.sync.dma_start(bout[:], bout_d[:])
            ident = big.tile([128, 128], dt.bfloat16, tag="ident")
            nc.sync.dma_start(ident[:], id_d[:])
            # stage via DVE copies (single-DMA-queue-consumer rule)
            ohT_raw = big.tile([K, T, BL], f32, tag="em")  # slot later: u, em
            nc.sync.dma_start(ohT_raw[:], oh_d[:])
            ohT = big.tile([K, T, BL], f32, tag="ohT")
            nc.vector.tensor_copy(ohT[:], ohT_raw[:])
            crf_raw = big.tile([K, 32], f32, tag="crf_raw")
            nc.sync.dma_start(crf_raw[:], crf_d[:])
            crf = big.tile([K, 32], f32, tag="crf")
            nc.vector.tensor_copy(crf[:], crf_raw[:])
            lmask_raw = big.tile([1, BL], f32, tag="lmask_raw")
            nc.sync.dma_start(lmask_raw[:], lmask_d[:])
            lmask = big.tile([1, BL], f32, tag="lmask")
            nc.vector.tensor_copy(lmask[:], lmask_raw[:])

            h0 = big.tile([128, NH, T, BL], dt.bfloat16, tag="h0")
            h1r = big.tile([128, NH, RB, BL], dt.bfloat16, tag="h1r")

            # DRAM bounce buffers for the pairwise exchanges. h0 is exchanged
            # in NS pipelined segments, each fired as soon as its blocks are
            # computed; only the last segment's latency is exposed at the
            # layer boundary.
            NS = 4
            SEG = T // NS
            b0_in = [
                dram.tile([128, NH, SEG, BL], dt.bfloat16, tag=f"b0_in{s}",
                          name=f"b0_in{s}")
                for s in range(NS)
            ]
            b0_out = [
                nc.dram_tensor(f"b0_out{s}", [2, 128, NH, SEG, BL], dt.bfloat16)
                for s in range(NS)
            ]
            b1_in = dram.tile([K, T, BL], f32, tag="b1_in")
            b1_out = nc.dram_tensor("b1_out", [2, K, T, BL], f32)

            em = None  # allocated after the h0 exchange (shares slot with u)

            # numerator transition-pairs scratch: filled one tile per layer-0
            # block (depends only on ohT/crf), reduced at layer-0 end
            oh_flat = ohT[:].rearrange("k t b -> k (t b)")
            NTC = 512
            NQm = NQ - BL
            scr2 = big.tile([K, BL, T], f32, tag="XC")
            scr2_tb = scr2[:].rearrange("k b t -> k t b")  # [K, T, BL]
            tr_t = tmp.tile([K, BL], f32, tag="trt")

            def pairs_task(nt):
                n0 = nt * NTC
                n1 = min(n0 + NTC, NQm)
                pa = ps.tile([K, NTC], f32, tag="misc", bufs=1, name=f"pa{nt}")
                nc.tensor.matmul(pa[:, 0:n1 - n0], crf[:, 0:K], oh_flat[:, n0:n1],
                                 start=True, stop=True)
                nc.vector.tensor_tensor(
                    scr2_tb[:, n0 // BL:n1 // BL, :],
                    pa[:, 0:n1 - n0], oh_flat[:, n0 + BL:n1 + BL],
                    mybir.AluOpType.mult,
                )
                if nt == NB - 1:
                    nc.vector.tensor_reduce(
                        tr_t[:], scr2[:, :, 0:T - 1], mybir.AxisListType.X,
                        mybir.AluOpType.add,
                    )

            class XgEmitter:
                """Incrementally emits the input-projection matmuls for one
                32-step block (12 gate chunks x 6 contract chunks) so they can
                be interleaved into the recurrence as PE filler."""

                def __init__(self, layer, blk, xg_tile, wih):
                    self.layer, self.blk, self.xg, self.wih = layer, blk, xg_tile, wih
                    self.j, self.kc, self.p = 0, 0, None

                def rhs(self, kc):
                    sl = slice(self.blk * BLK, (self.blk + 1) * BLK)
                    if self.layer == 0:
                        return xT[:, kc, sl, :]
                    if kc < NH:
                        return h0[:, kc, sl, :]
                    return xT[:, kc - NH, sl, :]  # partner h0 lives in xT[:, 0:3]

                def step(self):
                    if self.j >= NG:
                        return False
                    if self.kc == 0:
                        self.p = ps.tile([128, BLK * BL], dt.float32, tag="pxg", bufs=2)
                    j = self.j
                    nc.tensor.matmul(
                        self.p[:],
                        self.wih[:, self.kc, j * 128:(j + 1) * 128],
                        self.rhs(self.kc),
                        start=(self.kc == 0),
                        stop=(self.kc == NE - 1),
                    )
                    self.kc += 1
                    if self.kc == NE:
                        bcol = self.layer * NG + j
                        nc.scalar.add(self.xg[:, j], self.p[:], bias[:, bcol:bcol + 1])
                        self.kc = 0
                        self.j += 1
                    return True

                def drain(self):
                    while self.step():
                        pass

            # ---- two LSTM layers (one direction each; SPMD over cores) ----
            for layer in range(2):
                wih = big.tile([128, NE, G], dt.bfloat16, tag="wih")
                w_src = w0_d if layer == 0 else w1_d
                for ch in range(NE):
                    nc.sync.dma_start(wih[:, ch], w_src[ch])
                whh = big.tile([128, NH, G], dt.bfloat16, tag="whh")
                for kc in range(NH):
                    nc.sync.dma_start(whh[:, kc], whh_d[layer, kc])

                if layer == 1:
                    em = big.tile([K, T, BL], f32, tag="em")

                def h_chunk(t, kc):
                    if layer == 0:
                        return h0[:, kc, t, :]
                    return h1r[:, kc, t % RB, :]

                def h_full(t):
                    if layer == 0:
                        return h0[:, :, t, :]
                    return h1r[:, :, t % RB, :]

                xg_cur = xgp.tile([128, NG, BLK * BL], dt.bfloat16, tag="xg", bufs=2)
                em0 = XgEmitter(layer, 0, xg_cur, wih)
                em0.drain()

                c_st = None
                for blk in range(NB):
                    if blk + 1 < NB:
                        xg_nxt = xgp.tile(
                            [128, NG, BLK * BL], dt.bfloat16, tag="xg", bufs=2
                        )
                        nxt = XgEmitter(layer, blk + 1, xg_nxt, wih)
                    else:
                        xg_nxt, nxt = None, None

                    for tl in range(BLK):
                        t = blk * BLK + tl
                        first = t == 0
                        u0 = tl * BL

                        # Gate pre-activations land in three separate PSUM
                        # banks (i+f, g, o). Each bank's group: recurrent
                        # whh matmuls plus one identity-matmul per gate chunk
                        # that injects xg (incl. bias) straight into PSUM —
                        # no DVE pre-adds, and each activation reads its bank
                        # as soon as that group retires while the PE streams
                        # the next group.
                        gp_if = ps.tile([128, 2 * NH, BL], f32, tag="gp_if", bufs=1)
                        gp_g = ps.tile([128, NH, BL], f32, tag="gp_g", bufs=1)
                        gp_o = ps.tile([128, NH, BL], f32, tag="gp_o", bufs=1)

                        def emit_group(tile_, j0, nj):
                            # ONE identity matmul FIRST (start=True): it has
                            # no dependency on h(t-1), so it issues during the
                            # previous step's elementwise tail and injects xg
                            # for the whole group; the nj*NH recurrent matmuls
                            # then accumulate on top. kc-outer order: the
                            # first matmuls need only h chunk 0, which the
                            # chunk-split hmul below writes first.
                            total = (0 if first else nj * NH) + 1
                            nc.tensor.matmul(
                                tile_[:],
                                ident[:],
                                xg_cur[:, j0:j0 + nj, u0:u0 + BL],
                                start=True,
                                stop=(total == 1),
                            )
                            n = 1
                            if not first:
                                for jj in range(nj):
                                    j = j0 + jj
                                    for kc in range(NH):
                                        nc.tensor.matmul(
                                            tile_[:, jj],
                                            whh[:, kc, j * 128:(j + 1) * 128],
                                            h_chunk(t - 1, kc),
                                            start=False,
                                            stop=(n == total - 1),
                                        )
                                        n += 1

                        emit_group(gp_if, 0, 2 * NH)
                        emit_group(gp_g, 2 * NH, NH)
                        emit_group(gp_o, 3 * NH, NH)

                        sif = tmp.tile([128, 2 * NH, BL], f32, tag="sif")
                        nc.scalar.activation(sif[:], gp_if[:], AF.Sigmoid)
                        tg = tmp.tile([128, NH, BL], f32, tag="tg")
                        nc.scalar.activation(tg[:], gp_g[:], AF.Tanh)
                        so = tmp.tile([128, NH, BL], f32, tag="so")
                        nc.scalar.activation(so[:], gp_o[:], AF.Sigmoid)

                        cN = state.tile([128, NH, BL], f32, tag="c")
                        if first:
                            nc.vector.tensor_mul(cN[:], sif[:, 0:NH], tg[:])
                        else:
                            t2 = tmp.tile([128, NH, BL], f32, tag="t2")
                            nc.vector.tensor_mul(t2[:], sif[:, NH:2 * NH], c_st[:])
                            t1 = tmp.tile([128, NH, BL], f32, tag="t1")
                            nc.vector.tensor_mul(t1[:], sif[:, 0:NH], tg[:])
                            nc.vector.tensor_add(cN[:], t1[:], t2[:])
                        c_st = cN

                        tc_t = tmp.tile([128, NH, BL], f32, tag="tc")
                        nc.scalar.activation(tc_t[:], cN[:], AF.Tanh)
                        nc.vector.tensor_mul(h_full(t), so[:], tc_t[:])

                        # PE filler: next block's input projections
                        if nxt is not None:
                            for _ in range(3):
                                nxt.step()

                    if nxt is not None:
                        nxt.drain()
                        xg_cur = xg_nxt

                    if layer == 0:
                        # send this h0 block time-reversed into its segment
                        # bounce (per chunk: DMA APs are limited to 3 dims)
                        seg = blk // (BLK_PER_SEG := NB // NS)
                        t1r = SEG * (seg + 1) - blk * BLK
                        rsl = rev_slice(t1r - BLK, t1r)
                        for c in range(NH):
                            nc.sync.dma_start(
                                b0_in[seg][:, c, rsl, :],
                                h0[:, c, blk * BLK:(blk + 1) * BLK, :],
                            )
                        pairs_task(blk)
                        if blk % BLK_PER_SEG == BLK_PER_SEG - 1 and seg < NS - 1:
                            # segment collective fires as soon as its blocks
                            # are sent; transfer hides under remaining compute
                            nc.gpsimd.collective_compute(
                                "AllGather",
                                mybir.AluOpType.bypass,
                                replica_groups=groups,
                                ins=[b0_in[seg][:].opt()],
                                outs=[b0_out[seg][:].opt()],
                            )
                    else:
                        # emissions for the ring block just completed
                        r0 = (blk % 2) * BLK
                        pem = ps.tile([K, BLK, BL], f32, tag="misc", bufs=1)
                        for kc in range(NH):
                            nc.tensor.matmul(
                                pem[:],
                                wout[:, kc],
                                h1r[:, kc, r0:r0 + BLK, :],
                                start=(kc == 0),
                                stop=(kc == NH - 1),
                            )
                        nc.scalar.add(
                            em[:, blk * BLK:(blk + 1) * BLK, :], pem[:], bout[:, 0:1]
                        )

                if layer == 0:
                    # ---- last h0 segment exchange (only this one's latency
                    # is exposed) ----
                    nc.gpsimd.collective_compute(
                        "AllGather",
                        mybir.AluOpType.bypass,
                        replica_groups=groups,
                        ins=[b0_in[NS - 1][:].opt()],
                        outs=[b0_out[NS - 1][:].opt()],
                    )
                    # partner h0 = (slot0 + slot1) - own(reversed); fp32 sum
                    # makes the bf16 cancellation exact. Descending segments:
                    # the last segment holds partner t 0.., consumed first.
                    for s in range(NS - 1, -1, -1):
                        lo = T - SEG * (s + 1)
                        for c in range(NH):
                            s0c = big.tile([128, SEG, BL], dt.bfloat16, tag="XC")
                            nc.sync.dma_start(s0c[:], b0_out[s][0, :, c])
                            s1c = big.tile([128, SEG, BL], dt.bfloat16, tag="XD")
                            nc.sync.dma_start(s1c[:], b0_out[s][1, :, c])
                            u = big.tile([128, SEG, BL], f32, tag="em")
                            nc.vector.tensor_add(u[:], s0c[:], s1c[:])
                            nc.vector.tensor_sub(
                                xT[:, c, lo:lo + SEG, :], u[:],
                                h0[:, c, rev_slice(T - lo - SEG, T - lo), :],
                            )

            # ---- emissions exchange: em_full = own partial + partner partial ----
            nc.sync.dma_start(b1_in[:, ::-1, :], em[:])
            nc.gpsimd.collective_compute(
                "AllGather",
                mybir.AluOpType.bypass,
                replica_groups=groups,
                ins=[b1_in[:].opt()],
                outs=[b1_out[:].opt()],
            )
            s0e = big.tile([K, T, BL], f32, tag="wih")
            nc.sync.dma_start(s0e[:], b1_out[0])
            s1e = big.tile([K, T, BL], f32, tag="XC")
            nc.sync.dma_start(s1e[:], b1_out[1])
            nc.vector.tensor_add(s0e[:], s0e[:], s1e[:])
            # em_full = own + partner. The slot sum minus our (reversed) send
            # gives the PARTNER partial in our time order; our own partial
            # must be added back. (Written to a fresh tile: a reversed
            # self-read in one op would race an in-place write.)
            em2 = big.tile([K, T, BL], f32, tag="whh")
            nc.vector.tensor_sub(em2[:], s0e[:], em[:, ::-1, :])
            nc.vector.tensor_add(em2[:], em2[:], em[:])
            em = em2

            # ---- gold path score (numerator) ----
            scr = big.tile([K, BL, T], f32, tag="wih")
            nkb = tmp.tile([K, BL], f32, tag="nkb")
            nc.vector.tensor_tensor(
                scr[:].rearrange("k b t -> k t b"),
                em[:], ohT[:], mybir.AluOpType.mult,
            )
            nc.vector.tensor_reduce(
                nkb[:], scr[:], mybir.AxisListType.X, mybir.AluOpType.add
            )
            nc.vector.tensor_add(nkb[:], nkb[:], tr_t[:])
            pnum = ps.tile([1, BL], f32, tag="misc", bufs=1)
            nc.tensor.matmul(pnum[:], crf[:, 22:23], nkb[:], start=True, stop=False)
            nc.tensor.matmul(pnum[:], crf[:, 20:21], ohT[:, 0, :], start=False, stop=False)
            nc.tensor.matmul(pnum[:], crf[:, 21:22], ohT[:, T - 1, :], start=False, stop=True)
            num = tmp.tile([1, BL], f32, tag="num")
            nc.vector.tensor_copy(num[:], pnum[:])

            # ---- CRF forward algorithm (denominator), linear space ----
            # two independent 8-sequence chains interleaved so each chain's
            # PE->DVE latency hides under the other's ops
            eem = big.tile([K, T, BL], f32, tag="wih")
            nc.scalar.activation(eem[:], em[:], AF.Exp)
            HB = BL // 2
            ea = [None, None]
            logc = [None, None]
            for ch in range(2):
                eac = state.tile([K, HB], f32, tag=f"ea{ch}")
                nc.vector.tensor_tensor(
                    eac[:], eem[:, 0, ch * HB:(ch + 1) * HB],
                    crf[:, 18:19].broadcast_to((K, HB)),
                    mybir.AluOpType.mult,
                )
                ea[ch] = eac
            for t_ in range(1, T):
                pea = [None, None]
                for ch in range(2):
                    pea[ch] = ps.tile([K, HB], f32, tag=f"crf{ch}", bufs=1, name=f"pea{ch}")
                    nc.tensor.matmul(pea[ch][:], crf[:, 9:9 + K], ea[ch][:],
                                     start=True, stop=True)
                for ch in range(2):
                    eaN = state.tile([K, HB], f32, tag=f"ea{ch}")
                    nc.vector.tensor_tensor(
                        eaN[:], pea[ch][:], eem[:, t_, ch * HB:(ch + 1) * HB],
                        mybir.AluOpType.mult,
                    )
                    ea[ch] = eaN
                if t_ % CRF_S == 0:
                    r = [None, None]
                    for ch in range(2):
                        r[ch] = tmp.tile([1, HB], f32, tag=f"crf_r{ch}", name=f"r{ch}")
                        nc.vector.reciprocal(r[ch][:], ea[ch][0:1, :])
                    pbc = [None, None]
                    for ch in range(2):
                        pbc[ch] = ps.tile([K, HB], f32, tag=f"crf{ch}", bufs=1, name=f"pbc{ch}")
                        nc.tensor.matmul(pbc[ch][:], crf[0:1, 23:23 + K], r[ch][:],
                                         start=True, stop=True)
                    for ch in range(2):
                        lg = tmp.tile([1, HB], f32, tag=f"crf_lg{ch}")
                        nc.scalar.activation(lg[:], ea[ch][0:1, :], AF.Ln)
                        eaN2 = state.tile([K, HB], f32, tag=f"ea{ch}")
                        nc.vector.tensor_tensor(
                            eaN2[:], ea[ch][:], pbc[ch][:], mybir.AluOpType.mult
                        )
                        logcN = state.tile([1, HB], f32, tag=f"logc{ch}")
                        if logc[ch] is None:
                            nc.vector.tensor_copy(logcN[:], lg[:])
                        else:
                            nc.vector.tensor_add(logcN[:], logc[ch][:], lg[:])
                        logc[ch] = logcN
                        ea[ch] = eaN2
            eacat = tmp.tile([K, BL], f32, tag="eacat")
            for ch in range(2):
                nc.vector.tensor_copy(eacat[:, ch * HB:(ch + 1) * HB], ea[ch][:])
            pden = ps.tile([1, BL], f32, tag="misc", bufs=1)
            nc.tensor.matmul(pden[:], crf[:, 19:20], eacat[:], start=True, stop=True)
            den = tmp.tile([1, BL], f32, tag="den")
            nc.scalar.activation(den[:], pden[:], AF.Ln)
            lcat = tmp.tile([1, BL], f32, tag="lcat")
            for ch in range(2):
                nc.vector.tensor_copy(lcat[:, ch * HB:(ch + 1) * HB], logc[ch][:])
            nc.vector.tensor_add(den[:], den[:], lcat[:])

            # ---- loss = sum_b mask_b * (den_b - num_b) ----
            diff = tmp.tile([1, BL], f32, tag="diff")
            nc.vector.tensor_sub(diff[:], den[:], num[:])
            nc.vector.tensor_mul(diff[:], diff[:], lmask[:])
            lout = tmp.tile([1, 1], f32, tag="lout")
            nc.vector.tensor_reduce(
                lout[:], diff[:], mybir.AxisListType.X, mybir.AluOpType.add
            )
            nc.sync.dma_start(loss_d[:], lout[:])

    if legalize:
        split_waits(nc)
    nc.finalize()
    return nc


def stage_inputs(inputs):
    x = np.asarray(inputs["embedding"], np.float32)
    tags = np.asarray(inputs["target_tag"]).astype(np.int64)

    def pget(name):
        return np.asarray(inputs[name], np.float32)

    def wihT(name, row_order=None):
        w = pget(name)[GATE_PERM]            # [1536, in]
        wT = w.T                             # [in, 1536]
        if row_order is not None:
            wT = wT[row_order]
        return np.ascontiguousarray(wT).reshape(-1, 128, G).astype(bf16)

    def whhT(name):
        w = pget(name)[GATE_PERM]
        return np.ascontiguousarray(w.T).reshape(NH, 128, G).astype(bf16)

    def biasv(name):
        return pget(name)[GATE_PERM].reshape(NG, 128).T

    trans, st, et = pget("trans"), pget("start_trans"), pget("end_trans")
    w_out, b_out = pget("w_out"), pget("b_out")

    in_maps = []
    for c in range(NC):
        p, par = divmod(c, 2)
        d = "f" if par == 0 else "b"
        xs = x[16 * p:16 * p + 16]
        tg = tags[16 * p:16 * p + 16]
        if par:
            xs = xs[:, ::-1]
            tg = tg[:, ::-1]
        xT_c = np.ascontiguousarray(xs.transpose(2, 1, 0)).reshape(
            NE, 128, T, BL).astype(bf16)

        w0 = wihT(f"w_ih_0{d}")
        own = np.arange(0, H) if par == 0 else np.arange(H, 2 * H)
        oth = np.arange(H, 2 * H) if par == 0 else np.arange(0, H)
        w1 = wihT(f"w_ih_1{d}", row_order=np.concatenate([own, oth]))
        whh = np.stack([whhT(f"w_hh_0{d}"), whhT(f"w_hh_1{d}")])
        bias = np.concatenate([biasv(f"b_0{d}"), biasv(f"b_1{d}")], axis=1).astype(
            np.float32)
        wh = w_out[:, 0:H] if par == 0 else w_out[:, H:2 * H]
        woutT = np.ascontiguousarray(wh.T).reshape(NH, 128, K).astype(bf16)
        bout = (b_out if par == 0 else np.zeros(K, np.float32)).reshape(K, 1)

        oh = np.zeros((K, T, BL), np.float32)
        oh[tg.T.reshape(-1), np.repeat(np.arange(T), BL), np.tile(np.arange(BL), T)] = 1.0

        tr_eff = trans if par == 0 else np.ascontiguousarray(trans.T)
        st_eff = st if par == 0 else et
        et_eff = et if par == 0 else st
        crf_c = np.zeros((K, 32), np.float32)
        crf_c[:, 0:9] = tr_eff
        crf_c[:, 9:18] = np.exp(tr_eff)
        crf_c[:, 18] = np.exp(st_eff)
        crf_c[:, 19] = np.exp(et_eff)
        crf_c[:, 20] = st_eff
        crf_c[:, 21] = et_eff
        crf_c[:, 22] = 1.0
        crf_c[0, 23:32] = 1.0
        lm = np.zeros((1, BL), np.float32)
        if par == 0:
            lm[0, 0:8] = 1.0
        else:
            lm[0, 8:16] = 1.0

        in_maps.append(
            dict(
                xT=xT_c, w0T=w0, w1T=w1, whhT=whh, bias=bias, woutT=woutT,
                bout=bout, ohT=np.ascontiguousarray(oh), crf=crf_c, lmask=lm,
                ident=np.eye(128, dtype=bf16),
            )
        )
    return in_maps


_NC_CACHE = {}


def get_nc():
    if "nc" not in _NC_CACHE:
        _NC_CACHE["nc"] = build_nc()
    return _NC_CACHE["nc"]


def kernel(**inputs):
    from concourse.bass_utils import run_bass_kernel_spmd

    nc = get_nc()
    in_maps = stage_inputs(inputs)
    res = run_bass_kernel_spmd(nc, in_maps, list(range(NC)))
    total = np.float32(0.0)
    for r in res.results:
        total += np.float32(r["loss"].reshape(-1)[0])
    return np.asarray(total, dtype=np.float32)
